# revision 19
# baseline (speedup 1.0000x reference)
"""Causal multi-head attention on 8 Trainium2 NeuronCores.

Problem: nn_Attention_46643344835180
  x: [8, 1024, 768], 12 heads x 64 dh, causal softmax attention + output proj.

Sharding: data-parallel over batch (8 batch elements -> 8 cores, no collectives).

v3: full bf16 compute (PSUM stays f32), host-side transpose of x (xT fed
directly), weights resident in SBUF, dual DMA rings (sync + scalar HWDGE),
QK/V psum->sbuf copies on the Scalar engine (idle during the projection
phase), and a globally software-pipelined attention sweep: scores of group
g+1 are issued before exp/PV of group g, with projection/output chunks
spread between groups as PE filler.

Per-core dataflow (batch element b):
  xT = x_b.T (host)                                                  [768, 1024]
  QT = Wq.T @ xT  (+bq)            heads stacked on partitions       [768, 1024]
  KT = Wk.T @ xT  (+bk)                                              [768, 1024]
  V  = x_b @ Wv   (+bv)            + interleaved ones column         [1024, 12*66]
  per head h, query-chunk qc (512):
    S^T[k,q] = KT_h.T @ QT_h          keys on partitions
    P^T = exp(S^T / 8)                ScalarE, batched over 2 key-blocks
    causal: one 128-wide-mask multiply per diagonal block
    z^T[65,512] += [V_h | 1].T @ P^T  row 64 accumulates the denominator
    ZT_h = z^T[0:64] * approx(1/z^T[64])   (denom -> reciprocal ->
           gpsimd partition_broadcast -> multiply)
  out = ZT.T @ Wo (+bo)                                              [1024, 768]
"""

import sys

sys.path.insert(0, "/opt/trn_rl_repo")

from collections import deque

import ml_dtypes
import numpy as np

import concourse.bass as bass
import concourse.mybir as mybir
import concourse.tile as tile
from concourse import bacc
from concourse.bass_utils import run_bass_kernel_spmd

F32 = mybir.dt.float32
BF16 = mybir.dt.bfloat16
AF = mybir.ActivationFunctionType

SEQ = 1024
DM = 768
NH = 12
DH = 64
VH = DH + 2  # V head stride: 64 dims + ones col + pad (keeps 4B alignment)
BATCH = 8
NQT = SEQ // 128  # 8 seq tiles of 128
NDT = DM // 128  # 6 d_model tiles
QC = 512  # query chunk (moving dim)
NQC = SEQ // QC  # 2
WARMUP = 64  # HAM warmup matmuls (bf16 N=128, ~110ns each cold)


def build(with_bq, with_bk, with_bv, with_bo, debug_taps=False):
    nc = bacc.Bacc("TRN2", target_bir_lowering=False, debug=False)

    xt = nc.dram_tensor("xt", [DM, SEQ], BF16, kind="ExternalInput")
    wq = nc.dram_tensor("wq", [DM, DM], BF16, kind="ExternalInput")
    wk = nc.dram_tensor("wk", [DM, DM], BF16, kind="ExternalInput")
    wv = nc.dram_tensor("wv", [DM, DM], BF16, kind="ExternalInput")
    wo = nc.dram_tensor("wo", [DM, DM], BF16, kind="ExternalInput")
    wmask = nc.dram_tensor("wmask", [128, 128], BF16, kind="ExternalInput")
    identin = nc.dram_tensor("identin", [128, 128], BF16, kind="ExternalInput")
    bq = bk = bv = bo = None
    if with_bq:
        bq = nc.dram_tensor("bq", [128, NDT], F32, kind="ExternalInput")
    if with_bk:
        bk = nc.dram_tensor("bk", [128, NDT], F32, kind="ExternalInput")
    if with_bv:
        bv = nc.dram_tensor("bv", [1, DM], F32, kind="ExternalInput")
    if with_bo:
        bo = nc.dram_tensor("bo", [1, DM], F32, kind="ExternalInput")
    out = nc.dram_tensor("out", [SEQ, DM], F32, kind="ExternalOutput")
    taps = {}
    if debug_taps:
        for nm in ("QT_d", "KT_d", "ZT_d"):
            taps[nm] = nc.dram_tensor(nm, [DM, SEQ], BF16, kind="ExternalOutput")
        taps["V_d"] = nc.dram_tensor(
            "V_d", [SEQ, NH * VH], BF16, kind="ExternalOutput"
        )

    with tile.TileContext(nc) as tc:
        with (
            tc.tile_pool(name="persist", bufs=1) as persist,
            tc.tile_pool(name="pt", bufs=4) as pt_pool,
            tc.tile_pool(name="small", bufs=2) as small,
            tc.tile_pool(name="outst", bufs=2) as out_pool,
            tc.tile_pool(name="ps_st", bufs=2, space="PSUM") as ps_st,
            tc.tile_pool(name="ps_z", bufs=3, space="PSUM") as ps_z,
            tc.tile_pool(name="ps_mm", bufs=1, space="PSUM") as ps_mm,
        ):
            # ---- sync ring: ident, xT, WV, mask/ones.  scalar ring: wq/wk
            # (column-split so head pairs 0-1 unblock early), wo later ----
            ident = persist.tile([128, 128], BF16, tag="ident", name="ident")
            nc.sync.dma_start(out=ident, in_=identin[:, :])
            warm_ps = ps_mm.tile(
                [128, 128], F32, tag="proj", name="warm", padded_shape=[128, QC]
            )
            for _ in range(WARMUP):
                nc.tensor.matmul(warm_ps, lhsT=ident, rhs=ident, start=True, stop=True)

            # single sync ring for all inputs in priority order (the two
            # HWDGE rings share ~210GB/s of HBM read bandwidth, and DMAs on
            # the scalar ring block the ACT compute stream behind them)
            xT = [
                persist.tile([128, SEQ], BF16, tag=f"xT{d}", name=f"xT{d}")
                for d in range(NDT)
            ]
            for d in range(NDT):
                nc.sync.dma_start(out=xT[d], in_=xt[d * 128 : (d + 1) * 128, :])

            WQ = [
                persist.tile([128, DM], BF16, tag=f"WQ{d}", name=f"WQ{d}")
                for d in range(NDT)
            ]
            WK = [
                persist.tile([128, DM], BF16, tag=f"WK{d}", name=f"WK{d}")
                for d in range(NDT)
            ]
            WV = [
                persist.tile([128, DM], BF16, tag=f"WV{d}", name=f"WV{d}")
                for d in range(NDT)
            ]
            for src, dst in ((wq, WQ), (wk, WK)):
                for d in range(NDT):
                    nc.sync.dma_start(
                        out=dst[d][:, 0:256],
                        in_=src[d * 128 : (d + 1) * 128, 0:256],
                    )
            for d in range(NDT):
                nc.sync.dma_start(out=WV[d], in_=wv[d * 128 : (d + 1) * 128, :])
            for src, dst in ((wq, WQ), (wk, WK)):
                for d in range(NDT):
                    nc.sync.dma_start(
                        out=dst[d][:, 256:DM],
                        in_=src[d * 128 : (d + 1) * 128, 256:DM],
                    )

            wm_t = persist.tile([128, 128], BF16, tag="wmask", name="wmask")
            nc.sync.dma_start(out=wm_t, in_=wmask[:, :])

            bias_tiles = {}
            if with_bq:
                t = persist.tile([128, NDT], F32, tag="bq", name="bq")
                nc.scalar.dma_start(out=t, in_=bq[:, :])
                bias_tiles["bq"] = t
            if with_bk:
                t = persist.tile([128, NDT], F32, tag="bk", name="bk")
                nc.scalar.dma_start(out=t, in_=bk[:, :])
                bias_tiles["bk"] = t
            if with_bv:
                t = persist.tile([128, DM], F32, tag="bv", name="bv")
                nc.scalar.dma_start(out=t, in_=bv[0:1, :].to_broadcast((128, DM)))
                bias_tiles["bv"] = t
            if with_bo:
                t = persist.tile([128, DM], F32, tag="bo", name="bo")
                nc.scalar.dma_start(out=t, in_=bo[0:1, :].to_broadcast((128, DM)))
                bias_tiles["bo"] = t

            QT = [
                persist.tile([128, SEQ], BF16, tag=f"QT{d}", name=f"QT{d}")
                for d in range(NDT)
            ]
            KT = [
                persist.tile([128, SEQ], BF16, tag=f"KT{d}", name=f"KT{d}")
                for d in range(NDT)
            ]
            V = [
                persist.tile([128, NH * VH], BF16, tag=f"V{s}", name=f"V{s}")
                for s in range(NQT)
            ]
            for s in range(NQT):
                vv = V[s].rearrange("p (h e) -> p h e", e=VH)
                nc.vector.memset(vv[:, :, DH : DH + 1], 1.0)
            ZT = [
                persist.tile([128, SEQ], BF16, tag=f"ZT{d}", name=f"ZT{d}")
                for d in range(NDT)
            ]

            # ---- projection chunks (each = one psum round trip) ----
            def qk_chunk(hp, which, c, pool, tag):
                W, dst, bkey = (
                    (WQ, QT, "bq") if which == "q" else (WK, KT, "bk")
                )
                acc = pool.tile(
                    [128, QC], F32, tag=tag, name="proj",
                    padded_shape=[128, 2 * QC] if tag == "st" else [128, QC],
                )
                for d in range(NDT):
                    nc.tensor.matmul(
                        acc,
                        lhsT=W[d][:, hp * 128 : (hp + 1) * 128],
                        rhs=xT[d][:, c * QC : (c + 1) * QC],
                        start=(d == 0),
                        stop=(d == NDT - 1),
                    )
                o = dst[hp][:, c * QC : (c + 1) * QC]
                if bkey in bias_tiles:
                    nc.vector.tensor_scalar_add(
                        o, acc, bias_tiles[bkey][:, hp : hp + 1]
                    )
                else:
                    nc.vector.tensor_copy(o, acc)

            def qk_chunks(hp):
                return [
                    (lambda which=which, c=c: qk_chunk(hp, which, c, ps_mm, "proj"))
                    for which in ("q", "k")
                    for c in range(NQC)
                ]

            NVC = 2
            VC = DM // NVC  # 384

            def v_chunk(s, c, on_act=True):
                acc = ps_st.tile(
                    [128, VC], F32, tag="st", name="vacc",
                    padded_shape=[128, 2 * QC],
                )
                for d in range(NDT):
                    nc.tensor.matmul(
                        acc,
                        lhsT=xT[d][:, s * 128 : (s + 1) * 128],
                        rhs=WV[d][:, c * VC : (c + 1) * VC],
                        start=(d == 0),
                        stop=(d == NDT - 1),
                    )
                nh2 = VC // DH  # heads per chunk (6)
                o = V[s].rearrange("p (h e) -> p h e", e=VH)[
                    :, c * nh2 : (c + 1) * nh2, 0:DH
                ]
                if "bv" in bias_tiles:
                    nc.vector.tensor_add(
                        o,
                        acc.rearrange("p (h e) -> p h e", e=DH),
                        bias_tiles["bv"][:, c * VC : (c + 1) * VC].rearrange(
                            "p (h e) -> p h e", e=DH
                        ),
                    )
                elif on_act:
                    nc.scalar.activation(
                        o, acc.rearrange("p (h e) -> p h e", e=DH), AF.Copy
                    )
                else:
                    nc.vector.tensor_copy(
                        o, acc.rearrange("p (h e) -> p h e", e=DH)
                    )

            def v_chunks(s):
                return [lambda c=c: v_chunk(s, c) for c in range(NVC)]

            WO = []

            def wo_load():
                # sync ring: idle mid-attention (scalar ring would block ACT)
                for d in range(NDT):
                    t = persist.tile([128, DM], BF16, tag=f"WO{d}", name=f"WO{d}")
                    nc.sync.dma_start(out=t, in_=wo[d * 128 : (d + 1) * 128, :])
                    WO.append(t)

            def o_chunks(s):
                ot = [None]

                def chunk(c):
                    if c == 0:
                        ot[0] = out_pool.tile([128, DM], F32, tag="ostage", name="ostage")
                    pool, tag = ((ps_mm, "proj"), (ps_z, "z"))[c % 2]
                    acc = pool.tile(
                        [128, VC], F32, tag=tag, name="oacc",
                        padded_shape=[128, QC],
                    )
                    for d in range(NDT):
                        nc.tensor.matmul(
                            acc,
                            lhsT=ZT[d][:, s * 128 : (s + 1) * 128],
                            rhs=WO[d][:, c * VC : (c + 1) * VC],
                            start=(d == 0),
                            stop=(d == NDT - 1),
                        )
                    o = ot[0][:, c * VC : (c + 1) * VC]
                    if "bo" in bias_tiles:
                        nc.vector.tensor_add(
                            o, acc, bias_tiles["bo"][:, c * VC : (c + 1) * VC]
                        )
                    else:
                        nc.vector.tensor_copy(o, acc)
                    if c == NVC - 1:
                        # rows 512+ drain at the very end: use the scalar
                        # ring (idle by then) so the tail DMAs overlap
                        eng = nc.sync if s < 4 else nc.scalar
                        eng.dma_start(
                            out=out[s * 128 : (s + 1) * 128, :], in_=ot[0]
                        )

                return [lambda c=c: chunk(c) for c in range(NVC)]

            # ---- pipelined attention sweep ----
            zps_of = {}

            def issue_scores(hp, c, g, gsz):
                doffs = [max(0, (g + j) * 128 - c * QC) for j in range(gsz)]
                sts = {}
                for px in (0, 64):
                    sts[px] = ps_st.tile([128, gsz * QC], F32, tag="st", name="st")
                for j in range(gsz):
                    kb = g + j
                    off = doffs[j]
                    for px in (0, 64):
                        nc.tensor.matmul(
                            sts[px][:, j * QC + off : (j + 1) * QC],
                            lhsT=KT[hp][px : px + 64, kb * 128 : (kb + 1) * 128],
                            rhs=QT[hp][px : px + 64, c * QC + off : (c + 1) * QC],
                            start=True,
                            stop=True,
                        )
                return sts, doffs

            def issue_expv(hp, c, g, gsz, sts, doffs, last):
                nkb = 4 * (c + 1)
                if g == 0:
                    zps_of[(hp, c)] = {
                        px: ps_z.tile([128, QC], F32, tag="z", name="z")
                        for px in (0, 64)
                    }
                zps = zps_of[(hp, c)]
                pts = {}
                for px in (0, 64):
                    pt = pt_pool.tile([128, 2 * QC], BF16, tag="pt", name="pt")
                    nc.scalar.activation(
                        pt[:, doffs[0] : gsz * QC],
                        sts[px][:, doffs[0] : gsz * QC],
                        AF.Exp,
                        scale=0.125,
                    )
                    pts[px] = pt
                for j in range(gsz):
                    kb = g + j
                    doff = kb * 128 - c * QC
                    off = doffs[j]
                    for px in (0, 64):
                        pt = pts[px]
                        if 0 <= doff < QC:  # diagonal block: 128-wide triangle
                            blk = pt[:, j * QC + doff : j * QC + doff + 128]
                            nc.vector.tensor_mul(blk, blk, wm_t)
                        h = 2 * hp + (1 if px else 0)
                        nc.tensor.matmul(
                            zps[px][0 : DH + 1, off:QC],
                            lhsT=V[kb][:, h * VH : h * VH + DH + 1],
                            rhs=pt[:, j * QC + off : (j + 1) * QC],
                            start=(kb == 0),
                            stop=(kb == nkb - 1),
                        )
                if last:
                    for px in (0, 64):
                        dstage = small.tile([128, QC], F32, tag="dstage", name="dstage")
                        nc.vector.tensor_copy(dstage[0:1, :], zps[px][DH : DH + 1, :])
                        recip = small.tile([128, QC], F32, tag="recip", name="recip")
                        nc.vector.reciprocal_approx_fast(recip[0:1, :], dstage[0:1, :])
                        bcast = small.tile([64, QC], F32, tag="bcast", name="bcast")
                        nc.gpsimd.partition_broadcast(bcast, recip[0:1, :])
                        nc.vector.tensor_mul(
                            ZT[hp][px : px + 64, c * QC : (c + 1) * QC],
                            zps[px][0:64, :],
                            bcast,
                        )
                    del zps_of[(hp, c)]

            # ---- pre-phase: warmup already issued; project heads 0-1 (hp 0)
            # alternating psum rings, then V tiles 0-1 ----
            for i, (which, c) in enumerate(
                (w, c) for w in ("q", "k") for c in range(NQC)
            ):
                pool, tag = ((ps_mm, "proj"), (ps_st, "st"))[i % 2]
                qk_chunk(0, which, c, pool, tag)
            for s in (0, 1):
                for c in range(NVC):
                    # DVE copies: the ACT stream would stall behind the WV
                    # DMA and delay the first exps queued after it
                    v_chunk(s, c, on_act=False)

            # qc=0 and qc=1 units interleaved: spreads the exp-heavy qc=1
            # units (ACT-bound) across the whole span instead of
            # back-loading them
            units = [
                (0, 0), (1, 0), (0, 1), (2, 0), (1, 1), (3, 0),
                (2, 1), (4, 0), (3, 1), (5, 0), (4, 1), (5, 1),
            ]
            fillers = {
                0: qk_chunks(1) + v_chunks(2) + v_chunks(3),
                1: qk_chunks(2) + v_chunks(4),
                2: v_chunks(5) + v_chunks(6) + v_chunks(7),
                3: qk_chunks(3),
                4: [wo_load],
                5: qk_chunks(4),
                7: qk_chunks(5),
                10: o_chunks(0) + o_chunks(1),
                11: o_chunks(2) + o_chunks(3),
            }
            # units whose fillers read ZT written by the pending finalizer:
            # flush before popping fillers there (issue-order correctness)
            flush_first = {10, 11}

            pending = [None]

            def flush():
                if pending[0] is not None:
                    fn = pending[0]
                    pending[0] = None
                    fn()

            for ui, (hp, c) in enumerate(units):
                nkb = 4 * (c + 1)
                glist = [(g, min(2, nkb - g)) for g in range(0, nkb, 2)]
                chunks = deque(fillers.get(ui, []))
                n = len(glist)
                for gi, (g, gsz) in enumerate(glist):
                    sts, doffs = issue_scores(hp, c, g, gsz)
                    if ui in flush_first:
                        flush()
                    k = -(-len(chunks) // (n - gi)) if chunks else 0
                    for i in range(k):
                        chunks.popleft()()
                        if i == 0:
                            flush()
                    if k == 0:
                        flush()
                    pending[0] = (
                        lambda hp=hp, c=c, g=g, gsz=gsz, sts=sts, doffs=doffs,
                        last=(gi == n - 1): issue_expv(hp, c, g, gsz, sts, doffs, last)
                    )
            flush()

            # ---- tail: output rows 512-1024 ----
            for s in range(4, NQT):
                for f in o_chunks(s):
                    f()

            if debug_taps:
                for nm, tiles in (("QT_d", QT), ("KT_d", KT), ("ZT_d", ZT)):
                    for d in range(NDT):
                        nc.sync.dma_start(
                            out=taps[nm][d * 128 : (d + 1) * 128, :],
                            in_=tiles[d][:, :],
                        )
                for s in range(NQT):
                    nc.sync.dma_start(
                        out=taps["V_d"][s * 128 : (s + 1) * 128, :], in_=V[s][:, :]
                    )

    nc.compile()
    return nc


_CACHE = {}


def _get_nc(key):
    if key not in _CACHE:
        _CACHE[key] = build(*key)
    return _CACHE[key]


def _prep(inputs):
    BF = ml_dtypes.bfloat16
    x = np.asarray(inputs["normalized_resid_pre"], np.float32)
    wq = np.ascontiguousarray(
        np.asarray(inputs["W_Q"], np.float32).transpose(1, 0, 2).reshape(DM, DM)
    ).astype(BF)
    wk = np.ascontiguousarray(
        np.asarray(inputs["W_K"], np.float32).transpose(1, 0, 2).reshape(DM, DM)
    ).astype(BF)
    wv = np.ascontiguousarray(
        np.asarray(inputs["W_V"], np.float32).transpose(1, 0, 2).reshape(DM, DM)
    ).astype(BF)
    wo = np.ascontiguousarray(
        np.asarray(inputs["W_O"], np.float32).reshape(DM, DM)
    ).astype(BF)
    bq = np.asarray(inputs["b_Q"], np.float32).reshape(NDT, 128).T
    bk = np.asarray(inputs["b_K"], np.float32).reshape(NDT, 128).T
    bv = np.asarray(inputs["b_V"], np.float32).reshape(1, DM)
    bo = np.asarray(inputs["b_O"], np.float32).reshape(1, DM)
    jj, uu = np.meshgrid(np.arange(128), np.arange(128), indexing="ij")
    wmask = (uu >= jj).astype(BF)
    key = (
        bool(np.any(bq)),
        bool(np.any(bk)),
        bool(np.any(bv)),
        bool(np.any(bo)),
    )
    common = {
        "wq": wq, "wk": wk, "wv": wv, "wo": wo, "wmask": wmask,
        "identin": np.eye(128, dtype=np.float32).astype(BF),
    }
    if key[0]:
        common["bq"] = np.ascontiguousarray(bq)
    if key[1]:
        common["bk"] = np.ascontiguousarray(bk)
    if key[2]:
        common["bv"] = np.ascontiguousarray(bv)
    if key[3]:
        common["bo"] = np.ascontiguousarray(bo)
    in_maps = [
        dict(common, xt=np.ascontiguousarray(x[b].T).astype(BF))
        for b in range(BATCH)
    ]
    return key, in_maps


def run(inputs, trace=False, **kw):
    key, in_maps = _prep(inputs)
    nc = _get_nc(key)
    res = run_bass_kernel_spmd(
        nc, in_maps, core_ids=list(range(BATCH)), trace=trace, **kw
    )
    outs = np.stack([res.results[b]["out"] for b in range(BATCH)])
    return outs.astype(np.float32), res


def kernel(**inputs):
    out, _ = run(inputs)
    return out


if __name__ == "__main__":
    rng = np.random.default_rng(0)
    ins = {
        "normalized_resid_pre": rng.standard_normal((8, SEQ, DM)).astype(np.float32),
        "W_Q": (0.02 * rng.standard_normal((NH, DM, DH))).astype(np.float32),
        "b_Q": np.zeros((NH, DH), np.float32),
        "W_K": (0.02 * rng.standard_normal((NH, DM, DH))).astype(np.float32),
        "b_K": np.zeros((NH, DH), np.float32),
        "W_V": (0.02 * rng.standard_normal((NH, DM, DH))).astype(np.float32),
        "b_V": np.zeros((NH, DH), np.float32),
        "W_O": (0.02 * rng.standard_normal((NH, DH, DM))).astype(np.float32),
        "b_O": np.zeros((DM,), np.float32),
    }
    out = kernel(**ins)
    print("kernel output", out.shape, out.dtype, float(np.abs(out).max()))


# revision 21
# speedup vs baseline: 1.1276x; 1.1276x over previous
"""Causal multi-head attention on 8 Trainium2 NeuronCores.

Problem: nn_Attention_46643344835180
  x: [8, 1024, 768], 12 heads x 64 dh, causal softmax attention + output proj.

Sharding: data-parallel over batch (8 batch elements -> 8 cores, no collectives).

v3: full bf16 compute (PSUM stays f32), host-side transpose of x (xT fed
directly), weights resident in SBUF, dual DMA rings (sync + scalar HWDGE),
QK/V psum->sbuf copies on the Scalar engine (idle during the projection
phase), and a globally software-pipelined attention sweep: scores of group
g+1 are issued before exp/PV of group g, with projection/output chunks
spread between groups as PE filler.

Per-core dataflow (batch element b):
  xT = x_b.T (host)                                                  [768, 1024]
  QT = Wq.T @ xT  (+bq)            heads stacked on partitions       [768, 1024]
  KT = Wk.T @ xT  (+bk)                                              [768, 1024]
  V  = x_b @ Wv   (+bv)            + interleaved ones column         [1024, 12*66]
  per head h, query-chunk qc (512):
    S^T[k,q] = KT_h.T @ QT_h          keys on partitions
    P^T = exp(S^T / 8)                ScalarE, batched over 2 key-blocks
    causal: one 128-wide-mask multiply per diagonal block
    z^T[65,512] += [V_h | 1].T @ P^T  row 64 accumulates the denominator
    ZT_h = z^T[0:64] * approx(1/z^T[64])   (denom -> reciprocal ->
           gpsimd partition_broadcast -> multiply)
  out = ZT.T @ Wo (+bo)                                              [1024, 768]
"""

import sys

sys.path.insert(0, "/opt/trn_rl_repo")

from collections import deque

import ml_dtypes
import numpy as np

import concourse.bass as bass
import concourse.mybir as mybir
import concourse.tile as tile
from concourse import bacc
from concourse.bass_utils import run_bass_kernel_spmd

F32 = mybir.dt.float32
BF16 = mybir.dt.bfloat16
AF = mybir.ActivationFunctionType

SEQ = 1024
DM = 768
NH = 12
DH = 64
VH = DH + 2  # V head stride: 64 dims + ones col + pad (keeps 4B alignment)
BATCH = 8
NQT = SEQ // 128  # 8 seq tiles of 128
NDT = DM // 128  # 6 d_model tiles
QC = 512  # query chunk (moving dim)
NQC = SEQ // QC  # 2
WARMUP = 64  # HAM warmup matmuls (bf16 N=128, ~110ns each cold)


def build(with_bq, with_bk, with_bv, with_bo, debug_taps=False):
    nc = bacc.Bacc("TRN2", target_bir_lowering=False, debug=False)

    xt = nc.dram_tensor("xt", [DM, SEQ], BF16, kind="ExternalInput")
    wq = nc.dram_tensor("wq", [DM, DM], BF16, kind="ExternalInput")
    wk = nc.dram_tensor("wk", [DM, DM], BF16, kind="ExternalInput")
    wv = nc.dram_tensor("wv", [DM, DM], BF16, kind="ExternalInput")
    wo = nc.dram_tensor("wo", [DM, DM], BF16, kind="ExternalInput")
    wmask = nc.dram_tensor("wmask", [128, 128], BF16, kind="ExternalInput")
    identin = nc.dram_tensor("identin", [128, 128], BF16, kind="ExternalInput")
    bq = bk = bv = bo = None
    if with_bq:
        bq = nc.dram_tensor("bq", [128, NDT], F32, kind="ExternalInput")
    if with_bk:
        bk = nc.dram_tensor("bk", [128, NDT], F32, kind="ExternalInput")
    if with_bv:
        bv = nc.dram_tensor("bv", [1, DM], F32, kind="ExternalInput")
    if with_bo:
        bo = nc.dram_tensor("bo", [1, DM], F32, kind="ExternalInput")
    out = nc.dram_tensor("out", [SEQ, DM], F32, kind="ExternalOutput")
    taps = {}
    if debug_taps:
        for nm in ("QT_d", "KT_d", "ZT_d"):
            taps[nm] = nc.dram_tensor(nm, [DM, SEQ], BF16, kind="ExternalOutput")
        taps["V_d"] = nc.dram_tensor(
            "V_d", [SEQ, NH * VH], BF16, kind="ExternalOutput"
        )

    with tile.TileContext(nc) as tc:
        with (
            tc.tile_pool(name="persist", bufs=1) as persist,
            tc.tile_pool(name="pt", bufs=4) as pt_pool,
            tc.tile_pool(name="small", bufs=2) as small,
            tc.tile_pool(name="outst", bufs=2) as out_pool,
            tc.tile_pool(name="ps_st", bufs=2, space="PSUM") as ps_st,
            tc.tile_pool(name="ps_z", bufs=3, space="PSUM") as ps_z,
            tc.tile_pool(name="ps_mm", bufs=1, space="PSUM") as ps_mm,
        ):
            # ---- sync ring: ident, xT, WV, mask/ones.  scalar ring: wq/wk
            # (column-split so head pairs 0-1 unblock early), wo later ----
            ident = persist.tile([128, 128], BF16, tag="ident", name="ident")
            nc.sync.dma_start(out=ident, in_=identin[:, :])
            warm_ps = ps_mm.tile(
                [128, 128], F32, tag="proj", name="warm", padded_shape=[128, QC]
            )
            for _ in range(WARMUP):
                nc.tensor.matmul(warm_ps, lhsT=ident, rhs=ident, start=True, stop=True)

            # single sync ring for all inputs in priority order (the two
            # HWDGE rings share ~210GB/s of HBM read bandwidth, and DMAs on
            # the scalar ring block the ACT compute stream behind them)
            xT = [
                persist.tile([128, SEQ], BF16, tag=f"xT{d}", name=f"xT{d}")
                for d in range(NDT)
            ]
            for d in range(NDT):
                nc.sync.dma_start(out=xT[d], in_=xt[d * 128 : (d + 1) * 128, :])

            WQ = [
                persist.tile([128, DM], BF16, tag=f"WQ{d}", name=f"WQ{d}")
                for d in range(NDT)
            ]
            WK = [
                persist.tile([128, DM], BF16, tag=f"WK{d}", name=f"WK{d}")
                for d in range(NDT)
            ]
            WV = [
                persist.tile([128, DM], BF16, tag=f"WV{d}", name=f"WV{d}")
                for d in range(NDT)
            ]
            for src, dst in ((wq, WQ), (wk, WK)):
                for d in range(NDT):
                    nc.sync.dma_start(
                        out=dst[d][:, 0:256],
                        in_=src[d * 128 : (d + 1) * 128, 0:256],
                    )
            for d in range(NDT):
                nc.sync.dma_start(out=WV[d], in_=wv[d * 128 : (d + 1) * 128, :])
            for src, dst in ((wq, WQ), (wk, WK)):
                for d in range(NDT):
                    nc.sync.dma_start(
                        out=dst[d][:, 256:DM],
                        in_=src[d * 128 : (d + 1) * 128, 256:DM],
                    )

            wm_t = persist.tile([128, 128], BF16, tag="wmask", name="wmask")
            nc.sync.dma_start(out=wm_t, in_=wmask[:, :])

            bias_tiles = {}
            if with_bq:
                t = persist.tile([128, NDT], F32, tag="bq", name="bq")
                nc.scalar.dma_start(out=t, in_=bq[:, :])
                bias_tiles["bq"] = t
            if with_bk:
                t = persist.tile([128, NDT], F32, tag="bk", name="bk")
                nc.scalar.dma_start(out=t, in_=bk[:, :])
                bias_tiles["bk"] = t
            if with_bv:
                t = persist.tile([128, DM], F32, tag="bv", name="bv")
                nc.scalar.dma_start(out=t, in_=bv[0:1, :].to_broadcast((128, DM)))
                bias_tiles["bv"] = t
            if with_bo:
                t = persist.tile([128, DM], F32, tag="bo", name="bo")
                nc.scalar.dma_start(out=t, in_=bo[0:1, :].to_broadcast((128, DM)))
                bias_tiles["bo"] = t

            QT = [
                persist.tile([128, SEQ], BF16, tag=f"QT{d}", name=f"QT{d}")
                for d in range(NDT)
            ]
            KT = [
                persist.tile([128, SEQ], BF16, tag=f"KT{d}", name=f"KT{d}")
                for d in range(NDT)
            ]
            V = [
                persist.tile([128, NH * VH], BF16, tag=f"V{s}", name=f"V{s}")
                for s in range(NQT)
            ]
            for s in range(NQT):
                vv = V[s].rearrange("p (h e) -> p h e", e=VH)
                nc.vector.memset(vv[:, :, DH : DH + 1], 1.0)
            ZT = [
                persist.tile([128, SEQ], BF16, tag=f"ZT{d}", name=f"ZT{d}")
                for d in range(NDT)
            ]

            # ---- projection chunks (each = one psum round trip) ----
            def qk_chunk(hp, which, c, pool, tag):
                W, dst, bkey = (
                    (WQ, QT, "bq") if which == "q" else (WK, KT, "bk")
                )
                acc = pool.tile(
                    [128, QC], F32, tag=tag, name="proj",
                    padded_shape=[128, 2 * QC] if tag == "st" else [128, QC],
                )
                for d in range(NDT):
                    nc.tensor.matmul(
                        acc,
                        lhsT=W[d][:, hp * 128 : (hp + 1) * 128],
                        rhs=xT[d][:, c * QC : (c + 1) * QC],
                        start=(d == 0),
                        stop=(d == NDT - 1),
                    )
                o = dst[hp][:, c * QC : (c + 1) * QC]
                if bkey in bias_tiles:
                    nc.vector.tensor_scalar_add(
                        o, acc, bias_tiles[bkey][:, hp : hp + 1]
                    )
                else:
                    nc.vector.tensor_copy(o, acc)

            def qk_chunks(hp):
                return [
                    (lambda which=which, c=c: qk_chunk(hp, which, c, ps_mm, "proj"))
                    for which in ("q", "k")
                    for c in range(NQC)
                ]

            NVC = 2
            VC = DM // NVC  # 384

            def v_chunk(s, c, on_act=True):
                acc = ps_st.tile(
                    [128, VC], F32, tag="st", name="vacc",
                    padded_shape=[128, 2 * QC],
                )
                for d in range(NDT):
                    nc.tensor.matmul(
                        acc,
                        lhsT=xT[d][:, s * 128 : (s + 1) * 128],
                        rhs=WV[d][:, c * VC : (c + 1) * VC],
                        start=(d == 0),
                        stop=(d == NDT - 1),
                    )
                nh2 = VC // DH  # heads per chunk (6)
                o = V[s].rearrange("p (h e) -> p h e", e=VH)[
                    :, c * nh2 : (c + 1) * nh2, 0:DH
                ]
                if "bv" in bias_tiles:
                    nc.vector.tensor_add(
                        o,
                        acc.rearrange("p (h e) -> p h e", e=DH),
                        bias_tiles["bv"][:, c * VC : (c + 1) * VC].rearrange(
                            "p (h e) -> p h e", e=DH
                        ),
                    )
                elif on_act:
                    nc.scalar.activation(
                        o, acc.rearrange("p (h e) -> p h e", e=DH), AF.Copy
                    )
                else:
                    nc.vector.tensor_copy(
                        o, acc.rearrange("p (h e) -> p h e", e=DH)
                    )

            def v_chunks(s):
                return [lambda c=c: v_chunk(s, c) for c in range(NVC)]

            WO = []

            def wo_load():
                # sync ring: idle mid-attention (scalar ring would block ACT)
                for d in range(NDT):
                    t = persist.tile([128, DM], BF16, tag=f"WO{d}", name=f"WO{d}")
                    nc.sync.dma_start(out=t, in_=wo[d * 128 : (d + 1) * 128, :])
                    WO.append(t)

            def o_chunks(s):
                ot = [None]

                def chunk(c):
                    if c == 0:
                        ot[0] = out_pool.tile([128, DM], F32, tag="ostage", name="ostage")
                    pool, tag = ((ps_mm, "proj"), (ps_z, "z"))[c % 2]
                    acc = pool.tile(
                        [128, VC], F32, tag=tag, name="oacc",
                        padded_shape=[128, QC],
                    )
                    for d in range(NDT):
                        nc.tensor.matmul(
                            acc,
                            lhsT=ZT[d][:, s * 128 : (s + 1) * 128],
                            rhs=WO[d][:, c * VC : (c + 1) * VC],
                            start=(d == 0),
                            stop=(d == NDT - 1),
                        )
                    o = ot[0][:, c * VC : (c + 1) * VC]
                    if "bo" in bias_tiles:
                        nc.vector.tensor_add(
                            o, acc, bias_tiles["bo"][:, c * VC : (c + 1) * VC]
                        )
                    else:
                        # ACT: idle at the tail; keeps the psum recycle off
                        # the DVE fin-chain backlog
                        nc.scalar.activation(o, acc, AF.Copy)
                    if c == NVC - 1:
                        # rows 512+ drain at the very end: use the scalar
                        # ring (idle by then) so the tail DMAs overlap
                        eng = nc.sync if s < 4 else nc.scalar
                        eng.dma_start(
                            out=out[s * 128 : (s + 1) * 128, :], in_=ot[0]
                        )

                return [lambda c=c: chunk(c) for c in range(NVC)]

            # ---- pipelined attention sweep ----
            zps_of = {}

            def issue_scores(hp, c, g, gsz):
                doffs = [max(0, (g + j) * 128 - c * QC) for j in range(gsz)]
                sts = {}
                for px in (0, 64):
                    sts[px] = ps_st.tile([128, gsz * QC], F32, tag="st", name="st")
                for j in range(gsz):
                    kb = g + j
                    off = doffs[j]
                    for px in (0, 64):
                        nc.tensor.matmul(
                            sts[px][:, j * QC + off : (j + 1) * QC],
                            lhsT=KT[hp][px : px + 64, kb * 128 : (kb + 1) * 128],
                            rhs=QT[hp][px : px + 64, c * QC + off : (c + 1) * QC],
                            start=True,
                            stop=True,
                        )
                return sts, doffs

            def issue_expv(hp, c, g, gsz, sts, doffs, last):
                nkb = 4 * (c + 1)
                if g == 0:
                    zps_of[(hp, c)] = {
                        px: ps_z.tile([128, QC], F32, tag="z", name="z")
                        for px in (0, 64)
                    }
                zps = zps_of[(hp, c)]
                pts = {}
                for px in (0, 64):
                    pt = pt_pool.tile([128, 2 * QC], BF16, tag="pt", name="pt")
                    nc.scalar.activation(
                        pt[:, doffs[0] : gsz * QC],
                        sts[px][:, doffs[0] : gsz * QC],
                        AF.Exp,
                        scale=0.125,
                    )
                    pts[px] = pt
                for j in range(gsz):
                    kb = g + j
                    doff = kb * 128 - c * QC
                    off = doffs[j]
                    for px in (0, 64):
                        pt = pts[px]
                        if 0 <= doff < QC:  # diagonal block: 128-wide triangle
                            blk = pt[:, j * QC + doff : j * QC + doff + 128]
                            nc.vector.tensor_mul(blk, blk, wm_t)
                        h = 2 * hp + (1 if px else 0)
                        nc.tensor.matmul(
                            zps[px][0 : DH + 1, off:QC],
                            lhsT=V[kb][:, h * VH : h * VH + DH + 1],
                            rhs=pt[:, j * QC + off : (j + 1) * QC],
                            start=(kb == 0),
                            stop=(kb == nkb - 1),
                        )
                if last:
                    for px in (0, 64):
                        dstage = small.tile([128, QC], F32, tag="dstage", name="dstage")
                        nc.vector.tensor_copy(dstage[0:1, :], zps[px][DH : DH + 1, :])
                        recip = small.tile([128, QC], F32, tag="recip", name="recip")
                        nc.vector.reciprocal_approx_fast(recip[0:1, :], dstage[0:1, :])
                        bcast = small.tile([64, QC], F32, tag="bcast", name="bcast")
                        nc.gpsimd.partition_broadcast(bcast, recip[0:1, :])
                        nc.vector.tensor_mul(
                            ZT[hp][px : px + 64, c * QC : (c + 1) * QC],
                            zps[px][0:64, :],
                            bcast,
                        )
                    del zps_of[(hp, c)]

            # ---- pre-phase: warmup already issued; project heads 0-1 (hp 0)
            # alternating psum rings, then V tiles 0-1 ----
            for i, (which, c) in enumerate(
                (w, c) for w in ("q", "k") for c in range(NQC)
            ):
                pool, tag = ((ps_mm, "proj"), (ps_st, "st"))[i % 2]
                qk_chunk(0, which, c, pool, tag)
            for s in (0, 1):
                for c in range(NVC):
                    # DVE copies: the ACT stream would stall behind the WV
                    # DMA and delay the first exps queued after it
                    v_chunk(s, c, on_act=False)

            # qc=0 and qc=1 units interleaved: spreads the exp-heavy qc=1
            # units (ACT-bound) across the whole span instead of
            # back-loading them.  Every unit gets filler chunks so exp
            # latency is always hidden behind interposed PE work.
            units = [
                (0, 0), (1, 0), (0, 1), (2, 0), (1, 1), (3, 0),
                (2, 1), (4, 0), (5, 0), (3, 1), (4, 1), (5, 1),
            ]

            def vc(s, c):
                return lambda: v_chunk(s, c)

            qk1 = qk_chunks(1)
            fillers = {
                0: qk1[0:2] + [vc(2, 0)] + qk1[2:4] + [vc(3, 0)],
                1: qk_chunks(2),
                2: [vc(4, 0), vc(5, 0), vc(6, 0), vc(7, 0)],
                3: qk_chunks(3),
                4: [vc(2, 1), vc(3, 1)],
                5: qk_chunks(4),
                6: [vc(4, 1), vc(5, 1), vc(6, 1)],
                7: qk_chunks(5),
                8: [vc(7, 1), wo_load],
                9: o_chunks(0),
                10: o_chunks(1) + o_chunks(2),
                11: o_chunks(3),
            }
            # units whose fillers read ZT written by the pending finalizer:
            # flush before popping fillers there (issue-order correctness)
            flush_first = {9, 10, 11}

            pending = [None]

            def flush():
                if pending[0] is not None:
                    fn = pending[0]
                    pending[0] = None
                    fn()

            for ui, (hp, c) in enumerate(units):
                nkb = 4 * (c + 1)
                glist = [(g, min(2, nkb - g)) for g in range(0, nkb, 2)]
                chunks = deque(fillers.get(ui, []))
                n = len(glist)
                for gi, (g, gsz) in enumerate(glist):
                    sts, doffs = issue_scores(hp, c, g, gsz)
                    if ui in flush_first:
                        flush()
                    k = -(-len(chunks) // (n - gi)) if chunks else 0
                    for i in range(k):
                        chunks.popleft()()
                        if i == 0:
                            flush()
                    if k == 0:
                        flush()
                    pending[0] = (
                        lambda hp=hp, c=c, g=g, gsz=gsz, sts=sts, doffs=doffs,
                        last=(gi == n - 1): issue_expv(hp, c, g, gsz, sts, doffs, last)
                    )
            flush()

            # ---- tail: output rows 512-1024 ----
            for s in range(4, NQT):
                for f in o_chunks(s):
                    f()

            if debug_taps:
                for nm, tiles in (("QT_d", QT), ("KT_d", KT), ("ZT_d", ZT)):
                    for d in range(NDT):
                        nc.sync.dma_start(
                            out=taps[nm][d * 128 : (d + 1) * 128, :],
                            in_=tiles[d][:, :],
                        )
                for s in range(NQT):
                    nc.sync.dma_start(
                        out=taps["V_d"][s * 128 : (s + 1) * 128, :], in_=V[s][:, :]
                    )

    nc.compile()
    return nc


_CACHE = {}


def _get_nc(key):
    if key not in _CACHE:
        _CACHE[key] = build(*key)
    return _CACHE[key]


def _prep(inputs):
    BF = ml_dtypes.bfloat16
    x = np.asarray(inputs["normalized_resid_pre"], np.float32)
    wq = np.ascontiguousarray(
        np.asarray(inputs["W_Q"], np.float32).transpose(1, 0, 2).reshape(DM, DM)
    ).astype(BF)
    wk = np.ascontiguousarray(
        np.asarray(inputs["W_K"], np.float32).transpose(1, 0, 2).reshape(DM, DM)
    ).astype(BF)
    wv = np.ascontiguousarray(
        np.asarray(inputs["W_V"], np.float32).transpose(1, 0, 2).reshape(DM, DM)
    ).astype(BF)
    wo = np.ascontiguousarray(
        np.asarray(inputs["W_O"], np.float32).reshape(DM, DM)
    ).astype(BF)
    bq = np.asarray(inputs["b_Q"], np.float32).reshape(NDT, 128).T
    bk = np.asarray(inputs["b_K"], np.float32).reshape(NDT, 128).T
    bv = np.asarray(inputs["b_V"], np.float32).reshape(1, DM)
    bo = np.asarray(inputs["b_O"], np.float32).reshape(1, DM)
    jj, uu = np.meshgrid(np.arange(128), np.arange(128), indexing="ij")
    wmask = (uu >= jj).astype(BF)
    key = (
        bool(np.any(bq)),
        bool(np.any(bk)),
        bool(np.any(bv)),
        bool(np.any(bo)),
    )
    common = {
        "wq": wq, "wk": wk, "wv": wv, "wo": wo, "wmask": wmask,
        "identin": np.eye(128, dtype=np.float32).astype(BF),
    }
    if key[0]:
        common["bq"] = np.ascontiguousarray(bq)
    if key[1]:
        common["bk"] = np.ascontiguousarray(bk)
    if key[2]:
        common["bv"] = np.ascontiguousarray(bv)
    if key[3]:
        common["bo"] = np.ascontiguousarray(bo)
    in_maps = [
        dict(common, xt=np.ascontiguousarray(x[b].T).astype(BF))
        for b in range(BATCH)
    ]
    return key, in_maps


def run(inputs, trace=False, **kw):
    key, in_maps = _prep(inputs)
    nc = _get_nc(key)
    res = run_bass_kernel_spmd(
        nc, in_maps, core_ids=list(range(BATCH)), trace=trace, **kw
    )
    outs = np.stack([res.results[b]["out"] for b in range(BATCH)])
    return outs.astype(np.float32), res


def kernel(**inputs):
    out, _ = run(inputs)
    return out


if __name__ == "__main__":
    rng = np.random.default_rng(0)
    ins = {
        "normalized_resid_pre": rng.standard_normal((8, SEQ, DM)).astype(np.float32),
        "W_Q": (0.02 * rng.standard_normal((NH, DM, DH))).astype(np.float32),
        "b_Q": np.zeros((NH, DH), np.float32),
        "W_K": (0.02 * rng.standard_normal((NH, DM, DH))).astype(np.float32),
        "b_K": np.zeros((NH, DH), np.float32),
        "W_V": (0.02 * rng.standard_normal((NH, DM, DH))).astype(np.float32),
        "b_V": np.zeros((NH, DH), np.float32),
        "W_O": (0.02 * rng.standard_normal((NH, DH, DM))).astype(np.float32),
        "b_O": np.zeros((DM,), np.float32),
    }
    out = kernel(**ins)
    print("kernel output", out.shape, out.dtype, float(np.abs(out).max()))


# revision 26
# speedup vs baseline: 1.1313x; 1.0033x over previous
"""Causal multi-head attention on 8 Trainium2 NeuronCores.

Problem: nn_Attention_46643344835180
  x: [8, 1024, 768], 12 heads x 64 dh, causal softmax attention + output proj.

Sharding: data-parallel over batch (8 batch elements -> 8 cores, no collectives).

v3: full bf16 compute (PSUM stays f32), host-side transpose of x (xT fed
directly), weights resident in SBUF, dual DMA rings (sync + scalar HWDGE),
QK/V psum->sbuf copies on the Scalar engine (idle during the projection
phase), and a globally software-pipelined attention sweep: scores of group
g+1 are issued before exp/PV of group g, with projection/output chunks
spread between groups as PE filler.

Per-core dataflow (batch element b):
  xT = x_b.T (host)                                                  [768, 1024]
  QT = Wq.T @ xT  (+bq)            heads stacked on partitions       [768, 1024]
  KT = Wk.T @ xT  (+bk)                                              [768, 1024]
  V  = x_b @ Wv   (+bv)            + interleaved ones column         [1024, 12*66]
  per head h, query-chunk qc (512):
    S^T[k,q] = KT_h.T @ QT_h          keys on partitions
    P^T = exp(S^T / 8)                ScalarE, batched over 2 key-blocks
    causal: one 128-wide-mask multiply per diagonal block
    z^T[65,512] += [V_h | 1].T @ P^T  row 64 accumulates the denominator
    ZT_h = z^T[0:64] * approx(1/z^T[64])   (denom -> reciprocal ->
           gpsimd partition_broadcast -> multiply)
  out = ZT.T @ Wo (+bo)                                              [1024, 768]
"""

import sys

sys.path.insert(0, "/opt/trn_rl_repo")

from collections import deque

import ml_dtypes
import numpy as np

import concourse.bass as bass
import concourse.mybir as mybir
import concourse.tile as tile
from concourse import bacc
from concourse.bass_utils import run_bass_kernel_spmd

F32 = mybir.dt.float32
BF16 = mybir.dt.bfloat16
AF = mybir.ActivationFunctionType

SEQ = 1024
DM = 768
NH = 12
DH = 64
VH = DH + 2  # V head stride: 64 dims + ones col + pad (keeps 4B alignment)
BATCH = 8
NQT = SEQ // 128  # 8 seq tiles of 128
NDT = DM // 128  # 6 d_model tiles
QC = 512  # query chunk (moving dim)
NQC = SEQ // QC  # 2
WARMUP = 64  # HAM warmup matmuls (bf16 N=128, ~110ns each cold)


def build(with_bq, with_bk, with_bv, with_bo, debug_taps=False):
    nc = bacc.Bacc("TRN2", target_bir_lowering=False, debug=False)

    xt = nc.dram_tensor("xt", [DM, SEQ], BF16, kind="ExternalInput")
    wq = nc.dram_tensor("wq", [DM, DM], BF16, kind="ExternalInput")
    wk = nc.dram_tensor("wk", [DM, DM], BF16, kind="ExternalInput")
    wv = nc.dram_tensor("wv", [DM, DM], BF16, kind="ExternalInput")
    wo = nc.dram_tensor("wo", [DM, DM], BF16, kind="ExternalInput")
    wmask = nc.dram_tensor("wmask", [128, 128], BF16, kind="ExternalInput")
    identin = nc.dram_tensor("identin", [128, 128], BF16, kind="ExternalInput")
    bq = bk = bv = bo = None
    if with_bq:
        bq = nc.dram_tensor("bq", [128, NDT], F32, kind="ExternalInput")
    if with_bk:
        bk = nc.dram_tensor("bk", [128, NDT], F32, kind="ExternalInput")
    if with_bv:
        bv = nc.dram_tensor("bv", [1, DM], F32, kind="ExternalInput")
    if with_bo:
        bo = nc.dram_tensor("bo", [1, DM], F32, kind="ExternalInput")
    out = nc.dram_tensor("out", [SEQ, DM], F32, kind="ExternalOutput")
    taps = {}
    if debug_taps:
        for nm in ("QT_d", "KT_d", "ZT_d"):
            taps[nm] = nc.dram_tensor(nm, [DM, SEQ], BF16, kind="ExternalOutput")
        taps["V_d"] = nc.dram_tensor(
            "V_d", [SEQ, NH * VH], BF16, kind="ExternalOutput"
        )

    with tile.TileContext(nc) as tc:
        with (
            tc.tile_pool(name="persist", bufs=1) as persist,
            tc.tile_pool(name="pt", bufs=4) as pt_pool,
            tc.tile_pool(name="small", bufs=2) as small,
            tc.tile_pool(name="outst", bufs=2) as out_pool,
            tc.tile_pool(name="ps_st", bufs=2, space="PSUM") as ps_st,
            tc.tile_pool(name="ps_z", bufs=3, space="PSUM") as ps_z,
            tc.tile_pool(name="ps_mm", bufs=1, space="PSUM") as ps_mm,
        ):
            # ---- sync ring: ident, xT, WV, mask/ones.  scalar ring: wq/wk
            # (column-split so head pairs 0-1 unblock early), wo later ----
            ident = persist.tile([128, 128], BF16, tag="ident", name="ident")
            nc.sync.dma_start(out=ident, in_=identin[:, :])
            warm_ps = ps_mm.tile(
                [128, 128], F32, tag="proj", name="warm", padded_shape=[128, QC]
            )
            for _ in range(WARMUP):
                nc.tensor.matmul(warm_ps, lhsT=ident, rhs=ident, start=True, stop=True)

            # single sync ring for all inputs in priority order (the two
            # HWDGE rings share ~210GB/s of HBM read bandwidth, and DMAs on
            # the scalar ring block the ACT compute stream behind them).
            # Startup-critical set first: xt query-half 0, wq/wk cols for
            # head pairs 0-1, mask, WV.  xt half 1 and the remaining weight
            # columns are issued after the first projection chunks (below).
            xT = [
                persist.tile([128, SEQ], BF16, tag=f"xT{d}", name=f"xT{d}")
                for d in range(NDT)
            ]
            for d in range(NDT):
                nc.sync.dma_start(
                    out=xT[d][:, 0:QC], in_=xt[d * 128 : (d + 1) * 128, 0:QC]
                )

            WQ = [
                persist.tile([128, DM], BF16, tag=f"WQ{d}", name=f"WQ{d}")
                for d in range(NDT)
            ]
            WK = [
                persist.tile([128, DM], BF16, tag=f"WK{d}", name=f"WK{d}")
                for d in range(NDT)
            ]
            WV = [
                persist.tile([128, DM], BF16, tag=f"WV{d}", name=f"WV{d}")
                for d in range(NDT)
            ]
            for src, dst in ((wq, WQ), (wk, WK)):
                for d in range(NDT):
                    nc.sync.dma_start(
                        out=dst[d][:, 0:256],
                        in_=src[d * 128 : (d + 1) * 128, 0:256],
                    )

            wm_t = persist.tile([128, 128], BF16, tag="wmask", name="wmask")
            nc.sync.dma_start(out=wm_t, in_=wmask[:, :])

            for d in range(NDT):
                nc.sync.dma_start(out=WV[d], in_=wv[d * 128 : (d + 1) * 128, :])

            bias_tiles = {}
            if with_bq:
                t = persist.tile([128, NDT], F32, tag="bq", name="bq")
                nc.scalar.dma_start(out=t, in_=bq[:, :])
                bias_tiles["bq"] = t
            if with_bk:
                t = persist.tile([128, NDT], F32, tag="bk", name="bk")
                nc.scalar.dma_start(out=t, in_=bk[:, :])
                bias_tiles["bk"] = t
            if with_bv:
                t = persist.tile([128, DM], F32, tag="bv", name="bv")
                nc.scalar.dma_start(out=t, in_=bv[0:1, :].to_broadcast((128, DM)))
                bias_tiles["bv"] = t
            if with_bo:
                t = persist.tile([128, DM], F32, tag="bo", name="bo")
                nc.scalar.dma_start(out=t, in_=bo[0:1, :].to_broadcast((128, DM)))
                bias_tiles["bo"] = t

            QT = [
                persist.tile([128, SEQ], BF16, tag=f"QT{d}", name=f"QT{d}")
                for d in range(NDT)
            ]
            KT = [
                persist.tile([128, SEQ], BF16, tag=f"KT{d}", name=f"KT{d}")
                for d in range(NDT)
            ]
            V = [
                persist.tile([128, NH * VH], BF16, tag=f"V{s}", name=f"V{s}")
                for s in range(NQT)
            ]
            for s in range(NQT):
                vv = V[s].rearrange("p (h e) -> p h e", e=VH)
                nc.vector.memset(vv[:, :, DH : DH + 1], 1.0)
            ZT = [
                persist.tile([128, SEQ], BF16, tag=f"ZT{d}", name=f"ZT{d}")
                for d in range(NDT)
            ]

            # ---- projection chunks (each = one psum round trip) ----
            def qk_chunk(hp, which, c, pool, tag):
                W, dst, bkey = (
                    (WQ, QT, "bq") if which == "q" else (WK, KT, "bk")
                )
                acc = pool.tile(
                    [128, QC], F32, tag=tag, name="proj",
                    padded_shape=[128, 2 * QC] if tag == "st" else [128, QC],
                )
                for d in range(NDT):
                    nc.tensor.matmul(
                        acc,
                        lhsT=W[d][:, hp * 128 : (hp + 1) * 128],
                        rhs=xT[d][:, c * QC : (c + 1) * QC],
                        start=(d == 0),
                        stop=(d == NDT - 1),
                    )
                o = dst[hp][:, c * QC : (c + 1) * QC]
                if bkey in bias_tiles:
                    nc.vector.tensor_scalar_add(
                        o, acc, bias_tiles[bkey][:, hp : hp + 1]
                    )
                else:
                    nc.vector.tensor_copy(o, acc)

            def qk_chunks(hp):
                return [
                    (lambda which=which, c=c: qk_chunk(hp, which, c, ps_mm, "proj"))
                    for which in ("q", "k")
                    for c in range(NQC)
                ]

            NVC = 2
            VC = DM // NVC  # 384

            def v_chunk(s, c, on_act=True):
                acc = ps_st.tile(
                    [128, VC], F32, tag="st", name="vacc",
                    padded_shape=[128, 2 * QC],
                )
                for d in range(NDT):
                    nc.tensor.matmul(
                        acc,
                        lhsT=xT[d][:, s * 128 : (s + 1) * 128],
                        rhs=WV[d][:, c * VC : (c + 1) * VC],
                        start=(d == 0),
                        stop=(d == NDT - 1),
                    )
                nh2 = VC // DH  # heads per chunk (6)
                o = V[s].rearrange("p (h e) -> p h e", e=VH)[
                    :, c * nh2 : (c + 1) * nh2, 0:DH
                ]
                if "bv" in bias_tiles:
                    nc.vector.tensor_add(
                        o,
                        acc.rearrange("p (h e) -> p h e", e=DH),
                        bias_tiles["bv"][:, c * VC : (c + 1) * VC].rearrange(
                            "p (h e) -> p h e", e=DH
                        ),
                    )
                elif on_act:
                    nc.scalar.activation(
                        o, acc.rearrange("p (h e) -> p h e", e=DH), AF.Copy
                    )
                else:
                    nc.vector.tensor_copy(
                        o, acc.rearrange("p (h e) -> p h e", e=DH)
                    )

            def v_chunks(s):
                return [lambda c=c: v_chunk(s, c) for c in range(NVC)]

            WO = []

            def wo_load():
                # sync ring: idle mid-attention (scalar ring would block ACT)
                for d in range(NDT):
                    t = persist.tile([128, DM], BF16, tag=f"WO{d}", name=f"WO{d}")
                    nc.sync.dma_start(out=t, in_=wo[d * 128 : (d + 1) * 128, :])
                    WO.append(t)

            def o_chunks(s):
                ot = [None]

                def chunk(c):
                    if c == 0:
                        ot[0] = out_pool.tile([128, DM], F32, tag="ostage", name="ostage")
                    pool, tag = ((ps_mm, "proj"), (ps_z, "z"))[c % 2]
                    acc = pool.tile(
                        [128, VC], F32, tag=tag, name="oacc",
                        padded_shape=[128, QC],
                    )
                    for d in range(NDT):
                        nc.tensor.matmul(
                            acc,
                            lhsT=ZT[d][:, s * 128 : (s + 1) * 128],
                            rhs=WO[d][:, c * VC : (c + 1) * VC],
                            start=(d == 0),
                            stop=(d == NDT - 1),
                        )
                    o = ot[0][:, c * VC : (c + 1) * VC]
                    if "bo" in bias_tiles:
                        nc.vector.tensor_add(
                            o, acc, bias_tiles["bo"][:, c * VC : (c + 1) * VC]
                        )
                    else:
                        # DVE: the ACT stream is exp-saturated mid-phase and
                        # an in-order ACT copy would delay psum recycling
                        nc.vector.tensor_copy(o, acc)
                    if c == NVC - 1:
                        # rows 512+ drain at the very end: use the scalar
                        # ring (idle by then) so the tail DMAs overlap
                        eng = nc.sync if s < 4 else nc.scalar
                        eng.dma_start(
                            out=out[s * 128 : (s + 1) * 128, :], in_=ot[0]
                        )

                return [lambda c=c: chunk(c) for c in range(NVC)]

            # ---- pipelined attention sweep ----
            zps_of = {}

            def issue_scores(hp, c, g, gsz):
                doffs = [max(0, (g + j) * 128 - c * QC) for j in range(gsz)]
                sts = {}
                for px in (0, 64):
                    sts[px] = ps_st.tile([128, gsz * QC], F32, tag="st", name="st")
                for j in range(gsz):
                    kb = g + j
                    off = doffs[j]
                    for px in (0, 64):
                        nc.tensor.matmul(
                            sts[px][:, j * QC + off : (j + 1) * QC],
                            lhsT=KT[hp][px : px + 64, kb * 128 : (kb + 1) * 128],
                            rhs=QT[hp][px : px + 64, c * QC + off : (c + 1) * QC],
                            start=True,
                            stop=True,
                        )
                return sts, doffs

            def issue_expv(hp, c, g, gsz, sts, doffs, last):
                nkb = 4 * (c + 1)
                if g == 0:
                    zps_of[(hp, c)] = {
                        px: ps_z.tile([128, QC], F32, tag="z", name="z")
                        for px in (0, 64)
                    }
                zps = zps_of[(hp, c)]
                pts = {}
                for px in (0, 64):
                    pt = pt_pool.tile([128, 2 * QC], BF16, tag="pt", name="pt")
                    nc.scalar.activation(
                        pt[:, doffs[0] : gsz * QC],
                        sts[px][:, doffs[0] : gsz * QC],
                        AF.Exp,
                        scale=0.125,
                    )
                    pts[px] = pt
                for j in range(gsz):
                    kb = g + j
                    doff = kb * 128 - c * QC
                    off = doffs[j]
                    for px in (0, 64):
                        pt = pts[px]
                        if 0 <= doff < QC:  # diagonal block: 128-wide triangle
                            blk = pt[:, j * QC + doff : j * QC + doff + 128]
                            nc.vector.tensor_mul(blk, blk, wm_t)
                        h = 2 * hp + (1 if px else 0)
                        nc.tensor.matmul(
                            zps[px][0 : DH + 1, off:QC],
                            lhsT=V[kb][:, h * VH : h * VH + DH + 1],
                            rhs=pt[:, j * QC + off : (j + 1) * QC],
                            start=(kb == 0),
                            stop=(kb == nkb - 1),
                        )
                if last:
                    for px in (0, 64):
                        dstage = small.tile([128, QC], F32, tag="dstage", name="dstage")
                        nc.vector.tensor_copy(dstage[0:1, :], zps[px][DH : DH + 1, :])
                        recip = small.tile([128, QC], F32, tag="recip", name="recip")
                        nc.vector.reciprocal_approx_fast(recip[0:1, :], dstage[0:1, :])
                        bcast = small.tile([64, QC], F32, tag="bcast", name="bcast")
                        nc.gpsimd.partition_broadcast(bcast, recip[0:1, :])
                        nc.vector.tensor_mul(
                            ZT[hp][px : px + 64, c * QC : (c + 1) * QC],
                            zps[px][0:64, :],
                            bcast,
                        )
                    del zps_of[(hp, c)]

            # ---- pre-phase: project heads 0-1 for query half 0 only (the
            # qc=1 halves are computed as fillers much later), then issue
            # the non-critical DMAs, then V tiles 0-1 ----
            qk_chunk(0, "q", 0, ps_mm, "proj")
            qk_chunk(0, "k", 0, ps_st, "st")
            for d in range(NDT):
                nc.sync.dma_start(
                    out=xT[d][:, QC:SEQ], in_=xt[d * 128 : (d + 1) * 128, QC:SEQ]
                )
            for src, dst in ((wq, WQ), (wk, WK)):
                for d in range(NDT):
                    nc.sync.dma_start(
                        out=dst[d][:, 256:DM],
                        in_=src[d * 128 : (d + 1) * 128, 256:DM],
                    )
            for s in (0, 1):
                for c in range(NVC):
                    # DVE copies: the ACT stream would stall behind the WV
                    # DMA and delay the first exps queued after it
                    v_chunk(s, c, on_act=False)

            # qc=0 and qc=1 units interleaved: spreads the exp-heavy qc=1
            # units (ACT-bound) across the whole span instead of
            # back-loading them.  Every unit gets filler chunks so exp
            # latency is always hidden behind interposed PE work.
            units = [
                (0, 0), (1, 0), (0, 1), (2, 0), (1, 1), (3, 0),
                (2, 1), (4, 0), (5, 0), (3, 1), (4, 1), (5, 1),
            ]

            def vc(s, c):
                return lambda: v_chunk(s, c)

            def qkc(hp, which, c):
                return lambda: qk_chunk(hp, which, c, ps_mm, "proj")

            fillers = {
                0: [qkc(1, "q", 0), qkc(1, "k", 0), vc(2, 0), vc(3, 0)],
                1: [qkc(2, "q", 0), qkc(2, "k", 0), qkc(0, "q", 1), qkc(0, "k", 1)],
                2: [vc(4, 0), vc(5, 0), vc(6, 0), vc(7, 0)],
                3: [qkc(3, "q", 0), qkc(3, "k", 0), qkc(1, "q", 1), qkc(1, "k", 1)],
                4: [vc(2, 1), vc(3, 1), qkc(2, "q", 1)],
                5: [qkc(4, "q", 0), qkc(4, "k", 0), qkc(2, "k", 1)],
                6: [vc(4, 1), vc(5, 1), vc(6, 1), qkc(3, "q", 1)],
                7: [qkc(5, "q", 0), qkc(5, "k", 0), qkc(3, "k", 1)],
                8: [vc(7, 1), wo_load, qkc(4, "q", 1), qkc(4, "k", 1)],
                9: o_chunks(0) + [qkc(5, "q", 1)],
                10: o_chunks(1) + o_chunks(2) + [qkc(5, "k", 1)],
                11: o_chunks(3),
            }
            # units whose fillers read ZT written by the pending finalizer:
            # flush before popping fillers there (issue-order correctness)
            flush_first = {9, 10, 11}

            pending = [None]

            def flush():
                if pending[0] is not None:
                    fn = pending[0]
                    pending[0] = None
                    fn()

            for ui, (hp, c) in enumerate(units):
                nkb = 4 * (c + 1)
                glist = [(g, min(2, nkb - g)) for g in range(0, nkb, 2)]
                chunks = deque(fillers.get(ui, []))
                n = len(glist)
                for gi, (g, gsz) in enumerate(glist):
                    sts, doffs = issue_scores(hp, c, g, gsz)
                    if ui in flush_first:
                        flush()
                    k = -(-len(chunks) // (n - gi)) if chunks else 0
                    for i in range(k):
                        chunks.popleft()()
                        if i == 0:
                            flush()
                    if k == 0:
                        flush()
                    pending[0] = (
                        lambda hp=hp, c=c, g=g, gsz=gsz, sts=sts, doffs=doffs,
                        last=(gi == n - 1): issue_expv(hp, c, g, gsz, sts, doffs, last)
                    )
            flush()

            # ---- tail: output rows 512-1024.  Full-width accumulation on
            # the freed scores psum, one ACT copy (ACT is idle by now), out
            # DMAs alternating between the two rings ----
            for s in range(4, NQT):
                ot = out_pool.tile([128, DM], F32, tag="ostage", name="ostage")
                acc = ps_st.tile(
                    [128, DM], F32, tag="st", name="oacc",
                    padded_shape=[128, 2 * QC],
                )
                for lo, hi in ((0, QC), (QC, DM)):  # <=512 f32 cols per MM
                    for d in range(NDT):
                        nc.tensor.matmul(
                            acc[:, lo:hi],
                            lhsT=ZT[d][:, s * 128 : (s + 1) * 128],
                            rhs=WO[d][:, lo:hi],
                            start=(d == 0),
                            stop=(d == NDT - 1),
                        )
                if "bo" in bias_tiles:
                    nc.vector.tensor_add(ot, acc, bias_tiles["bo"])
                else:
                    nc.scalar.activation(ot, acc, AF.Copy)
                eng = nc.scalar if s % 2 else nc.sync
                eng.dma_start(out=out[s * 128 : (s + 1) * 128, :], in_=ot)

            if debug_taps:
                for nm, tiles in (("QT_d", QT), ("KT_d", KT), ("ZT_d", ZT)):
                    for d in range(NDT):
                        nc.sync.dma_start(
                            out=taps[nm][d * 128 : (d + 1) * 128, :],
                            in_=tiles[d][:, :],
                        )
                for s in range(NQT):
                    nc.sync.dma_start(
                        out=taps["V_d"][s * 128 : (s + 1) * 128, :], in_=V[s][:, :]
                    )

    nc.compile()
    return nc


_CACHE = {}


def _get_nc(key):
    if key not in _CACHE:
        _CACHE[key] = build(*key)
    return _CACHE[key]


def _prep(inputs):
    BF = ml_dtypes.bfloat16
    x = np.asarray(inputs["normalized_resid_pre"], np.float32)
    wq = np.ascontiguousarray(
        np.asarray(inputs["W_Q"], np.float32).transpose(1, 0, 2).reshape(DM, DM)
    ).astype(BF)
    wk = np.ascontiguousarray(
        np.asarray(inputs["W_K"], np.float32).transpose(1, 0, 2).reshape(DM, DM)
    ).astype(BF)
    wv = np.ascontiguousarray(
        np.asarray(inputs["W_V"], np.float32).transpose(1, 0, 2).reshape(DM, DM)
    ).astype(BF)
    wo = np.ascontiguousarray(
        np.asarray(inputs["W_O"], np.float32).reshape(DM, DM)
    ).astype(BF)
    bq = np.asarray(inputs["b_Q"], np.float32).reshape(NDT, 128).T
    bk = np.asarray(inputs["b_K"], np.float32).reshape(NDT, 128).T
    bv = np.asarray(inputs["b_V"], np.float32).reshape(1, DM)
    bo = np.asarray(inputs["b_O"], np.float32).reshape(1, DM)
    jj, uu = np.meshgrid(np.arange(128), np.arange(128), indexing="ij")
    wmask = (uu >= jj).astype(BF)
    key = (
        bool(np.any(bq)),
        bool(np.any(bk)),
        bool(np.any(bv)),
        bool(np.any(bo)),
    )
    common = {
        "wq": wq, "wk": wk, "wv": wv, "wo": wo, "wmask": wmask,
        "identin": np.eye(128, dtype=np.float32).astype(BF),
    }
    if key[0]:
        common["bq"] = np.ascontiguousarray(bq)
    if key[1]:
        common["bk"] = np.ascontiguousarray(bk)
    if key[2]:
        common["bv"] = np.ascontiguousarray(bv)
    if key[3]:
        common["bo"] = np.ascontiguousarray(bo)
    in_maps = [
        dict(common, xt=np.ascontiguousarray(x[b].T).astype(BF))
        for b in range(BATCH)
    ]
    return key, in_maps


def run(inputs, trace=False, **kw):
    key, in_maps = _prep(inputs)
    nc = _get_nc(key)
    res = run_bass_kernel_spmd(
        nc, in_maps, core_ids=list(range(BATCH)), trace=trace, **kw
    )
    outs = np.stack([res.results[b]["out"] for b in range(BATCH)])
    return outs.astype(np.float32), res


def kernel(**inputs):
    out, _ = run(inputs)
    return out


if __name__ == "__main__":
    rng = np.random.default_rng(0)
    ins = {
        "normalized_resid_pre": rng.standard_normal((8, SEQ, DM)).astype(np.float32),
        "W_Q": (0.02 * rng.standard_normal((NH, DM, DH))).astype(np.float32),
        "b_Q": np.zeros((NH, DH), np.float32),
        "W_K": (0.02 * rng.standard_normal((NH, DM, DH))).astype(np.float32),
        "b_K": np.zeros((NH, DH), np.float32),
        "W_V": (0.02 * rng.standard_normal((NH, DM, DH))).astype(np.float32),
        "b_V": np.zeros((NH, DH), np.float32),
        "W_O": (0.02 * rng.standard_normal((NH, DH, DM))).astype(np.float32),
        "b_O": np.zeros((DM,), np.float32),
    }
    out = kernel(**ins)
    print("kernel output", out.shape, out.dtype, float(np.abs(out).max()))


# revision 29
# speedup vs baseline: 1.1714x; 1.0354x over previous
"""Causal multi-head attention on 8 Trainium2 NeuronCores.

Problem: nn_Attention_46643344835180
  x: [8, 1024, 768], 12 heads x 64 dh, causal softmax attention + output proj.

Sharding: data-parallel over batch (8 batch elements -> 8 cores, no collectives).

v3: full bf16 compute (PSUM stays f32), host-side transpose of x (xT fed
directly), weights resident in SBUF, dual DMA rings (sync + scalar HWDGE),
QK/V psum->sbuf copies on the Scalar engine (idle during the projection
phase), and a globally software-pipelined attention sweep: scores of group
g+1 are issued before exp/PV of group g, with projection/output chunks
spread between groups as PE filler.

Per-core dataflow (batch element b):
  xT = x_b.T (host)                                                  [768, 1024]
  QT = Wq.T @ xT  (+bq)            heads stacked on partitions       [768, 1024]
  KT = Wk.T @ xT  (+bk)                                              [768, 1024]
  V  = x_b @ Wv   (+bv)            + interleaved ones column         [1024, 12*66]
  per head h, query-chunk qc (512):
    S^T[k,q] = KT_h.T @ QT_h          keys on partitions
    P^T = exp(S^T / 8)                ScalarE, batched over 2 key-blocks
    causal: one 128-wide-mask multiply per diagonal block
    z^T[65,512] += [V_h | 1].T @ P^T  row 64 accumulates the denominator
    ZT_h = z^T[0:64] * approx(1/z^T[64])   (denom -> reciprocal ->
           gpsimd partition_broadcast -> multiply)
  out = ZT.T @ Wo (+bo)                                              [1024, 768]
"""

import sys

sys.path.insert(0, "/opt/trn_rl_repo")

from collections import deque

import ml_dtypes
import numpy as np

import concourse.bass as bass
import concourse.mybir as mybir
import concourse.tile as tile
from concourse import bacc
from concourse.bass_utils import run_bass_kernel_spmd

F32 = mybir.dt.float32
BF16 = mybir.dt.bfloat16
AF = mybir.ActivationFunctionType

SEQ = 1024
DM = 768
NH = 12
DH = 64
VH = DH + 2  # V head stride: 64 dims + ones col + pad (keeps 4B alignment)
BATCH = 8
NQT = SEQ // 128  # 8 seq tiles of 128
NDT = DM // 128  # 6 d_model tiles
QC = 512  # query chunk (moving dim)
NQC = SEQ // QC  # 2
WARMUP = 64  # HAM warmup matmuls (bf16 N=128, ~110ns each cold)


def build(with_bq, with_bk, with_bv, with_bo, debug_taps=False):
    nc = bacc.Bacc("TRN2", target_bir_lowering=False, debug=False)

    xt = nc.dram_tensor("xt", [DM, SEQ], BF16, kind="ExternalInput")
    wq = nc.dram_tensor("wq", [DM, DM], BF16, kind="ExternalInput")
    wk = nc.dram_tensor("wk", [DM, DM], BF16, kind="ExternalInput")
    wv = nc.dram_tensor("wv", [DM, DM], BF16, kind="ExternalInput")
    wo = nc.dram_tensor("wo", [DM, DM], BF16, kind="ExternalInput")
    wmask = nc.dram_tensor("wmask", [128, 128], BF16, kind="ExternalInput")
    identin = nc.dram_tensor("identin", [128, 128], BF16, kind="ExternalInput")
    bq = bk = bv = bo = None
    if with_bq:
        bq = nc.dram_tensor("bq", [128, NDT], F32, kind="ExternalInput")
    if with_bk:
        bk = nc.dram_tensor("bk", [128, NDT], F32, kind="ExternalInput")
    if with_bv:
        bv = nc.dram_tensor("bv", [1, DM], F32, kind="ExternalInput")
    if with_bo:
        bo = nc.dram_tensor("bo", [1, DM], F32, kind="ExternalInput")
    out = nc.dram_tensor("out", [SEQ, DM], F32, kind="ExternalOutput")
    taps = {}
    if debug_taps:
        for nm in ("QT_d", "KT_d", "ZT_d"):
            taps[nm] = nc.dram_tensor(nm, [DM, SEQ], BF16, kind="ExternalOutput")
        taps["V_d"] = nc.dram_tensor(
            "V_d", [SEQ, NH * VH], BF16, kind="ExternalOutput"
        )

    with tile.TileContext(nc) as tc:
        with (
            tc.tile_pool(name="persist", bufs=1) as persist,
            tc.tile_pool(name="pt", bufs=4) as pt_pool,
            tc.tile_pool(name="small", bufs=2) as small,
            tc.tile_pool(name="outst", bufs=2) as out_pool,
            tc.tile_pool(name="ps_st", bufs=2, space="PSUM") as ps_st,
            tc.tile_pool(name="ps_z", bufs=3, space="PSUM") as ps_z,
            tc.tile_pool(name="ps_mm", bufs=1, space="PSUM") as ps_mm,
        ):
            # ---- sync ring: ident, xT, WV, mask/ones.  scalar ring: wq/wk
            # (column-split so head pairs 0-1 unblock early), wo later ----
            ident = persist.tile([128, 128], BF16, tag="ident", name="ident")
            nc.sync.dma_start(out=ident, in_=identin[:, :])
            warm_ps = ps_mm.tile(
                [128, 128], F32, tag="proj", name="warm", padded_shape=[128, QC]
            )
            for _ in range(WARMUP):
                nc.tensor.matmul(warm_ps, lhsT=ident, rhs=ident, start=True, stop=True)

            # single sync ring for all inputs in priority order (the two
            # HWDGE rings share ~210GB/s of HBM read bandwidth, and DMAs on
            # the scalar ring block the ACT compute stream behind them).
            # Startup-critical set first: xt query-half 0, wq/wk cols for
            # head pairs 0-1, mask, WV.  xt half 1 and the remaining weight
            # columns are issued after the first projection chunks (below).
            xT = [
                persist.tile([128, SEQ], BF16, tag=f"xT{d}", name=f"xT{d}")
                for d in range(NDT)
            ]
            for d in range(NDT):
                nc.sync.dma_start(
                    out=xT[d][:, 0:QC], in_=xt[d * 128 : (d + 1) * 128, 0:QC]
                )

            WQ = [
                persist.tile([128, DM], BF16, tag=f"WQ{d}", name=f"WQ{d}")
                for d in range(NDT)
            ]
            WK = [
                persist.tile([128, DM], BF16, tag=f"WK{d}", name=f"WK{d}")
                for d in range(NDT)
            ]
            WV = [
                persist.tile([128, DM], BF16, tag=f"WV{d}", name=f"WV{d}")
                for d in range(NDT)
            ]
            for src, dst in ((wq, WQ), (wk, WK)):
                for d in range(NDT):
                    nc.sync.dma_start(
                        out=dst[d][:, 0:256],
                        in_=src[d * 128 : (d + 1) * 128, 0:256],
                    )

            wm_t = persist.tile([128, 128], BF16, tag="wmask", name="wmask")
            nc.sync.dma_start(out=wm_t, in_=wmask[:, :])

            for d in range(NDT):
                nc.sync.dma_start(out=WV[d], in_=wv[d * 128 : (d + 1) * 128, :])

            bias_tiles = {}
            if with_bq:
                t = persist.tile([128, NDT], F32, tag="bq", name="bq")
                nc.scalar.dma_start(out=t, in_=bq[:, :])
                bias_tiles["bq"] = t
            if with_bk:
                t = persist.tile([128, NDT], F32, tag="bk", name="bk")
                nc.scalar.dma_start(out=t, in_=bk[:, :])
                bias_tiles["bk"] = t
            if with_bv:
                t = persist.tile([128, DM], F32, tag="bv", name="bv")
                nc.scalar.dma_start(out=t, in_=bv[0:1, :].to_broadcast((128, DM)))
                bias_tiles["bv"] = t
            if with_bo:
                t = persist.tile([128, DM], F32, tag="bo", name="bo")
                nc.scalar.dma_start(out=t, in_=bo[0:1, :].to_broadcast((128, DM)))
                bias_tiles["bo"] = t

            QT = [
                persist.tile([128, SEQ], BF16, tag=f"QT{d}", name=f"QT{d}")
                for d in range(NDT)
            ]
            KT = [
                persist.tile([128, SEQ], BF16, tag=f"KT{d}", name=f"KT{d}")
                for d in range(NDT)
            ]
            V = [
                persist.tile([128, NH * VH], BF16, tag=f"V{s}", name=f"V{s}")
                for s in range(NQT)
            ]
            for s in range(NQT):
                vv = V[s].rearrange("p (h e) -> p h e", e=VH)
                nc.vector.memset(vv[:, :, DH : DH + 1], 1.0)
            ZT = [
                persist.tile([128, SEQ], BF16, tag=f"ZT{d}", name=f"ZT{d}")
                for d in range(NDT)
            ]

            # ---- projection chunks (each = one psum round trip) ----
            def qk_chunk(hp, which, c, pool, tag):
                W, dst, bkey = (
                    (WQ, QT, "bq") if which == "q" else (WK, KT, "bk")
                )
                acc = pool.tile(
                    [128, QC], F32, tag=tag, name="proj",
                    padded_shape=[128, 2 * QC] if tag == "st" else [128, QC],
                )
                for d in range(NDT):
                    nc.tensor.matmul(
                        acc,
                        lhsT=W[d][:, hp * 128 : (hp + 1) * 128],
                        rhs=xT[d][:, c * QC : (c + 1) * QC],
                        start=(d == 0),
                        stop=(d == NDT - 1),
                    )
                o = dst[hp][:, c * QC : (c + 1) * QC]
                if bkey in bias_tiles:
                    nc.vector.tensor_scalar_add(
                        o, acc, bias_tiles[bkey][:, hp : hp + 1]
                    )
                else:
                    nc.vector.tensor_copy(o, acc)

            def qk_chunks(hp):
                return [
                    (lambda which=which, c=c: qk_chunk(hp, which, c, ps_mm, "proj"))
                    for which in ("q", "k")
                    for c in range(NQC)
                ]

            NVC = 2
            VC = DM // NVC  # 384

            def v_chunk(s, c, on_act=True):
                acc = ps_st.tile(
                    [128, VC], F32, tag="st", name="vacc",
                    padded_shape=[128, 2 * QC],
                )
                for d in range(NDT):
                    nc.tensor.matmul(
                        acc,
                        lhsT=xT[d][:, s * 128 : (s + 1) * 128],
                        rhs=WV[d][:, c * VC : (c + 1) * VC],
                        start=(d == 0),
                        stop=(d == NDT - 1),
                    )
                nh2 = VC // DH  # heads per chunk (6)
                o = V[s].rearrange("p (h e) -> p h e", e=VH)[
                    :, c * nh2 : (c + 1) * nh2, 0:DH
                ]
                if "bv" in bias_tiles:
                    nc.vector.tensor_add(
                        o,
                        acc.rearrange("p (h e) -> p h e", e=DH),
                        bias_tiles["bv"][:, c * VC : (c + 1) * VC].rearrange(
                            "p (h e) -> p h e", e=DH
                        ),
                    )
                elif on_act:
                    nc.scalar.activation(
                        o, acc.rearrange("p (h e) -> p h e", e=DH), AF.Copy
                    )
                else:
                    nc.vector.tensor_copy(
                        o, acc.rearrange("p (h e) -> p h e", e=DH)
                    )

            def v_chunks(s):
                return [lambda c=c: v_chunk(s, c) for c in range(NVC)]

            WO = []

            def wo_load():
                # sync ring: idle mid-attention (scalar ring would block ACT)
                for d in range(NDT):
                    t = persist.tile([128, DM], BF16, tag=f"WO{d}", name=f"WO{d}")
                    nc.sync.dma_start(out=t, in_=wo[d * 128 : (d + 1) * 128, :])
                    WO.append(t)

            def o_chunks(s):
                ot = [None]

                def chunk(c):
                    if c == 0:
                        ot[0] = out_pool.tile([128, DM], F32, tag="ostage", name="ostage")
                    pool, tag = ((ps_mm, "proj"), (ps_z, "z"))[c % 2]
                    acc = pool.tile(
                        [128, VC], F32, tag=tag, name="oacc",
                        padded_shape=[128, QC],
                    )
                    for d in range(NDT):
                        nc.tensor.matmul(
                            acc,
                            lhsT=ZT[d][:, s * 128 : (s + 1) * 128],
                            rhs=WO[d][:, c * VC : (c + 1) * VC],
                            start=(d == 0),
                            stop=(d == NDT - 1),
                        )
                    o = ot[0][:, c * VC : (c + 1) * VC]
                    if "bo" in bias_tiles:
                        nc.vector.tensor_add(
                            o, acc, bias_tiles["bo"][:, c * VC : (c + 1) * VC]
                        )
                    else:
                        # DVE: the ACT stream is exp-saturated mid-phase and
                        # an in-order ACT copy would delay psum recycling
                        nc.vector.tensor_copy(o, acc)
                    if c == NVC - 1:
                        # rows 512+ drain at the very end: use the scalar
                        # ring (idle by then) so the tail DMAs overlap
                        eng = nc.sync if s < 4 else nc.scalar
                        eng.dma_start(
                            out=out[s * 128 : (s + 1) * 128, :], in_=ot[0]
                        )

                return [lambda c=c: chunk(c) for c in range(NVC)]

            # ---- pipelined attention sweep ----
            zps_of = {}

            def issue_scores(hp, c, g, gsz):
                doffs = [max(0, (g + j) * 128 - c * QC) for j in range(gsz)]
                sts = {}
                for px in (0, 64):
                    sts[px] = ps_st.tile([128, gsz * QC], F32, tag="st", name="st")
                for j in range(gsz):
                    kb = g + j
                    off = doffs[j]
                    for px in (0, 64):
                        nc.tensor.matmul(
                            sts[px][:, j * QC + off : (j + 1) * QC],
                            lhsT=KT[hp][px : px + 64, kb * 128 : (kb + 1) * 128],
                            rhs=QT[hp][px : px + 64, c * QC + off : (c + 1) * QC],
                            start=True,
                            stop=True,
                        )
                return sts, doffs

            def issue_expv(hp, c, g, gsz, sts, doffs, last):
                nkb = 4 * (c + 1)
                if g == 0:
                    zps_of[(hp, c)] = {
                        px: ps_z.tile([128, QC], F32, tag="z", name="z")
                        for px in (0, 64)
                    }
                zps = zps_of[(hp, c)]
                pts = {}
                for px in (0, 64):
                    pt = pt_pool.tile([128, 2 * QC], BF16, tag="pt", name="pt")
                    nc.scalar.activation(
                        pt[:, doffs[0] : gsz * QC],
                        sts[px][:, doffs[0] : gsz * QC],
                        AF.Exp,
                        scale=0.125,
                    )
                    pts[px] = pt
                for j in range(gsz):
                    kb = g + j
                    doff = kb * 128 - c * QC
                    off = doffs[j]
                    for px in (0, 64):
                        pt = pts[px]
                        if 0 <= doff < QC:  # diagonal block: 128-wide triangle
                            blk = pt[:, j * QC + doff : j * QC + doff + 128]
                            nc.vector.tensor_mul(blk, blk, wm_t)
                        h = 2 * hp + (1 if px else 0)
                        nc.tensor.matmul(
                            zps[px][0 : DH + 1, off:QC],
                            lhsT=V[kb][:, h * VH : h * VH + DH + 1],
                            rhs=pt[:, j * QC + off : (j + 1) * QC],
                            start=(kb == 0),
                            stop=(kb == nkb - 1),
                        )
                if last:
                    for px in (0, 64):
                        dstage = small.tile([128, QC], F32, tag="dstage", name="dstage")
                        nc.vector.tensor_copy(dstage[0:1, :], zps[px][DH : DH + 1, :])
                        recip = small.tile([128, QC], F32, tag="recip", name="recip")
                        nc.vector.reciprocal_approx_fast(recip[0:1, :], dstage[0:1, :])
                        bcast = small.tile([64, QC], F32, tag="bcast", name="bcast")
                        nc.gpsimd.partition_broadcast(bcast, recip[0:1, :])
                        nc.vector.tensor_mul(
                            ZT[hp][px : px + 64, c * QC : (c + 1) * QC],
                            zps[px][0:64, :],
                            bcast,
                        )
                    del zps_of[(hp, c)]

            # ---- pre-phase: project heads 0-1 for query half 0 only (the
            # qc=1 halves are computed as fillers much later), then issue
            # the non-critical DMAs, then V tiles 0-1 ----
            qk_chunk(0, "q", 0, ps_mm, "proj")
            qk_chunk(0, "k", 0, ps_st, "st")
            for d in range(NDT):
                nc.sync.dma_start(
                    out=xT[d][:, QC:SEQ], in_=xt[d * 128 : (d + 1) * 128, QC:SEQ]
                )
            # remaining weight columns per head pair, in first-use order
            for hp in range(2, NH // 2):
                lo, hi = hp * 128, (hp + 1) * 128
                for src, dst in ((wq, WQ), (wk, WK)):
                    for d in range(NDT):
                        nc.sync.dma_start(
                            out=dst[d][:, lo:hi],
                            in_=src[d * 128 : (d + 1) * 128, lo:hi],
                        )
            for s in (0, 1):
                for c in range(NVC):
                    # DVE copies: the ACT stream would stall behind the WV
                    # DMA and delay the first exps queued after it
                    v_chunk(s, c, on_act=False)

            # qc=0 and qc=1 units interleaved: spreads the exp-heavy qc=1
            # units (ACT-bound) across the whole span instead of
            # back-loading them.  Every unit gets filler chunks so exp
            # latency is always hidden behind interposed PE work.
            # the first four units need only the startup-critical DMA set
            # (xt + head-pair-0/1 weight columns); later head pairs' weight
            # columns stream in per-pair just ahead of first use
            units = [
                (0, 0), (1, 0), (0, 1), (1, 1), (2, 0), (3, 0),
                (2, 1), (4, 0), (5, 0), (3, 1), (4, 1), (5, 1),
            ]

            def vc(s, c):
                return lambda: v_chunk(s, c)

            def qkc(hp, which, c):
                return lambda: qk_chunk(hp, which, c, ps_mm, "proj")

            fillers = {
                0: [qkc(1, "q", 0), qkc(1, "k", 0), vc(2, 0), vc(3, 0)],
                1: [qkc(0, "q", 1), qkc(0, "k", 1), vc(4, 0), vc(5, 0)],
                2: [qkc(1, "q", 1), qkc(1, "k", 1), vc(6, 0), vc(7, 0)],
                3: [qkc(2, "q", 0), qkc(2, "k", 0), vc(2, 1), vc(3, 1)],
                4: [qkc(3, "q", 0), qkc(3, "k", 0), vc(4, 1)],
                5: [qkc(2, "q", 1), qkc(2, "k", 1), vc(5, 1)],
                6: [qkc(4, "q", 0), qkc(4, "k", 0), vc(6, 1)],
                7: [qkc(5, "q", 0), qkc(5, "k", 0), vc(7, 1)],
                8: [qkc(3, "q", 1), qkc(3, "k", 1), wo_load],
                9: o_chunks(0) + [qkc(4, "q", 1), qkc(4, "k", 1)],
                10: o_chunks(1) + o_chunks(2) + [qkc(5, "q", 1), qkc(5, "k", 1)],
                11: o_chunks(3),
            }
            # units whose fillers read ZT written by the pending finalizer:
            # flush before popping fillers there (issue-order correctness)
            flush_first = {9, 10, 11}

            pending = [None]

            def flush():
                if pending[0] is not None:
                    fn = pending[0]
                    pending[0] = None
                    fn()

            for ui, (hp, c) in enumerate(units):
                nkb = 4 * (c + 1)
                glist = [(g, min(2, nkb - g)) for g in range(0, nkb, 2)]
                chunks = deque(fillers.get(ui, []))
                n = len(glist)
                for gi, (g, gsz) in enumerate(glist):
                    sts, doffs = issue_scores(hp, c, g, gsz)
                    if ui in flush_first:
                        flush()
                    k = -(-len(chunks) // (n - gi)) if chunks else 0
                    for i in range(k):
                        chunks.popleft()()
                        if i == 0:
                            flush()
                    if k == 0:
                        flush()
                    pending[0] = (
                        lambda hp=hp, c=c, g=g, gsz=gsz, sts=sts, doffs=doffs,
                        last=(gi == n - 1): issue_expv(hp, c, g, gsz, sts, doffs, last)
                    )
            flush()

            # ---- tail: output rows 512-1024.  Full-width accumulation on
            # the freed scores psum, one ACT copy (ACT is idle by now), out
            # DMAs alternating between the two rings.  s=4/5 accumulate
            # d=0..4 first so the PE is busy while the last unit's
            # normalization chain (recip/broadcast/ZT-mul for ZT[5]) runs.
            def o_full_mm(s, acc, drange, start):
                for lo, hi in ((0, QC), (QC, DM)):  # <=512 f32 cols per MM
                    for d in drange:
                        nc.tensor.matmul(
                            acc[:, lo:hi],
                            lhsT=ZT[d][:, s * 128 : (s + 1) * 128],
                            rhs=WO[d][:, lo:hi],
                            start=(start and d == drange[0]),
                            stop=(d == NDT - 1),
                        )

            def o_full_out(s, acc):
                ot = out_pool.tile([128, DM], F32, tag="ostage", name="ostage")
                if "bo" in bias_tiles:
                    nc.vector.tensor_add(ot, acc, bias_tiles["bo"])
                else:
                    nc.scalar.activation(ot, acc, AF.Copy)
                eng = nc.scalar if s % 2 else nc.sync
                eng.dma_start(out=out[s * 128 : (s + 1) * 128, :], in_=ot)

            accs = {}
            for s in (4, 5):
                accs[s] = ps_st.tile(
                    [128, DM], F32, tag="st", name="oacc",
                    padded_shape=[128, 2 * QC],
                )
                o_full_mm(s, accs[s], list(range(NDT - 1)), start=True)
            for s in (4, 5):
                o_full_mm(s, accs[s], [NDT - 1], start=False)
                o_full_out(s, accs[s])
            for s in (6, 7):
                acc = ps_st.tile(
                    [128, DM], F32, tag="st", name="oacc",
                    padded_shape=[128, 2 * QC],
                )
                o_full_mm(s, acc, list(range(NDT)), start=True)
                o_full_out(s, acc)

            if debug_taps:
                for nm, tiles in (("QT_d", QT), ("KT_d", KT), ("ZT_d", ZT)):
                    for d in range(NDT):
                        nc.sync.dma_start(
                            out=taps[nm][d * 128 : (d + 1) * 128, :],
                            in_=tiles[d][:, :],
                        )
                for s in range(NQT):
                    nc.sync.dma_start(
                        out=taps["V_d"][s * 128 : (s + 1) * 128, :], in_=V[s][:, :]
                    )

    nc.compile()
    return nc


_CACHE = {}


def _get_nc(key):
    if key not in _CACHE:
        _CACHE[key] = build(*key)
    return _CACHE[key]


def _prep(inputs):
    BF = ml_dtypes.bfloat16
    x = np.asarray(inputs["normalized_resid_pre"], np.float32)
    wq = np.ascontiguousarray(
        np.asarray(inputs["W_Q"], np.float32).transpose(1, 0, 2).reshape(DM, DM)
    ).astype(BF)
    wk = np.ascontiguousarray(
        np.asarray(inputs["W_K"], np.float32).transpose(1, 0, 2).reshape(DM, DM)
    ).astype(BF)
    wv = np.ascontiguousarray(
        np.asarray(inputs["W_V"], np.float32).transpose(1, 0, 2).reshape(DM, DM)
    ).astype(BF)
    wo = np.ascontiguousarray(
        np.asarray(inputs["W_O"], np.float32).reshape(DM, DM)
    ).astype(BF)
    bq = np.asarray(inputs["b_Q"], np.float32).reshape(NDT, 128).T
    bk = np.asarray(inputs["b_K"], np.float32).reshape(NDT, 128).T
    bv = np.asarray(inputs["b_V"], np.float32).reshape(1, DM)
    bo = np.asarray(inputs["b_O"], np.float32).reshape(1, DM)
    jj, uu = np.meshgrid(np.arange(128), np.arange(128), indexing="ij")
    wmask = (uu >= jj).astype(BF)
    key = (
        bool(np.any(bq)),
        bool(np.any(bk)),
        bool(np.any(bv)),
        bool(np.any(bo)),
    )
    common = {
        "wq": wq, "wk": wk, "wv": wv, "wo": wo, "wmask": wmask,
        "identin": np.eye(128, dtype=np.float32).astype(BF),
    }
    if key[0]:
        common["bq"] = np.ascontiguousarray(bq)
    if key[1]:
        common["bk"] = np.ascontiguousarray(bk)
    if key[2]:
        common["bv"] = np.ascontiguousarray(bv)
    if key[3]:
        common["bo"] = np.ascontiguousarray(bo)
    in_maps = [
        dict(common, xt=np.ascontiguousarray(x[b].T).astype(BF))
        for b in range(BATCH)
    ]
    return key, in_maps


def run(inputs, trace=False, **kw):
    key, in_maps = _prep(inputs)
    nc = _get_nc(key)
    res = run_bass_kernel_spmd(
        nc, in_maps, core_ids=list(range(BATCH)), trace=trace, **kw
    )
    outs = np.stack([res.results[b]["out"] for b in range(BATCH)])
    return outs.astype(np.float32), res


def kernel(**inputs):
    out, _ = run(inputs)
    return out


if __name__ == "__main__":
    rng = np.random.default_rng(0)
    ins = {
        "normalized_resid_pre": rng.standard_normal((8, SEQ, DM)).astype(np.float32),
        "W_Q": (0.02 * rng.standard_normal((NH, DM, DH))).astype(np.float32),
        "b_Q": np.zeros((NH, DH), np.float32),
        "W_K": (0.02 * rng.standard_normal((NH, DM, DH))).astype(np.float32),
        "b_K": np.zeros((NH, DH), np.float32),
        "W_V": (0.02 * rng.standard_normal((NH, DM, DH))).astype(np.float32),
        "b_V": np.zeros((NH, DH), np.float32),
        "W_O": (0.02 * rng.standard_normal((NH, DH, DM))).astype(np.float32),
        "b_O": np.zeros((DM,), np.float32),
    }
    out = kernel(**ins)
    print("kernel output", out.shape, out.dtype, float(np.abs(out).max()))


# revision 32
# speedup vs baseline: 1.1742x; 1.0024x over previous
"""Causal multi-head attention on 8 Trainium2 NeuronCores.

Problem: nn_Attention_46643344835180
  x: [8, 1024, 768], 12 heads x 64 dh, causal softmax attention + output proj.

Sharding: data-parallel over batch (8 batch elements -> 8 cores, no collectives).

v3: full bf16 compute (PSUM stays f32), host-side transpose of x (xT fed
directly), weights resident in SBUF, dual DMA rings (sync + scalar HWDGE),
QK/V psum->sbuf copies on the Scalar engine (idle during the projection
phase), and a globally software-pipelined attention sweep: scores of group
g+1 are issued before exp/PV of group g, with projection/output chunks
spread between groups as PE filler.

Per-core dataflow (batch element b):
  xT = x_b.T (host)                                                  [768, 1024]
  QT = Wq.T @ xT  (+bq)            heads stacked on partitions       [768, 1024]
  KT = Wk.T @ xT  (+bk)                                              [768, 1024]
  V  = x_b @ Wv   (+bv)            + interleaved ones column         [1024, 12*66]
  per head h, query-chunk qc (512):
    S^T[k,q] = KT_h.T @ QT_h          keys on partitions
    P^T = exp(S^T / 8)                ScalarE, batched over 2 key-blocks
    causal: one 128-wide-mask multiply per diagonal block
    z^T[65,512] += [V_h | 1].T @ P^T  row 64 accumulates the denominator
    ZT_h = z^T[0:64] * approx(1/z^T[64])   (denom -> reciprocal ->
           gpsimd partition_broadcast -> multiply)
  out = ZT.T @ Wo (+bo)                                              [1024, 768]
"""

import sys

sys.path.insert(0, "/opt/trn_rl_repo")

from collections import deque

import ml_dtypes
import numpy as np

import concourse.bass as bass
import concourse.mybir as mybir
import concourse.tile as tile
from concourse import bacc
from concourse.bass_utils import run_bass_kernel_spmd

F32 = mybir.dt.float32
BF16 = mybir.dt.bfloat16
AF = mybir.ActivationFunctionType

SEQ = 1024
DM = 768
NH = 12
DH = 64
VH = DH + 2  # V head stride: 64 dims + ones col + pad (keeps 4B alignment)
BATCH = 8
NQT = SEQ // 128  # 8 seq tiles of 128
NDT = DM // 128  # 6 d_model tiles
QC = 512  # query chunk (moving dim)
NQC = SEQ // QC  # 2
WARMUP = 64  # HAM warmup matmuls (bf16 N=128, ~110ns each cold)


def build(with_bq, with_bk, with_bv, with_bo, debug_taps=False):
    nc = bacc.Bacc("TRN2", target_bir_lowering=False, debug=False)

    xt = nc.dram_tensor("xt", [DM, SEQ], BF16, kind="ExternalInput")
    wq = nc.dram_tensor("wq", [DM, DM], BF16, kind="ExternalInput")
    wk = nc.dram_tensor("wk", [DM, DM], BF16, kind="ExternalInput")
    wv = nc.dram_tensor("wv", [DM, DM], BF16, kind="ExternalInput")
    wo = nc.dram_tensor("wo", [DM, DM], BF16, kind="ExternalInput")
    wmask = nc.dram_tensor("wmask", [128, 128], BF16, kind="ExternalInput")
    identin = nc.dram_tensor("identin", [128, 128], BF16, kind="ExternalInput")
    bq = bk = bv = bo = None
    if with_bq:
        bq = nc.dram_tensor("bq", [128, NDT], F32, kind="ExternalInput")
    if with_bk:
        bk = nc.dram_tensor("bk", [128, NDT], F32, kind="ExternalInput")
    if with_bv:
        bv = nc.dram_tensor("bv", [1, DM], F32, kind="ExternalInput")
    if with_bo:
        bo = nc.dram_tensor("bo", [1, DM], F32, kind="ExternalInput")
    out = nc.dram_tensor("out", [SEQ, DM], F32, kind="ExternalOutput")
    taps = {}
    if debug_taps:
        for nm in ("QT_d", "KT_d", "ZT_d"):
            taps[nm] = nc.dram_tensor(nm, [DM, SEQ], BF16, kind="ExternalOutput")
        taps["V_d"] = nc.dram_tensor(
            "V_d", [SEQ, NH * VH], BF16, kind="ExternalOutput"
        )

    with tile.TileContext(nc) as tc:
        with (
            tc.tile_pool(name="persist", bufs=1) as persist,
            tc.tile_pool(name="pt", bufs=4) as pt_pool,
            tc.tile_pool(name="small", bufs=2) as small,
            tc.tile_pool(name="outst", bufs=2) as out_pool,
            tc.tile_pool(name="ps_st", bufs=2, space="PSUM") as ps_st,
            tc.tile_pool(name="ps_z", bufs=3, space="PSUM") as ps_z,
            tc.tile_pool(name="ps_mm", bufs=1, space="PSUM") as ps_mm,
        ):
            # ---- sync ring: ident, xT, WV, mask/ones.  scalar ring: wq/wk
            # (column-split so head pairs 0-1 unblock early), wo later ----
            ident = persist.tile([128, 128], BF16, tag="ident", name="ident")
            nc.sync.dma_start(out=ident, in_=identin[:, :])
            warm_ps = ps_mm.tile(
                [128, 128], F32, tag="proj", name="warm", padded_shape=[128, QC]
            )
            for _ in range(WARMUP):
                nc.tensor.matmul(warm_ps, lhsT=ident, rhs=ident, start=True, stop=True)

            # single sync ring for all inputs in priority order (the two
            # HWDGE rings share ~210GB/s of HBM read bandwidth, and DMAs on
            # the scalar ring block the ACT compute stream behind them).
            # Startup-critical set first: xt query-half 0, wq/wk cols for
            # head pairs 0-1, mask, WV.  xt half 1 and the remaining weight
            # columns are issued after the first projection chunks (below).
            xT = [
                persist.tile([128, SEQ], BF16, tag=f"xT{d}", name=f"xT{d}")
                for d in range(NDT)
            ]
            for d in range(NDT):
                nc.sync.dma_start(
                    out=xT[d][:, 0:QC], in_=xt[d * 128 : (d + 1) * 128, 0:QC]
                )

            WQ = [
                persist.tile([128, DM], BF16, tag=f"WQ{d}", name=f"WQ{d}")
                for d in range(NDT)
            ]
            WK = [
                persist.tile([128, DM], BF16, tag=f"WK{d}", name=f"WK{d}")
                for d in range(NDT)
            ]
            WV = [
                persist.tile([128, DM], BF16, tag=f"WV{d}", name=f"WV{d}")
                for d in range(NDT)
            ]
            for src, dst in ((wq, WQ), (wk, WK)):
                for d in range(NDT):
                    nc.sync.dma_start(
                        out=dst[d][:, 0:256],
                        in_=src[d * 128 : (d + 1) * 128, 0:256],
                    )

            wm_t = persist.tile([128, 128], BF16, tag="wmask", name="wmask")
            nc.sync.dma_start(out=wm_t, in_=wmask[:, :])

            for d in range(NDT):
                nc.sync.dma_start(out=WV[d], in_=wv[d * 128 : (d + 1) * 128, :])

            bias_tiles = {}
            if with_bq:
                t = persist.tile([128, NDT], F32, tag="bq", name="bq")
                nc.scalar.dma_start(out=t, in_=bq[:, :])
                bias_tiles["bq"] = t
            if with_bk:
                t = persist.tile([128, NDT], F32, tag="bk", name="bk")
                nc.scalar.dma_start(out=t, in_=bk[:, :])
                bias_tiles["bk"] = t
            if with_bv:
                t = persist.tile([128, DM], F32, tag="bv", name="bv")
                nc.scalar.dma_start(out=t, in_=bv[0:1, :].to_broadcast((128, DM)))
                bias_tiles["bv"] = t
            if with_bo:
                t = persist.tile([128, DM], F32, tag="bo", name="bo")
                nc.scalar.dma_start(out=t, in_=bo[0:1, :].to_broadcast((128, DM)))
                bias_tiles["bo"] = t

            QT = [
                persist.tile([128, SEQ], BF16, tag=f"QT{d}", name=f"QT{d}")
                for d in range(NDT)
            ]
            KT = [
                persist.tile([128, SEQ], BF16, tag=f"KT{d}", name=f"KT{d}")
                for d in range(NDT)
            ]
            V = [
                persist.tile([128, NH * VH], BF16, tag=f"V{s}", name=f"V{s}")
                for s in range(NQT)
            ]
            for s in range(NQT):
                vv = V[s].rearrange("p (h e) -> p h e", e=VH)
                nc.vector.memset(vv[:, :, DH : DH + 1], 1.0)
            ZT = [
                persist.tile([128, SEQ], BF16, tag=f"ZT{d}", name=f"ZT{d}")
                for d in range(NDT)
            ]

            # ---- projection chunks (each = one psum round trip) ----
            def qk_chunk(hp, which, c, pool, tag):
                W, dst, bkey = (
                    (WQ, QT, "bq") if which == "q" else (WK, KT, "bk")
                )
                acc = pool.tile(
                    [128, QC], F32, tag=tag, name="proj",
                    padded_shape=[128, 2 * QC] if tag == "st" else [128, QC],
                )
                for d in range(NDT):
                    nc.tensor.matmul(
                        acc,
                        lhsT=W[d][:, hp * 128 : (hp + 1) * 128],
                        rhs=xT[d][:, c * QC : (c + 1) * QC],
                        start=(d == 0),
                        stop=(d == NDT - 1),
                    )
                o = dst[hp][:, c * QC : (c + 1) * QC]
                if bkey in bias_tiles:
                    nc.vector.tensor_scalar_add(
                        o, acc, bias_tiles[bkey][:, hp : hp + 1]
                    )
                else:
                    nc.vector.tensor_copy(o, acc)

            def qk_chunks(hp):
                return [
                    (lambda which=which, c=c: qk_chunk(hp, which, c, ps_mm, "proj"))
                    for which in ("q", "k")
                    for c in range(NQC)
                ]

            NVC = 2
            VC = DM // NVC  # 384

            def v_chunk(s, c, on_act=True):
                acc = ps_st.tile(
                    [128, VC], F32, tag="st", name="vacc",
                    padded_shape=[128, 2 * QC],
                )
                for d in range(NDT):
                    nc.tensor.matmul(
                        acc,
                        lhsT=xT[d][:, s * 128 : (s + 1) * 128],
                        rhs=WV[d][:, c * VC : (c + 1) * VC],
                        start=(d == 0),
                        stop=(d == NDT - 1),
                    )
                nh2 = VC // DH  # heads per chunk (6)
                o = V[s].rearrange("p (h e) -> p h e", e=VH)[
                    :, c * nh2 : (c + 1) * nh2, 0:DH
                ]
                if "bv" in bias_tiles:
                    nc.vector.tensor_add(
                        o,
                        acc.rearrange("p (h e) -> p h e", e=DH),
                        bias_tiles["bv"][:, c * VC : (c + 1) * VC].rearrange(
                            "p (h e) -> p h e", e=DH
                        ),
                    )
                elif on_act:
                    nc.scalar.activation(
                        o, acc.rearrange("p (h e) -> p h e", e=DH), AF.Copy
                    )
                else:
                    nc.vector.tensor_copy(
                        o, acc.rearrange("p (h e) -> p h e", e=DH)
                    )

            def v_chunks(s):
                return [lambda c=c: v_chunk(s, c) for c in range(NVC)]

            WO = []

            def wo_load():
                # sync ring: idle mid-attention (scalar ring would block ACT)
                for d in range(NDT):
                    t = persist.tile([128, DM], BF16, tag=f"WO{d}", name=f"WO{d}")
                    nc.sync.dma_start(out=t, in_=wo[d * 128 : (d + 1) * 128, :])
                    WO.append(t)

            def o_chunks(s):
                ot = [None]

                def chunk(c):
                    if c == 0:
                        ot[0] = out_pool.tile([128, DM], F32, tag="ostage", name="ostage")
                    pool, tag = ((ps_mm, "proj"), (ps_z, "z"))[c % 2]
                    acc = pool.tile(
                        [128, VC], F32, tag=tag, name="oacc",
                        padded_shape=[128, QC],
                    )
                    for d in range(NDT):
                        nc.tensor.matmul(
                            acc,
                            lhsT=ZT[d][:, s * 128 : (s + 1) * 128],
                            rhs=WO[d][:, c * VC : (c + 1) * VC],
                            start=(d == 0),
                            stop=(d == NDT - 1),
                        )
                    o = ot[0][:, c * VC : (c + 1) * VC]
                    if "bo" in bias_tiles:
                        nc.vector.tensor_add(
                            o, acc, bias_tiles["bo"][:, c * VC : (c + 1) * VC]
                        )
                    else:
                        # DVE: the ACT stream is exp-saturated mid-phase and
                        # an in-order ACT copy would delay psum recycling
                        nc.vector.tensor_copy(o, acc)
                    if c == NVC - 1:
                        # rows 512+ drain at the very end: use the scalar
                        # ring (idle by then) so the tail DMAs overlap
                        eng = nc.sync if s < 4 else nc.scalar
                        eng.dma_start(
                            out=out[s * 128 : (s + 1) * 128, :], in_=ot[0]
                        )

                return [lambda c=c: chunk(c) for c in range(NVC)]

            # ---- pipelined attention sweep ----
            zps_of = {}

            def issue_scores(hp, c, g, gsz):
                doffs = [max(0, (g + j) * 128 - c * QC) for j in range(gsz)]
                sts = {}
                for px in (0, 64):
                    sts[px] = ps_st.tile([128, gsz * QC], F32, tag="st", name="st")
                for j in range(gsz):
                    kb = g + j
                    off = doffs[j]
                    for px in (0, 64):
                        nc.tensor.matmul(
                            sts[px][:, j * QC + off : (j + 1) * QC],
                            lhsT=KT[hp][px : px + 64, kb * 128 : (kb + 1) * 128],
                            rhs=QT[hp][px : px + 64, c * QC + off : (c + 1) * QC],
                            start=True,
                            stop=True,
                        )
                return sts, doffs

            def issue_expv(hp, c, g, gsz, sts, doffs, last):
                nkb = 4 * (c + 1)
                if g == 0:
                    zps_of[(hp, c)] = {
                        px: ps_z.tile([128, QC], F32, tag="z", name="z")
                        for px in (0, 64)
                    }
                zps = zps_of[(hp, c)]
                pts = {}
                for px in (0, 64):
                    pt = pt_pool.tile([128, 2 * QC], BF16, tag="pt", name="pt")
                    nc.scalar.activation(
                        pt[:, doffs[0] : gsz * QC],
                        sts[px][:, doffs[0] : gsz * QC],
                        AF.Exp,
                        scale=0.125,
                    )
                    pts[px] = pt
                for j in range(gsz):
                    kb = g + j
                    doff = kb * 128 - c * QC
                    off = doffs[j]
                    for px in (0, 64):
                        pt = pts[px]
                        if 0 <= doff < QC:  # diagonal block: 128-wide triangle
                            blk = pt[:, j * QC + doff : j * QC + doff + 128]
                            nc.vector.tensor_mul(blk, blk, wm_t)
                        h = 2 * hp + (1 if px else 0)
                        nc.tensor.matmul(
                            zps[px][0 : DH + 1, off:QC],
                            lhsT=V[kb][:, h * VH : h * VH + DH + 1],
                            rhs=pt[:, j * QC + off : (j + 1) * QC],
                            start=(kb == 0),
                            stop=(kb == nkb - 1),
                        )
                if last:
                    for px in (0, 64):
                        dstage = small.tile([128, QC], F32, tag="dstage", name="dstage")
                        nc.vector.tensor_copy(dstage[0:1, :], zps[px][DH : DH + 1, :])
                        recip = small.tile([128, QC], F32, tag="recip", name="recip")
                        nc.vector.reciprocal_approx_fast(recip[0:1, :], dstage[0:1, :])
                        bcast = small.tile([64, QC], F32, tag="bcast", name="bcast")
                        nc.gpsimd.partition_broadcast(bcast, recip[0:1, :])
                        nc.vector.tensor_mul(
                            ZT[hp][px : px + 64, c * QC : (c + 1) * QC],
                            zps[px][0:64, :],
                            bcast,
                        )
                    del zps_of[(hp, c)]

            # ---- pre-phase: project heads 0-1 for query half 0 only (the
            # qc=1 halves are computed as fillers much later), then issue
            # the non-critical DMAs, then V tiles 0-1 ----
            qk_chunk(0, "q", 0, ps_mm, "proj")
            qk_chunk(0, "k", 0, ps_st, "st")
            for d in range(NDT):
                nc.sync.dma_start(
                    out=xT[d][:, QC:SEQ], in_=xt[d * 128 : (d + 1) * 128, QC:SEQ]
                )
            # remaining weight columns in one batched piece per row tile
            # (each DMA has a ~600ns floor: fewer, larger transfers win)
            for src, dst in ((wq, WQ), (wk, WK)):
                for d in range(NDT):
                    nc.sync.dma_start(
                        out=dst[d][:, 256:DM],
                        in_=src[d * 128 : (d + 1) * 128, 256:DM],
                    )

            # qc=0 and qc=1 units interleaved: spreads the exp-heavy qc=1
            # units (ACT-bound) across the whole span instead of
            # back-loading them.  Every unit gets filler chunks so exp
            # latency is always hidden behind interposed PE work.
            # the first four units need only the startup-critical DMA set
            # (xt + head-pair-0/1 weight columns); later head pairs' weight
            # columns stream in per-pair just ahead of first use
            units = [
                (0, 0), (1, 0), (0, 1), (1, 1), (2, 0), (3, 0),
                (2, 1), (4, 0), (5, 0), (3, 1), (4, 1), (5, 1),
            ]

            def vc(s, c):
                return lambda: v_chunk(s, c)

            def qkc(hp, which, c):
                return lambda: qk_chunk(hp, which, c, ps_mm, "proj")

            fillers = {
                # v0/v1 here (not pre-phase): their WV-gated matmuls must
                # not sit in the PE stream ahead of the first scores.
                # DVE copies so the first exps aren't queued behind them.
                0: [
                    lambda: v_chunk(0, 0, on_act=False),
                    lambda: v_chunk(0, 1, on_act=False),
                    lambda: v_chunk(1, 0, on_act=False),
                    lambda: v_chunk(1, 1, on_act=False),
                    qkc(1, "q", 0), qkc(1, "k", 0), vc(2, 0), vc(3, 0),
                ],
                1: [qkc(0, "q", 1), qkc(0, "k", 1), vc(4, 0), vc(5, 0)],
                2: [qkc(1, "q", 1), qkc(1, "k", 1), vc(6, 0), vc(7, 0)],
                3: [qkc(2, "q", 0), qkc(2, "k", 0), vc(2, 1), vc(3, 1)],
                4: [qkc(3, "q", 0), qkc(3, "k", 0), vc(4, 1)],
                5: [qkc(2, "q", 1), qkc(2, "k", 1), vc(5, 1)],
                6: [qkc(4, "q", 0), qkc(4, "k", 0), vc(6, 1)],
                7: [qkc(5, "q", 0), qkc(5, "k", 0), vc(7, 1)],
                8: [qkc(3, "q", 1), qkc(3, "k", 1), wo_load],
                9: o_chunks(0) + [qkc(4, "q", 1), qkc(4, "k", 1)],
                10: o_chunks(1) + o_chunks(2) + [qkc(5, "q", 1), qkc(5, "k", 1)],
                11: o_chunks(3),
            }
            # units whose fillers read ZT written by the pending finalizer:
            # flush before popping fillers there (issue-order correctness)
            flush_first = {9, 10, 11}

            pending = [None]

            def flush():
                if pending[0] is not None:
                    fn = pending[0]
                    pending[0] = None
                    fn()

            for ui, (hp, c) in enumerate(units):
                nkb = 4 * (c + 1)
                glist = [(g, min(2, nkb - g)) for g in range(0, nkb, 2)]
                chunks = deque(fillers.get(ui, []))
                n = len(glist)
                for gi, (g, gsz) in enumerate(glist):
                    sts, doffs = issue_scores(hp, c, g, gsz)
                    if ui in flush_first:
                        flush()
                    k = -(-len(chunks) // (n - gi)) if chunks else 0
                    for i in range(k):
                        chunks.popleft()()
                        if i == 0:
                            flush()
                    if k == 0:
                        flush()
                    pending[0] = (
                        lambda hp=hp, c=c, g=g, gsz=gsz, sts=sts, doffs=doffs,
                        last=(gi == n - 1): issue_expv(hp, c, g, gsz, sts, doffs, last)
                    )
            flush()

            # ---- tail: output rows 512-1024.  Full-width accumulation on
            # the freed scores psum, one ACT copy (ACT is idle by now), out
            # DMAs alternating between the two rings.  s=4/5 accumulate
            # d=0..4 first so the PE is busy while the last unit's
            # normalization chain (recip/broadcast/ZT-mul for ZT[5]) runs.
            def o_full_mm(s, acc, drange, start):
                for lo, hi in ((0, QC), (QC, DM)):  # <=512 f32 cols per MM
                    for d in drange:
                        nc.tensor.matmul(
                            acc[:, lo:hi],
                            lhsT=ZT[d][:, s * 128 : (s + 1) * 128],
                            rhs=WO[d][:, lo:hi],
                            start=(start and d == drange[0]),
                            stop=(d == NDT - 1),
                        )

            def o_full_out(s, acc):
                ot = out_pool.tile([128, DM], F32, tag="ostage", name="ostage")
                if "bo" in bias_tiles:
                    nc.vector.tensor_add(ot, acc, bias_tiles["bo"])
                else:
                    nc.scalar.activation(ot, acc, AF.Copy)
                eng = nc.scalar if s % 2 else nc.sync
                eng.dma_start(out=out[s * 128 : (s + 1) * 128, :], in_=ot)

            accs = {}
            for s in (4, 5):
                accs[s] = ps_st.tile(
                    [128, DM], F32, tag="st", name="oacc",
                    padded_shape=[128, 2 * QC],
                )
                o_full_mm(s, accs[s], list(range(NDT - 1)), start=True)
            # s=6 partials on the proj/z psum rings: more PE work ahead of
            # the ZT[5] dependency
            acc6 = {}
            for cc, (pool, tag) in enumerate(((ps_mm, "proj"), (ps_z, "z"))):
                a = pool.tile(
                    [128, VC], F32, tag=tag, name="oacc", padded_shape=[128, QC]
                )
                for d in range(NDT - 1):
                    nc.tensor.matmul(
                        a,
                        lhsT=ZT[d][:, 6 * 128 : 7 * 128],
                        rhs=WO[d][:, cc * VC : (cc + 1) * VC],
                        start=(d == 0),
                        stop=False,
                    )
                acc6[cc] = a
            for s in (4, 5):
                o_full_mm(s, accs[s], [NDT - 1], start=False)
                o_full_out(s, accs[s])
            ot6 = out_pool.tile([128, DM], F32, tag="ostage", name="ostage")
            for cc in (0, 1):
                nc.tensor.matmul(
                    acc6[cc],
                    lhsT=ZT[NDT - 1][:, 6 * 128 : 7 * 128],
                    rhs=WO[NDT - 1][:, cc * VC : (cc + 1) * VC],
                    start=False,
                    stop=True,
                )
                o = ot6[:, cc * VC : (cc + 1) * VC]
                if "bo" in bias_tiles:
                    nc.vector.tensor_add(
                        o, acc6[cc], bias_tiles["bo"][:, cc * VC : (cc + 1) * VC]
                    )
                else:
                    nc.scalar.activation(o, acc6[cc], AF.Copy)
            nc.sync.dma_start(out=out[6 * 128 : 7 * 128, :], in_=ot6)
            acc = ps_st.tile(
                [128, DM], F32, tag="st", name="oacc",
                padded_shape=[128, 2 * QC],
            )
            o_full_mm(7, acc, list(range(NDT)), start=True)
            o_full_out(7, acc)

            if debug_taps:
                for nm, tiles in (("QT_d", QT), ("KT_d", KT), ("ZT_d", ZT)):
                    for d in range(NDT):
                        nc.sync.dma_start(
                            out=taps[nm][d * 128 : (d + 1) * 128, :],
                            in_=tiles[d][:, :],
                        )
                for s in range(NQT):
                    nc.sync.dma_start(
                        out=taps["V_d"][s * 128 : (s + 1) * 128, :], in_=V[s][:, :]
                    )

    nc.compile()
    return nc


_CACHE = {}


def _get_nc(key):
    if key not in _CACHE:
        _CACHE[key] = build(*key)
    return _CACHE[key]


def _prep(inputs):
    BF = ml_dtypes.bfloat16
    x = np.asarray(inputs["normalized_resid_pre"], np.float32)
    wq = np.ascontiguousarray(
        np.asarray(inputs["W_Q"], np.float32).transpose(1, 0, 2).reshape(DM, DM)
    ).astype(BF)
    wk = np.ascontiguousarray(
        np.asarray(inputs["W_K"], np.float32).transpose(1, 0, 2).reshape(DM, DM)
    ).astype(BF)
    wv = np.ascontiguousarray(
        np.asarray(inputs["W_V"], np.float32).transpose(1, 0, 2).reshape(DM, DM)
    ).astype(BF)
    wo = np.ascontiguousarray(
        np.asarray(inputs["W_O"], np.float32).reshape(DM, DM)
    ).astype(BF)
    bq = np.asarray(inputs["b_Q"], np.float32).reshape(NDT, 128).T
    bk = np.asarray(inputs["b_K"], np.float32).reshape(NDT, 128).T
    bv = np.asarray(inputs["b_V"], np.float32).reshape(1, DM)
    bo = np.asarray(inputs["b_O"], np.float32).reshape(1, DM)
    jj, uu = np.meshgrid(np.arange(128), np.arange(128), indexing="ij")
    wmask = (uu >= jj).astype(BF)
    key = (
        bool(np.any(bq)),
        bool(np.any(bk)),
        bool(np.any(bv)),
        bool(np.any(bo)),
    )
    common = {
        "wq": wq, "wk": wk, "wv": wv, "wo": wo, "wmask": wmask,
        "identin": np.eye(128, dtype=np.float32).astype(BF),
    }
    if key[0]:
        common["bq"] = np.ascontiguousarray(bq)
    if key[1]:
        common["bk"] = np.ascontiguousarray(bk)
    if key[2]:
        common["bv"] = np.ascontiguousarray(bv)
    if key[3]:
        common["bo"] = np.ascontiguousarray(bo)
    in_maps = [
        dict(common, xt=np.ascontiguousarray(x[b].T).astype(BF))
        for b in range(BATCH)
    ]
    return key, in_maps


def run(inputs, trace=False, **kw):
    key, in_maps = _prep(inputs)
    nc = _get_nc(key)
    res = run_bass_kernel_spmd(
        nc, in_maps, core_ids=list(range(BATCH)), trace=trace, **kw
    )
    outs = np.stack([res.results[b]["out"] for b in range(BATCH)])
    return outs.astype(np.float32), res


def kernel(**inputs):
    out, _ = run(inputs)
    return out


if __name__ == "__main__":
    rng = np.random.default_rng(0)
    ins = {
        "normalized_resid_pre": rng.standard_normal((8, SEQ, DM)).astype(np.float32),
        "W_Q": (0.02 * rng.standard_normal((NH, DM, DH))).astype(np.float32),
        "b_Q": np.zeros((NH, DH), np.float32),
        "W_K": (0.02 * rng.standard_normal((NH, DM, DH))).astype(np.float32),
        "b_K": np.zeros((NH, DH), np.float32),
        "W_V": (0.02 * rng.standard_normal((NH, DM, DH))).astype(np.float32),
        "b_V": np.zeros((NH, DH), np.float32),
        "W_O": (0.02 * rng.standard_normal((NH, DH, DM))).astype(np.float32),
        "b_O": np.zeros((DM,), np.float32),
    }
    out = kernel(**ins)
    print("kernel output", out.shape, out.dtype, float(np.abs(out).max()))


# revision 36
# speedup vs baseline: 1.1817x; 1.0063x over previous
"""Causal multi-head attention on 8 Trainium2 NeuronCores.

Problem: nn_Attention_46643344835180
  x: [8, 1024, 768], 12 heads x 64 dh, causal softmax attention + output proj.

Sharding: data-parallel over batch (8 batch elements -> 8 cores, no collectives).

v3: full bf16 compute (PSUM stays f32), host-side transpose of x (xT fed
directly), weights resident in SBUF, dual DMA rings (sync + scalar HWDGE),
QK/V psum->sbuf copies on the Scalar engine (idle during the projection
phase), and a globally software-pipelined attention sweep: scores of group
g+1 are issued before exp/PV of group g, with projection/output chunks
spread between groups as PE filler.

Per-core dataflow (batch element b):
  xT = x_b.T (host)                                                  [768, 1024]
  QT = Wq.T @ xT  (+bq)            heads stacked on partitions       [768, 1024]
  KT = Wk.T @ xT  (+bk)                                              [768, 1024]
  V  = x_b @ Wv   (+bv)            + interleaved ones column         [1024, 12*66]
  per head h, query-chunk qc (512):
    S^T[k,q] = KT_h.T @ QT_h          keys on partitions
    P^T = exp(S^T / 8)                ScalarE, batched over 2 key-blocks
    causal: one 128-wide-mask multiply per diagonal block
    z^T[65,512] += [V_h | 1].T @ P^T  row 64 accumulates the denominator
    ZT_h = z^T[0:64] * approx(1/z^T[64])   (denom -> reciprocal ->
           gpsimd partition_broadcast -> multiply)
  out = ZT.T @ Wo (+bo)                                              [1024, 768]
"""

import sys

sys.path.insert(0, "/opt/trn_rl_repo")

from collections import deque

import ml_dtypes
import numpy as np

import concourse.bass as bass
import concourse.mybir as mybir
import concourse.tile as tile
from concourse import bacc
from concourse.bass_utils import run_bass_kernel_spmd

F32 = mybir.dt.float32
BF16 = mybir.dt.bfloat16
AF = mybir.ActivationFunctionType

SEQ = 1024
DM = 768
NH = 12
DH = 64
VH = DH + 2  # V head stride: 64 dims + ones col + pad (keeps 4B alignment)
BATCH = 8
NQT = SEQ // 128  # 8 seq tiles of 128
NDT = DM // 128  # 6 d_model tiles
QC = 512  # query chunk (moving dim)
NQC = SEQ // QC  # 2
WARMUP = 64  # HAM warmup matmuls (bf16 N=128, ~110ns each cold)


def build(with_bq, with_bk, with_bv, with_bo, debug_taps=False):
    nc = bacc.Bacc("TRN2", target_bir_lowering=False, debug=False)

    xt = nc.dram_tensor("xt", [DM, SEQ], BF16, kind="ExternalInput")
    wq = nc.dram_tensor("wq", [DM, DM], BF16, kind="ExternalInput")
    wk = nc.dram_tensor("wk", [DM, DM], BF16, kind="ExternalInput")
    wv = nc.dram_tensor("wv", [DM, DM], BF16, kind="ExternalInput")
    wo = nc.dram_tensor("wo", [DM, DM], BF16, kind="ExternalInput")
    wmask = nc.dram_tensor("wmask", [128, 256], BF16, kind="ExternalInput")
    identin = nc.dram_tensor("identin", [128, 128], BF16, kind="ExternalInput")
    bq = bk = bv = bo = None
    if with_bq:
        bq = nc.dram_tensor("bq", [128, NDT], F32, kind="ExternalInput")
    if with_bk:
        bk = nc.dram_tensor("bk", [128, NDT], F32, kind="ExternalInput")
    if with_bv:
        bv = nc.dram_tensor("bv", [1, DM], F32, kind="ExternalInput")
    if with_bo:
        bo = nc.dram_tensor("bo", [1, DM], F32, kind="ExternalInput")
    out = nc.dram_tensor("out", [SEQ, DM], F32, kind="ExternalOutput")
    taps = {}
    if debug_taps:
        for nm in ("QT_d", "KT_d", "ZT_d"):
            taps[nm] = nc.dram_tensor(nm, [DM, SEQ], BF16, kind="ExternalOutput")
        taps["V_d"] = nc.dram_tensor(
            "V_d", [SEQ, NH * VH], BF16, kind="ExternalOutput"
        )

    with tile.TileContext(nc) as tc:
        with (
            tc.tile_pool(name="persist", bufs=1) as persist,
            tc.tile_pool(name="pt", bufs=4) as pt_pool,
            tc.tile_pool(name="small", bufs=2) as small,
            tc.tile_pool(name="outst", bufs=2) as out_pool,
            tc.tile_pool(name="ps_st", bufs=2, space="PSUM") as ps_st,
            tc.tile_pool(name="ps_z", bufs=3, space="PSUM") as ps_z,
            tc.tile_pool(name="ps_mm", bufs=1, space="PSUM") as ps_mm,
        ):
            # ---- sync ring: ident, xT, WV, mask/ones.  scalar ring: wq/wk
            # (column-split so head pairs 0-1 unblock early), wo later ----
            ident = persist.tile([128, 128], BF16, tag="ident", name="ident")
            nc.sync.dma_start(out=ident, in_=identin[:, :])
            warm_ps = ps_mm.tile(
                [128, 128], F32, tag="proj", name="warm", padded_shape=[128, QC]
            )
            for _ in range(WARMUP):
                nc.tensor.matmul(warm_ps, lhsT=ident, rhs=ident, start=True, stop=True)

            # single sync ring for all inputs in priority order (the two
            # HWDGE rings share ~210GB/s of HBM read bandwidth, and DMAs on
            # the scalar ring block the ACT compute stream behind them).
            # Startup-critical set first: xt query-half 0, wq/wk cols for
            # head pairs 0-1, mask, WV.  xt half 1 and the remaining weight
            # columns are issued after the first projection chunks (below).
            xT = [
                persist.tile([128, SEQ], BF16, tag=f"xT{d}", name=f"xT{d}")
                for d in range(NDT)
            ]
            for d in range(NDT):
                nc.sync.dma_start(
                    out=xT[d][:, 0:QC], in_=xt[d * 128 : (d + 1) * 128, 0:QC]
                )

            WQ = [
                persist.tile([128, DM], BF16, tag=f"WQ{d}", name=f"WQ{d}")
                for d in range(NDT)
            ]
            WK = [
                persist.tile([128, DM], BF16, tag=f"WK{d}", name=f"WK{d}")
                for d in range(NDT)
            ]
            WV = [
                persist.tile([128, DM], BF16, tag=f"WV{d}", name=f"WV{d}")
                for d in range(NDT)
            ]
            for src, dst in ((wq, WQ), (wk, WK)):
                for d in range(NDT):
                    nc.sync.dma_start(
                        out=dst[d][:, 0:256],
                        in_=src[d * 128 : (d + 1) * 128, 0:256],
                    )

            wm_t = persist.tile([128, 256], BF16, tag="wmask", name="wmask")
            nc.sync.dma_start(out=wm_t, in_=wmask[:, :])

            for d in range(NDT):
                nc.sync.dma_start(out=WV[d], in_=wv[d * 128 : (d + 1) * 128, :])

            bias_tiles = {}
            if with_bq:
                t = persist.tile([128, NDT], F32, tag="bq", name="bq")
                nc.scalar.dma_start(out=t, in_=bq[:, :])
                bias_tiles["bq"] = t
            if with_bk:
                t = persist.tile([128, NDT], F32, tag="bk", name="bk")
                nc.scalar.dma_start(out=t, in_=bk[:, :])
                bias_tiles["bk"] = t
            if with_bv:
                t = persist.tile([128, DM], F32, tag="bv", name="bv")
                nc.scalar.dma_start(out=t, in_=bv[0:1, :].to_broadcast((128, DM)))
                bias_tiles["bv"] = t
            if with_bo:
                t = persist.tile([128, DM], F32, tag="bo", name="bo")
                nc.scalar.dma_start(out=t, in_=bo[0:1, :].to_broadcast((128, DM)))
                bias_tiles["bo"] = t

            QT = [
                persist.tile([128, SEQ], BF16, tag=f"QT{d}", name=f"QT{d}")
                for d in range(NDT)
            ]
            KT = [
                persist.tile([128, SEQ], BF16, tag=f"KT{d}", name=f"KT{d}")
                for d in range(NDT)
            ]
            V = [
                persist.tile([128, NH * VH], BF16, tag=f"V{s}", name=f"V{s}")
                for s in range(NQT)
            ]
            for s in range(NQT):
                vv = V[s].rearrange("p (h e) -> p h e", e=VH)
                nc.vector.memset(vv[:, :, DH : DH + 1], 1.0)
            ZT = [
                persist.tile([128, SEQ], BF16, tag=f"ZT{d}", name=f"ZT{d}")
                for d in range(NDT)
            ]

            # ---- projection chunks (each = one psum round trip) ----
            def qk_chunk(hp, which, c, pool, tag):
                W, dst, bkey = (
                    (WQ, QT, "bq") if which == "q" else (WK, KT, "bk")
                )
                acc = pool.tile(
                    [128, QC], F32, tag=tag, name="proj",
                    padded_shape=[128, 2 * QC] if tag == "st" else [128, QC],
                )
                for d in range(NDT):
                    nc.tensor.matmul(
                        acc,
                        lhsT=W[d][:, hp * 128 : (hp + 1) * 128],
                        rhs=xT[d][:, c * QC : (c + 1) * QC],
                        start=(d == 0),
                        stop=(d == NDT - 1),
                    )
                o = dst[hp][:, c * QC : (c + 1) * QC]
                if bkey in bias_tiles:
                    nc.vector.tensor_scalar_add(
                        o, acc, bias_tiles[bkey][:, hp : hp + 1]
                    )
                else:
                    nc.vector.tensor_copy(o, acc)

            def qk_chunks(hp):
                return [
                    (lambda which=which, c=c: qk_chunk(hp, which, c, ps_mm, "proj"))
                    for which in ("q", "k")
                    for c in range(NQC)
                ]

            NVC = 2
            VC = DM // NVC  # 384

            def v_chunk(s, c, on_act=True):
                acc = ps_st.tile(
                    [128, VC], F32, tag="st", name="vacc",
                    padded_shape=[128, 2 * QC],
                )
                for d in range(NDT):
                    nc.tensor.matmul(
                        acc,
                        lhsT=xT[d][:, s * 128 : (s + 1) * 128],
                        rhs=WV[d][:, c * VC : (c + 1) * VC],
                        start=(d == 0),
                        stop=(d == NDT - 1),
                    )
                nh2 = VC // DH  # heads per chunk (6)
                o = V[s].rearrange("p (h e) -> p h e", e=VH)[
                    :, c * nh2 : (c + 1) * nh2, 0:DH
                ]
                if "bv" in bias_tiles:
                    nc.vector.tensor_add(
                        o,
                        acc.rearrange("p (h e) -> p h e", e=DH),
                        bias_tiles["bv"][:, c * VC : (c + 1) * VC].rearrange(
                            "p (h e) -> p h e", e=DH
                        ),
                    )
                elif on_act:
                    nc.scalar.activation(
                        o, acc.rearrange("p (h e) -> p h e", e=DH), AF.Copy
                    )
                else:
                    nc.vector.tensor_copy(
                        o, acc.rearrange("p (h e) -> p h e", e=DH)
                    )

            def v_chunks(s):
                return [lambda c=c: v_chunk(s, c) for c in range(NVC)]

            WO = []

            def wo_load():
                # sync ring: idle mid-attention (scalar ring would block ACT)
                for d in range(NDT):
                    t = persist.tile([128, DM], BF16, tag=f"WO{d}", name=f"WO{d}")
                    nc.sync.dma_start(out=t, in_=wo[d * 128 : (d + 1) * 128, :])
                    WO.append(t)

            def o_chunks(s):
                ot = [None]

                def chunk(c):
                    if c == 0:
                        ot[0] = out_pool.tile([128, DM], F32, tag="ostage", name="ostage")
                    pool, tag = ((ps_mm, "proj"), (ps_z, "z"))[c % 2]
                    acc = pool.tile(
                        [128, VC], F32, tag=tag, name="oacc",
                        padded_shape=[128, QC],
                    )
                    for d in range(NDT):
                        nc.tensor.matmul(
                            acc,
                            lhsT=ZT[d][:, s * 128 : (s + 1) * 128],
                            rhs=WO[d][:, c * VC : (c + 1) * VC],
                            start=(d == 0),
                            stop=(d == NDT - 1),
                        )
                    o = ot[0][:, c * VC : (c + 1) * VC]
                    if "bo" in bias_tiles:
                        nc.vector.tensor_add(
                            o, acc, bias_tiles["bo"][:, c * VC : (c + 1) * VC]
                        )
                    else:
                        # DVE: the ACT stream is exp-saturated mid-phase and
                        # an in-order ACT copy would delay psum recycling
                        nc.vector.tensor_copy(o, acc)
                    if c == NVC - 1:
                        # rows 512+ drain at the very end: use the scalar
                        # ring (idle by then) so the tail DMAs overlap
                        eng = nc.sync if s < 4 else nc.scalar
                        eng.dma_start(
                            out=out[s * 128 : (s + 1) * 128, :], in_=ot[0]
                        )

                return [lambda c=c: chunk(c) for c in range(NVC)]

            # ---- pipelined attention sweep ----
            zps_of = {}

            def issue_scores(hp, c, g, gsz):
                doffs = [max(0, (g + j) * 128 - c * QC) for j in range(gsz)]
                sts = {}
                for px in (0, 64):
                    sts[px] = ps_st.tile([128, gsz * QC], F32, tag="st", name="st")
                for j in range(gsz):
                    kb = g + j
                    off = doffs[j]
                    for px in (0, 64):
                        nc.tensor.matmul(
                            sts[px][:, j * QC + off : (j + 1) * QC],
                            lhsT=KT[hp][px : px + 64, kb * 128 : (kb + 1) * 128],
                            rhs=QT[hp][px : px + 64, c * QC + off : (c + 1) * QC],
                            start=True,
                            stop=True,
                        )
                return sts, doffs

            def issue_expv(hp, c, g, gsz, sts, doffs, last):
                nkb = 4 * (c + 1)
                if g == 0:
                    zps_of[(hp, c)] = {
                        px: ps_z.tile([128, QC], F32, tag="z", name="z")
                        for px in (0, 64)
                    }
                zps = zps_of[(hp, c)]
                pts = {}
                for px in (0, 64):
                    pt = pt_pool.tile([128, 2 * QC], BF16, tag="pt", name="pt")
                    nc.scalar.activation(
                        pt[:, doffs[0] : gsz * QC],
                        sts[px][:, doffs[0] : gsz * QC],
                        AF.Exp,
                        scale=0.125,
                    )
                    pts[px] = pt
                # diagonal groups (both blocks straddle the diagonal): one
                # paired mask multiply per px covers both 128-wide triangles
                # via a stride-640 view (blocks sit at doff0 + 640*j)
                if g * 128 - c * QC >= 0:
                    base = doffs[0]
                    msk = wm_t[:, :].rearrange("p (a b) -> p a b", b=128)
                    for px in (0, 64):
                        blk = pts[px][:, base : base + 768].rearrange(
                            "p (a b) -> p a b", b=128
                        )[:, 0:6:5, :]
                        nc.vector.tensor_mul(blk, blk, msk)
                for j in range(gsz):
                    kb = g + j
                    off = doffs[j]
                    for px in (0, 64):
                        pt = pts[px]
                        h = 2 * hp + (1 if px else 0)
                        nc.tensor.matmul(
                            zps[px][0 : DH + 1, off:QC],
                            lhsT=V[kb][:, h * VH : h * VH + DH + 1],
                            rhs=pt[:, j * QC + off : (j + 1) * QC],
                            start=(kb == 0),
                            stop=(kb == nkb - 1),
                        )
                if last:
                    for px in (0, 64):
                        dstage = small.tile([128, QC], F32, tag="dstage", name="dstage")
                        nc.vector.tensor_copy(dstage[0:1, :], zps[px][DH : DH + 1, :])
                        recip = small.tile([128, QC], F32, tag="recip", name="recip")
                        nc.vector.reciprocal_approx_fast(recip[0:1, :], dstage[0:1, :])
                        bcast = small.tile([64, QC], F32, tag="bcast", name="bcast")
                        nc.gpsimd.partition_broadcast(bcast, recip[0:1, :])
                        nc.vector.tensor_mul(
                            ZT[hp][px : px + 64, c * QC : (c + 1) * QC],
                            zps[px][0:64, :],
                            bcast,
                        )
                    del zps_of[(hp, c)]

            # ---- pre-phase: project heads 0-1 for query half 0 only (the
            # qc=1 halves are computed as fillers much later), then issue
            # the non-critical DMAs, then V tiles 0-1 ----
            qk_chunk(0, "q", 0, ps_mm, "proj")
            qk_chunk(0, "k", 0, ps_st, "st")
            for d in range(NDT):
                nc.sync.dma_start(
                    out=xT[d][:, QC:SEQ], in_=xt[d * 128 : (d + 1) * 128, QC:SEQ]
                )
            # remaining weight columns in one batched piece per row tile
            # (each DMA has a ~600ns floor: fewer, larger transfers win)
            for src, dst in ((wq, WQ), (wk, WK)):
                for d in range(NDT):
                    nc.sync.dma_start(
                        out=dst[d][:, 256:DM],
                        in_=src[d * 128 : (d + 1) * 128, 256:DM],
                    )

            # qc=0 and qc=1 units interleaved: spreads the exp-heavy qc=1
            # units (ACT-bound) across the whole span instead of
            # back-loading them.  Every unit gets filler chunks so exp
            # latency is always hidden behind interposed PE work.
            # the first four units need only the startup-critical DMA set
            # (xt + head-pair-0/1 weight columns); later head pairs' weight
            # columns stream in per-pair just ahead of first use
            units = [
                (0, 0), (1, 0), (0, 1), (1, 1), (2, 0), (3, 0),
                (2, 1), (4, 0), (5, 0), (3, 1), (4, 1), (5, 1),
            ]

            def vc(s, c):
                return lambda: v_chunk(s, c)

            def qkc(hp, which, c):
                return lambda: qk_chunk(hp, which, c, ps_mm, "proj")

            fillers = {
                # v0/v1 here (not pre-phase): their WV-gated matmuls must
                # not sit in the PE stream ahead of the first scores.
                # DVE copies so the first exps aren't queued behind them.
                0: [
                    lambda: v_chunk(0, 0, on_act=False),
                    lambda: v_chunk(0, 1, on_act=False),
                    lambda: v_chunk(1, 0, on_act=False),
                    lambda: v_chunk(1, 1, on_act=False),
                    qkc(1, "q", 0), qkc(1, "k", 0), vc(2, 0), vc(3, 0),
                ],
                1: [qkc(0, "q", 1), qkc(0, "k", 1), vc(4, 0), vc(5, 0)],
                2: [qkc(1, "q", 1), qkc(1, "k", 1), vc(6, 0), vc(7, 0)],
                3: [qkc(2, "q", 0), qkc(2, "k", 0), vc(2, 1), vc(3, 1)],
                4: [qkc(3, "q", 0), qkc(3, "k", 0), vc(4, 1)],
                5: [qkc(2, "q", 1), qkc(2, "k", 1), vc(5, 1)],
                6: [qkc(4, "q", 0), qkc(4, "k", 0), vc(6, 1)],
                7: [qkc(5, "q", 0), qkc(5, "k", 0), vc(7, 1)],
                8: [qkc(3, "q", 1), qkc(3, "k", 1), wo_load],
                9: o_chunks(0) + [qkc(4, "q", 1), qkc(4, "k", 1)],
                10: o_chunks(1) + o_chunks(2) + [qkc(5, "q", 1), qkc(5, "k", 1)],
                11: o_chunks(3),
            }
            # units whose fillers read ZT written by the pending finalizer:
            # flush before popping fillers there (issue-order correctness)
            flush_first = {9, 10, 11}

            pending = [None]

            def flush():
                if pending[0] is not None:
                    fn = pending[0]
                    pending[0] = None
                    fn()

            for ui, (hp, c) in enumerate(units):
                nkb = 4 * (c + 1)
                glist = [(g, min(2, nkb - g)) for g in range(0, nkb, 2)]
                chunks = deque(fillers.get(ui, []))
                n = len(glist)
                for gi, (g, gsz) in enumerate(glist):
                    sts, doffs = issue_scores(hp, c, g, gsz)
                    if ui in flush_first:
                        flush()
                    k = -(-len(chunks) // (n - gi)) if chunks else 0
                    for i in range(k):
                        chunks.popleft()()
                        if i == 0:
                            flush()
                    if k == 0:
                        flush()
                    pending[0] = (
                        lambda hp=hp, c=c, g=g, gsz=gsz, sts=sts, doffs=doffs,
                        last=(gi == n - 1): issue_expv(hp, c, g, gsz, sts, doffs, last)
                    )
            flush()

            # ---- tail: output rows 512-1024.  Full-width accumulation on
            # the freed scores psum, one ACT copy (ACT is idle by now), out
            # DMAs alternating between the two rings.  s=4/5 accumulate
            # d=0..4 first so the PE is busy while the last unit's
            # normalization chain (recip/broadcast/ZT-mul for ZT[5]) runs.
            def o_full_mm(s, acc, drange, start):
                for lo, hi in ((0, QC), (QC, DM)):  # <=512 f32 cols per MM
                    for d in drange:
                        nc.tensor.matmul(
                            acc[:, lo:hi],
                            lhsT=ZT[d][:, s * 128 : (s + 1) * 128],
                            rhs=WO[d][:, lo:hi],
                            start=(start and d == drange[0]),
                            stop=(d == NDT - 1),
                        )

            def o_full_out(s, acc):
                ot = out_pool.tile([128, DM], F32, tag="ostage", name="ostage")
                if "bo" in bias_tiles:
                    nc.vector.tensor_add(ot, acc, bias_tiles["bo"])
                else:
                    nc.scalar.activation(ot, acc, AF.Copy)
                eng = nc.scalar if s % 2 else nc.sync
                eng.dma_start(out=out[s * 128 : (s + 1) * 128, :], in_=ot)

            accs = {}
            for s in (4, 5):
                accs[s] = ps_st.tile(
                    [128, DM], F32, tag="st", name="oacc",
                    padded_shape=[128, 2 * QC],
                )
                o_full_mm(s, accs[s], list(range(NDT - 1)), start=True)
            # s=6 partials on the proj/z psum rings: more PE work ahead of
            # the ZT[5] dependency
            acc6 = {}
            for cc, (pool, tag) in enumerate(((ps_mm, "proj"), (ps_z, "z"))):
                a = pool.tile(
                    [128, VC], F32, tag=tag, name="oacc", padded_shape=[128, QC]
                )
                for d in range(NDT - 1):
                    nc.tensor.matmul(
                        a,
                        lhsT=ZT[d][:, 6 * 128 : 7 * 128],
                        rhs=WO[d][:, cc * VC : (cc + 1) * VC],
                        start=(d == 0),
                        stop=False,
                    )
                acc6[cc] = a
            for s in (4, 5):
                o_full_mm(s, accs[s], [NDT - 1], start=False)
                o_full_out(s, accs[s])
            ot6 = out_pool.tile([128, DM], F32, tag="ostage", name="ostage")
            for cc in (0, 1):
                nc.tensor.matmul(
                    acc6[cc],
                    lhsT=ZT[NDT - 1][:, 6 * 128 : 7 * 128],
                    rhs=WO[NDT - 1][:, cc * VC : (cc + 1) * VC],
                    start=False,
                    stop=True,
                )
                o = ot6[:, cc * VC : (cc + 1) * VC]
                if "bo" in bias_tiles:
                    nc.vector.tensor_add(
                        o, acc6[cc], bias_tiles["bo"][:, cc * VC : (cc + 1) * VC]
                    )
                else:
                    nc.scalar.activation(o, acc6[cc], AF.Copy)
            nc.sync.dma_start(out=out[6 * 128 : 7 * 128, :], in_=ot6)
            acc = ps_st.tile(
                [128, DM], F32, tag="st", name="oacc",
                padded_shape=[128, 2 * QC],
            )
            o_full_mm(7, acc, list(range(NDT)), start=True)
            o_full_out(7, acc)

            if debug_taps:
                for nm, tiles in (("QT_d", QT), ("KT_d", KT), ("ZT_d", ZT)):
                    for d in range(NDT):
                        nc.sync.dma_start(
                            out=taps[nm][d * 128 : (d + 1) * 128, :],
                            in_=tiles[d][:, :],
                        )
                for s in range(NQT):
                    nc.sync.dma_start(
                        out=taps["V_d"][s * 128 : (s + 1) * 128, :], in_=V[s][:, :]
                    )

    nc.compile()
    return nc


_CACHE = {}


def _get_nc(key):
    if key not in _CACHE:
        _CACHE[key] = build(*key)
    return _CACHE[key]


def _prep(inputs):
    BF = ml_dtypes.bfloat16
    x = np.asarray(inputs["normalized_resid_pre"], np.float32)
    wq = np.ascontiguousarray(
        np.asarray(inputs["W_Q"], np.float32).transpose(1, 0, 2).reshape(DM, DM)
    ).astype(BF)
    wk = np.ascontiguousarray(
        np.asarray(inputs["W_K"], np.float32).transpose(1, 0, 2).reshape(DM, DM)
    ).astype(BF)
    wv = np.ascontiguousarray(
        np.asarray(inputs["W_V"], np.float32).transpose(1, 0, 2).reshape(DM, DM)
    ).astype(BF)
    wo = np.ascontiguousarray(
        np.asarray(inputs["W_O"], np.float32).reshape(DM, DM)
    ).astype(BF)
    bq = np.asarray(inputs["b_Q"], np.float32).reshape(NDT, 128).T
    bk = np.asarray(inputs["b_K"], np.float32).reshape(NDT, 128).T
    bv = np.asarray(inputs["b_V"], np.float32).reshape(1, DM)
    bo = np.asarray(inputs["b_O"], np.float32).reshape(1, DM)
    jj, uu = np.meshgrid(np.arange(128), np.arange(128), indexing="ij")
    wmask = np.tile((uu >= jj).astype(BF), (1, 2))
    key = (
        bool(np.any(bq)),
        bool(np.any(bk)),
        bool(np.any(bv)),
        bool(np.any(bo)),
    )
    common = {
        "wq": wq, "wk": wk, "wv": wv, "wo": wo, "wmask": wmask,
        "identin": np.eye(128, dtype=np.float32).astype(BF),
    }
    if key[0]:
        common["bq"] = np.ascontiguousarray(bq)
    if key[1]:
        common["bk"] = np.ascontiguousarray(bk)
    if key[2]:
        common["bv"] = np.ascontiguousarray(bv)
    if key[3]:
        common["bo"] = np.ascontiguousarray(bo)
    in_maps = [
        dict(common, xt=np.ascontiguousarray(x[b].T).astype(BF))
        for b in range(BATCH)
    ]
    return key, in_maps


def run(inputs, trace=False, **kw):
    key, in_maps = _prep(inputs)
    nc = _get_nc(key)
    res = run_bass_kernel_spmd(
        nc, in_maps, core_ids=list(range(BATCH)), trace=trace, **kw
    )
    outs = np.stack([res.results[b]["out"] for b in range(BATCH)])
    return outs.astype(np.float32), res


def kernel(**inputs):
    out, _ = run(inputs)
    return out


if __name__ == "__main__":
    rng = np.random.default_rng(0)
    ins = {
        "normalized_resid_pre": rng.standard_normal((8, SEQ, DM)).astype(np.float32),
        "W_Q": (0.02 * rng.standard_normal((NH, DM, DH))).astype(np.float32),
        "b_Q": np.zeros((NH, DH), np.float32),
        "W_K": (0.02 * rng.standard_normal((NH, DM, DH))).astype(np.float32),
        "b_K": np.zeros((NH, DH), np.float32),
        "W_V": (0.02 * rng.standard_normal((NH, DM, DH))).astype(np.float32),
        "b_V": np.zeros((NH, DH), np.float32),
        "W_O": (0.02 * rng.standard_normal((NH, DH, DM))).astype(np.float32),
        "b_O": np.zeros((DM,), np.float32),
    }
    out = kernel(**ins)
    print("kernel output", out.shape, out.dtype, float(np.abs(out).max()))


# revision 38
# speedup vs baseline: 1.1962x; 1.0123x over previous
"""Causal multi-head attention on 8 Trainium2 NeuronCores.

Problem: nn_Attention_46643344835180
  x: [8, 1024, 768], 12 heads x 64 dh, causal softmax attention + output proj.

Sharding: data-parallel over batch (8 batch elements -> 8 cores, no collectives).

v3: full bf16 compute (PSUM stays f32), host-side transpose of x (xT fed
directly), weights resident in SBUF, dual DMA rings (sync + scalar HWDGE),
QK/V psum->sbuf copies on the Scalar engine (idle during the projection
phase), and a globally software-pipelined attention sweep: scores of group
g+1 are issued before exp/PV of group g, with projection/output chunks
spread between groups as PE filler.

Per-core dataflow (batch element b):
  xT = x_b.T (host)                                                  [768, 1024]
  QT = Wq.T @ xT  (+bq)            heads stacked on partitions       [768, 1024]
  KT = Wk.T @ xT  (+bk)                                              [768, 1024]
  V  = x_b @ Wv   (+bv)            + interleaved ones column         [1024, 12*66]
  per head h, query-chunk qc (512):
    S^T[k,q] = KT_h.T @ QT_h          keys on partitions
    P^T = exp(S^T / 8)                ScalarE, batched over 2 key-blocks
    causal: one 128-wide-mask multiply per diagonal block
    z^T[65,512] += [V_h | 1].T @ P^T  row 64 accumulates the denominator
    ZT_h = z^T[0:64] * approx(1/z^T[64])   (denom -> reciprocal ->
           gpsimd partition_broadcast -> multiply)
  out = ZT.T @ Wo (+bo)                                              [1024, 768]
"""

import sys

sys.path.insert(0, "/opt/trn_rl_repo")

from collections import deque

import ml_dtypes
import numpy as np

import concourse.bass as bass
import concourse.mybir as mybir
import concourse.tile as tile
from concourse import bacc
from concourse.bass_utils import run_bass_kernel_spmd

F32 = mybir.dt.float32
BF16 = mybir.dt.bfloat16
AF = mybir.ActivationFunctionType

SEQ = 1024
DM = 768
NH = 12
DH = 64
VH = DH + 2  # V head stride: 64 dims + ones col + pad (keeps 4B alignment)
BATCH = 8
NQT = SEQ // 128  # 8 seq tiles of 128
NDT = DM // 128  # 6 d_model tiles
QC = 512  # query chunk (moving dim)
NQC = SEQ // QC  # 2
WARMUP = 64  # HAM warmup matmuls (bf16 N=128, ~110ns each cold)


def build(with_bq, with_bk, with_bv, with_bo, debug_taps=False):
    nc = bacc.Bacc("TRN2", target_bir_lowering=False, debug=False)

    xt = nc.dram_tensor("xt", [DM, SEQ], BF16, kind="ExternalInput")
    wq = nc.dram_tensor("wq", [DM, DM], BF16, kind="ExternalInput")
    wk = nc.dram_tensor("wk", [DM, DM], BF16, kind="ExternalInput")
    wv = nc.dram_tensor("wv", [DM, DM], BF16, kind="ExternalInput")
    wo = nc.dram_tensor("wo", [DM, DM], BF16, kind="ExternalInput")
    wmask = nc.dram_tensor("wmask", [128, 256], BF16, kind="ExternalInput")
    identin = nc.dram_tensor("identin", [128, 128], BF16, kind="ExternalInput")
    bq = bk = bv = bo = None
    if with_bq:
        bq = nc.dram_tensor("bq", [128, NDT], F32, kind="ExternalInput")
    if with_bk:
        bk = nc.dram_tensor("bk", [128, NDT], F32, kind="ExternalInput")
    if with_bv:
        bv = nc.dram_tensor("bv", [1, DM], F32, kind="ExternalInput")
    if with_bo:
        bo = nc.dram_tensor("bo", [1, DM], F32, kind="ExternalInput")
    out = nc.dram_tensor("out", [SEQ, DM], F32, kind="ExternalOutput")
    taps = {}
    if debug_taps:
        for nm in ("QT_d", "KT_d", "ZT_d"):
            taps[nm] = nc.dram_tensor(nm, [DM, SEQ], BF16, kind="ExternalOutput")
        taps["V_d"] = nc.dram_tensor(
            "V_d", [SEQ, NH * VH], BF16, kind="ExternalOutput"
        )

    with tile.TileContext(nc) as tc:
        with (
            tc.tile_pool(name="persist", bufs=1) as persist,
            tc.tile_pool(name="pt", bufs=4) as pt_pool,
            tc.tile_pool(name="small", bufs=2) as small,
            tc.tile_pool(name="outst", bufs=2) as out_pool,
            tc.tile_pool(name="ps_st", bufs=2, space="PSUM") as ps_st,
            tc.tile_pool(name="ps_z", bufs=3, space="PSUM") as ps_z,
            tc.tile_pool(name="ps_mm", bufs=1, space="PSUM") as ps_mm,
        ):
            # ---- sync ring: ident, xT, WV, mask/ones.  scalar ring: wq/wk
            # (column-split so head pairs 0-1 unblock early), wo later ----
            ident = persist.tile([128, 128], BF16, tag="ident", name="ident")
            nc.sync.dma_start(out=ident, in_=identin[:, :])
            warm_ps = ps_mm.tile(
                [128, 128], F32, tag="proj", name="warm", padded_shape=[128, QC]
            )
            for _ in range(WARMUP):
                nc.tensor.matmul(warm_ps, lhsT=ident, rhs=ident, start=True, stop=True)

            # single sync ring for all inputs in priority order (the two
            # HWDGE rings share ~210GB/s of HBM read bandwidth, and DMAs on
            # the scalar ring block the ACT compute stream behind them).
            # Startup-critical set first: xt query-half 0, wq/wk cols for
            # head pairs 0-1, mask, WV.  xt half 1 and the remaining weight
            # columns are issued after the first projection chunks (below).
            xT = [
                persist.tile([128, SEQ], BF16, tag=f"xT{d}", name=f"xT{d}")
                for d in range(NDT)
            ]
            for d in range(NDT):
                nc.sync.dma_start(
                    out=xT[d][:, 0:QC], in_=xt[d * 128 : (d + 1) * 128, 0:QC]
                )

            WQ = [
                persist.tile([128, DM], BF16, tag=f"WQ{d}", name=f"WQ{d}")
                for d in range(NDT)
            ]
            WK = [
                persist.tile([128, DM], BF16, tag=f"WK{d}", name=f"WK{d}")
                for d in range(NDT)
            ]
            WV = [
                persist.tile([128, DM], BF16, tag=f"WV{d}", name=f"WV{d}")
                for d in range(NDT)
            ]
            # full-row weight DMAs: 1536B partition lines run ~3x the
            # bandwidth of 512B column-piece lines, and fewer DMAs clear
            # the ring sooner for WV / xt half 1
            for src, dst in ((wq, WQ), (wk, WK)):
                for d in range(NDT):
                    nc.sync.dma_start(out=dst[d], in_=src[d * 128 : (d + 1) * 128, :])

            wm_t = persist.tile([128, 256], BF16, tag="wmask", name="wmask")
            nc.sync.dma_start(out=wm_t, in_=wmask[:, :])

            for d in range(NDT):
                nc.sync.dma_start(out=WV[d], in_=wv[d * 128 : (d + 1) * 128, :])

            bias_tiles = {}
            if with_bq:
                t = persist.tile([128, NDT], F32, tag="bq", name="bq")
                nc.scalar.dma_start(out=t, in_=bq[:, :])
                bias_tiles["bq"] = t
            if with_bk:
                t = persist.tile([128, NDT], F32, tag="bk", name="bk")
                nc.scalar.dma_start(out=t, in_=bk[:, :])
                bias_tiles["bk"] = t
            if with_bv:
                t = persist.tile([128, DM], F32, tag="bv", name="bv")
                nc.scalar.dma_start(out=t, in_=bv[0:1, :].to_broadcast((128, DM)))
                bias_tiles["bv"] = t
            if with_bo:
                t = persist.tile([128, DM], F32, tag="bo", name="bo")
                nc.scalar.dma_start(out=t, in_=bo[0:1, :].to_broadcast((128, DM)))
                bias_tiles["bo"] = t

            QT = [
                persist.tile([128, SEQ], BF16, tag=f"QT{d}", name=f"QT{d}")
                for d in range(NDT)
            ]
            KT = [
                persist.tile([128, SEQ], BF16, tag=f"KT{d}", name=f"KT{d}")
                for d in range(NDT)
            ]
            V = [
                persist.tile([128, NH * VH], BF16, tag=f"V{s}", name=f"V{s}")
                for s in range(NQT)
            ]
            for s in range(NQT):
                vv = V[s].rearrange("p (h e) -> p h e", e=VH)
                nc.vector.memset(vv[:, :, DH : DH + 1], 1.0)
            ZT = [
                persist.tile([128, SEQ], BF16, tag=f"ZT{d}", name=f"ZT{d}")
                for d in range(NDT)
            ]

            # ---- projection chunks (each = one psum round trip) ----
            def qk_chunk(hp, which, c, pool, tag):
                W, dst, bkey = (
                    (WQ, QT, "bq") if which == "q" else (WK, KT, "bk")
                )
                acc = pool.tile(
                    [128, QC], F32, tag=tag, name="proj",
                    padded_shape=[128, 2 * QC] if tag == "st" else [128, QC],
                )
                for d in range(NDT):
                    nc.tensor.matmul(
                        acc,
                        lhsT=W[d][:, hp * 128 : (hp + 1) * 128],
                        rhs=xT[d][:, c * QC : (c + 1) * QC],
                        start=(d == 0),
                        stop=(d == NDT - 1),
                    )
                o = dst[hp][:, c * QC : (c + 1) * QC]
                if bkey in bias_tiles:
                    nc.vector.tensor_scalar_add(
                        o, acc, bias_tiles[bkey][:, hp : hp + 1]
                    )
                else:
                    nc.vector.tensor_copy(o, acc)

            def qk_chunks(hp):
                return [
                    (lambda which=which, c=c: qk_chunk(hp, which, c, ps_mm, "proj"))
                    for which in ("q", "k")
                    for c in range(NQC)
                ]

            NVC = 2
            VC = DM // NVC  # 384

            def v_chunk(s, c, on_act=True):
                acc = ps_st.tile(
                    [128, VC], F32, tag="st", name="vacc",
                    padded_shape=[128, 2 * QC],
                )
                for d in range(NDT):
                    nc.tensor.matmul(
                        acc,
                        lhsT=xT[d][:, s * 128 : (s + 1) * 128],
                        rhs=WV[d][:, c * VC : (c + 1) * VC],
                        start=(d == 0),
                        stop=(d == NDT - 1),
                    )
                nh2 = VC // DH  # heads per chunk (6)
                o = V[s].rearrange("p (h e) -> p h e", e=VH)[
                    :, c * nh2 : (c + 1) * nh2, 0:DH
                ]
                if "bv" in bias_tiles:
                    nc.vector.tensor_add(
                        o,
                        acc.rearrange("p (h e) -> p h e", e=DH),
                        bias_tiles["bv"][:, c * VC : (c + 1) * VC].rearrange(
                            "p (h e) -> p h e", e=DH
                        ),
                    )
                elif on_act:
                    nc.scalar.activation(
                        o, acc.rearrange("p (h e) -> p h e", e=DH), AF.Copy
                    )
                else:
                    nc.vector.tensor_copy(
                        o, acc.rearrange("p (h e) -> p h e", e=DH)
                    )

            def v_chunks(s):
                return [lambda c=c: v_chunk(s, c) for c in range(NVC)]

            WO = []

            def wo_load():
                # sync ring: idle mid-attention (scalar ring would block ACT)
                for d in range(NDT):
                    t = persist.tile([128, DM], BF16, tag=f"WO{d}", name=f"WO{d}")
                    nc.sync.dma_start(out=t, in_=wo[d * 128 : (d + 1) * 128, :])
                    WO.append(t)

            def o_chunks(s):
                ot = [None]

                def chunk(c):
                    if c == 0:
                        ot[0] = out_pool.tile([128, DM], F32, tag="ostage", name="ostage")
                    pool, tag = ((ps_mm, "proj"), (ps_z, "z"))[c % 2]
                    acc = pool.tile(
                        [128, VC], F32, tag=tag, name="oacc",
                        padded_shape=[128, QC],
                    )
                    for d in range(NDT):
                        nc.tensor.matmul(
                            acc,
                            lhsT=ZT[d][:, s * 128 : (s + 1) * 128],
                            rhs=WO[d][:, c * VC : (c + 1) * VC],
                            start=(d == 0),
                            stop=(d == NDT - 1),
                        )
                    o = ot[0][:, c * VC : (c + 1) * VC]
                    if "bo" in bias_tiles:
                        nc.vector.tensor_add(
                            o, acc, bias_tiles["bo"][:, c * VC : (c + 1) * VC]
                        )
                    else:
                        # DVE: the ACT stream is exp-saturated mid-phase and
                        # an in-order ACT copy would delay psum recycling
                        nc.vector.tensor_copy(o, acc)
                    if c == NVC - 1:
                        # rows 512+ drain at the very end: use the scalar
                        # ring (idle by then) so the tail DMAs overlap
                        eng = nc.sync if s < 4 else nc.scalar
                        eng.dma_start(
                            out=out[s * 128 : (s + 1) * 128, :], in_=ot[0]
                        )

                return [lambda c=c: chunk(c) for c in range(NVC)]

            # ---- pipelined attention sweep ----
            zps_of = {}

            def issue_scores(hp, c, g, gsz):
                doffs = [max(0, (g + j) * 128 - c * QC) for j in range(gsz)]
                sts = {}
                for px in (0, 64):
                    sts[px] = ps_st.tile([128, gsz * QC], F32, tag="st", name="st")
                for j in range(gsz):
                    kb = g + j
                    off = doffs[j]
                    for px in (0, 64):
                        nc.tensor.matmul(
                            sts[px][:, j * QC + off : (j + 1) * QC],
                            lhsT=KT[hp][px : px + 64, kb * 128 : (kb + 1) * 128],
                            rhs=QT[hp][px : px + 64, c * QC + off : (c + 1) * QC],
                            start=True,
                            stop=True,
                        )
                return sts, doffs

            def issue_expv(hp, c, g, gsz, sts, doffs, last):
                nkb = 4 * (c + 1)
                if g == 0:
                    zps_of[(hp, c)] = {
                        px: ps_z.tile([128, QC], F32, tag="z", name="z")
                        for px in (0, 64)
                    }
                zps = zps_of[(hp, c)]
                pts = {}
                for px in (0, 64):
                    pt = pt_pool.tile([128, 2 * QC], BF16, tag="pt", name="pt")
                    nc.scalar.activation(
                        pt[:, doffs[0] : gsz * QC],
                        sts[px][:, doffs[0] : gsz * QC],
                        AF.Exp,
                        scale=0.125,
                    )
                    pts[px] = pt
                # diagonal groups (both blocks straddle the diagonal): one
                # paired mask multiply per px covers both 128-wide triangles
                # via a stride-640 view (blocks sit at doff0 + 640*j)
                if g * 128 - c * QC >= 0:
                    base = doffs[0]
                    msk = wm_t[:, :].rearrange("p (a b) -> p a b", b=128)
                    for px in (0, 64):
                        blk = pts[px][:, base : base + 768].rearrange(
                            "p (a b) -> p a b", b=128
                        )[:, 0:6:5, :]
                        nc.vector.tensor_mul(blk, blk, msk)
                for j in range(gsz):
                    kb = g + j
                    off = doffs[j]
                    for px in (0, 64):
                        pt = pts[px]
                        h = 2 * hp + (1 if px else 0)
                        nc.tensor.matmul(
                            zps[px][0 : DH + 1, off:QC],
                            lhsT=V[kb][:, h * VH : h * VH + DH + 1],
                            rhs=pt[:, j * QC + off : (j + 1) * QC],
                            start=(kb == 0),
                            stop=(kb == nkb - 1),
                        )
                if last:
                    for px in (0, 64):
                        dstage = small.tile([128, QC], F32, tag="dstage", name="dstage")
                        nc.vector.tensor_copy(dstage[0:1, :], zps[px][DH : DH + 1, :])
                        recip = small.tile([128, QC], F32, tag="recip", name="recip")
                        nc.vector.reciprocal_approx_fast(recip[0:1, :], dstage[0:1, :])
                        bcast = small.tile([64, QC], F32, tag="bcast", name="bcast")
                        nc.gpsimd.partition_broadcast(bcast, recip[0:1, :])
                        nc.vector.tensor_mul(
                            ZT[hp][px : px + 64, c * QC : (c + 1) * QC],
                            zps[px][0:64, :],
                            bcast,
                        )
                    del zps_of[(hp, c)]

            # ---- pre-phase: project heads 0-1 for query half 0 only (the
            # qc=1 halves are computed as fillers much later), then issue
            # the non-critical DMAs, then V tiles 0-1 ----
            qk_chunk(0, "q", 0, ps_mm, "proj")
            qk_chunk(0, "k", 0, ps_st, "st")
            for d in range(NDT):
                nc.sync.dma_start(
                    out=xT[d][:, QC:SEQ], in_=xt[d * 128 : (d + 1) * 128, QC:SEQ]
                )


            # qc=0 and qc=1 units interleaved: spreads the exp-heavy qc=1
            # units (ACT-bound) across the whole span instead of
            # back-loading them.  Every unit gets filler chunks so exp
            # latency is always hidden behind interposed PE work.
            # the first four units need only the startup-critical DMA set
            # (xt + head-pair-0/1 weight columns); later head pairs' weight
            # columns stream in per-pair just ahead of first use
            units = [
                (0, 0), (1, 0), (0, 1), (1, 1), (2, 0), (3, 0),
                (2, 1), (4, 0), (5, 0), (3, 1), (4, 1), (5, 1),
            ]

            def vc(s, c):
                return lambda: v_chunk(s, c)

            def qkc(hp, which, c):
                return lambda: qk_chunk(hp, which, c, ps_mm, "proj")

            fillers = {
                # v0/v1 here (not pre-phase): their WV-gated matmuls must
                # not sit in the PE stream ahead of the first scores.
                # DVE copies so the first exps aren't queued behind them.
                0: [
                    lambda: v_chunk(0, 0, on_act=False),
                    lambda: v_chunk(0, 1, on_act=False),
                    lambda: v_chunk(1, 0, on_act=False),
                    lambda: v_chunk(1, 1, on_act=False),
                    qkc(1, "q", 0), qkc(1, "k", 0), vc(2, 0), vc(3, 0),
                ],
                1: [qkc(0, "q", 1), qkc(0, "k", 1), vc(4, 0), vc(5, 0)],
                2: [qkc(1, "q", 1), qkc(1, "k", 1), vc(6, 0), vc(7, 0)],
                3: [qkc(2, "q", 0), qkc(2, "k", 0), vc(2, 1), vc(3, 1)],
                4: [qkc(3, "q", 0), qkc(3, "k", 0), vc(4, 1)],
                5: [qkc(2, "q", 1), qkc(2, "k", 1), vc(5, 1)],
                6: [qkc(4, "q", 0), qkc(4, "k", 0), vc(6, 1)],
                7: [qkc(5, "q", 0), qkc(5, "k", 0), vc(7, 1)],
                8: [qkc(3, "q", 1), qkc(3, "k", 1), wo_load],
                9: o_chunks(0) + [qkc(4, "q", 1), qkc(4, "k", 1)],
                10: o_chunks(1) + o_chunks(2) + [qkc(5, "q", 1), qkc(5, "k", 1)],
                11: o_chunks(3),
            }
            # units whose fillers read ZT written by the pending finalizer:
            # flush before popping fillers there (issue-order correctness)
            flush_first = {9, 10, 11}

            pending = [None]

            def flush():
                if pending[0] is not None:
                    fn = pending[0]
                    pending[0] = None
                    fn()

            for ui, (hp, c) in enumerate(units):
                nkb = 4 * (c + 1)
                glist = [(g, min(2, nkb - g)) for g in range(0, nkb, 2)]
                chunks = deque(fillers.get(ui, []))
                n = len(glist)
                for gi, (g, gsz) in enumerate(glist):
                    sts, doffs = issue_scores(hp, c, g, gsz)
                    if ui in flush_first:
                        flush()
                    k = -(-len(chunks) // (n - gi)) if chunks else 0
                    for i in range(k):
                        chunks.popleft()()
                        if i == 0:
                            flush()
                    if k == 0:
                        flush()
                    pending[0] = (
                        lambda hp=hp, c=c, g=g, gsz=gsz, sts=sts, doffs=doffs,
                        last=(gi == n - 1): issue_expv(hp, c, g, gsz, sts, doffs, last)
                    )
            flush()

            # ---- tail: output rows 512-1024.  Full-width accumulation on
            # the freed scores psum, one ACT copy (ACT is idle by now), out
            # DMAs alternating between the two rings.  s=4/5 accumulate
            # d=0..4 first so the PE is busy while the last unit's
            # normalization chain (recip/broadcast/ZT-mul for ZT[5]) runs.
            def o_full_mm(s, acc, drange, start):
                for lo, hi in ((0, QC), (QC, DM)):  # <=512 f32 cols per MM
                    for d in drange:
                        nc.tensor.matmul(
                            acc[:, lo:hi],
                            lhsT=ZT[d][:, s * 128 : (s + 1) * 128],
                            rhs=WO[d][:, lo:hi],
                            start=(start and d == drange[0]),
                            stop=(d == NDT - 1),
                        )

            def o_full_out(s, acc):
                ot = out_pool.tile([128, DM], F32, tag="ostage", name="ostage")
                if "bo" in bias_tiles:
                    nc.vector.tensor_add(ot, acc, bias_tiles["bo"])
                else:
                    nc.scalar.activation(ot, acc, AF.Copy)
                eng = nc.scalar if s % 2 else nc.sync
                eng.dma_start(out=out[s * 128 : (s + 1) * 128, :], in_=ot)

            accs = {}
            for s in (4, 5):
                accs[s] = ps_st.tile(
                    [128, DM], F32, tag="st", name="oacc",
                    padded_shape=[128, 2 * QC],
                )
                o_full_mm(s, accs[s], list(range(NDT - 1)), start=True)
            # s=6 partials on the proj/z psum rings: more PE work ahead of
            # the ZT[5] dependency
            acc6 = {}
            for cc, (pool, tag) in enumerate(((ps_mm, "proj"), (ps_z, "z"))):
                a = pool.tile(
                    [128, VC], F32, tag=tag, name="oacc", padded_shape=[128, QC]
                )
                for d in range(NDT - 1):
                    nc.tensor.matmul(
                        a,
                        lhsT=ZT[d][:, 6 * 128 : 7 * 128],
                        rhs=WO[d][:, cc * VC : (cc + 1) * VC],
                        start=(d == 0),
                        stop=False,
                    )
                acc6[cc] = a
            for s in (4, 5):
                o_full_mm(s, accs[s], [NDT - 1], start=False)
                o_full_out(s, accs[s])
            ot6 = out_pool.tile([128, DM], F32, tag="ostage", name="ostage")
            for cc in (0, 1):
                nc.tensor.matmul(
                    acc6[cc],
                    lhsT=ZT[NDT - 1][:, 6 * 128 : 7 * 128],
                    rhs=WO[NDT - 1][:, cc * VC : (cc + 1) * VC],
                    start=False,
                    stop=True,
                )
                o = ot6[:, cc * VC : (cc + 1) * VC]
                if "bo" in bias_tiles:
                    nc.vector.tensor_add(
                        o, acc6[cc], bias_tiles["bo"][:, cc * VC : (cc + 1) * VC]
                    )
                else:
                    nc.scalar.activation(o, acc6[cc], AF.Copy)
            nc.sync.dma_start(out=out[6 * 128 : 7 * 128, :], in_=ot6)
            acc = ps_st.tile(
                [128, DM], F32, tag="st", name="oacc",
                padded_shape=[128, 2 * QC],
            )
            o_full_mm(7, acc, list(range(NDT)), start=True)
            o_full_out(7, acc)

            if debug_taps:
                for nm, tiles in (("QT_d", QT), ("KT_d", KT), ("ZT_d", ZT)):
                    for d in range(NDT):
                        nc.sync.dma_start(
                            out=taps[nm][d * 128 : (d + 1) * 128, :],
                            in_=tiles[d][:, :],
                        )
                for s in range(NQT):
                    nc.sync.dma_start(
                        out=taps["V_d"][s * 128 : (s + 1) * 128, :], in_=V[s][:, :]
                    )

    nc.compile()
    return nc


_CACHE = {}


def _get_nc(key):
    if key not in _CACHE:
        _CACHE[key] = build(*key)
    return _CACHE[key]


def _prep(inputs):
    BF = ml_dtypes.bfloat16
    x = np.asarray(inputs["normalized_resid_pre"], np.float32)
    wq = np.ascontiguousarray(
        np.asarray(inputs["W_Q"], np.float32).transpose(1, 0, 2).reshape(DM, DM)
    ).astype(BF)
    wk = np.ascontiguousarray(
        np.asarray(inputs["W_K"], np.float32).transpose(1, 0, 2).reshape(DM, DM)
    ).astype(BF)
    wv = np.ascontiguousarray(
        np.asarray(inputs["W_V"], np.float32).transpose(1, 0, 2).reshape(DM, DM)
    ).astype(BF)
    wo = np.ascontiguousarray(
        np.asarray(inputs["W_O"], np.float32).reshape(DM, DM)
    ).astype(BF)
    bq = np.asarray(inputs["b_Q"], np.float32).reshape(NDT, 128).T
    bk = np.asarray(inputs["b_K"], np.float32).reshape(NDT, 128).T
    bv = np.asarray(inputs["b_V"], np.float32).reshape(1, DM)
    bo = np.asarray(inputs["b_O"], np.float32).reshape(1, DM)
    jj, uu = np.meshgrid(np.arange(128), np.arange(128), indexing="ij")
    wmask = np.tile((uu >= jj).astype(BF), (1, 2))
    key = (
        bool(np.any(bq)),
        bool(np.any(bk)),
        bool(np.any(bv)),
        bool(np.any(bo)),
    )
    common = {
        "wq": wq, "wk": wk, "wv": wv, "wo": wo, "wmask": wmask,
        "identin": np.eye(128, dtype=np.float32).astype(BF),
    }
    if key[0]:
        common["bq"] = np.ascontiguousarray(bq)
    if key[1]:
        common["bk"] = np.ascontiguousarray(bk)
    if key[2]:
        common["bv"] = np.ascontiguousarray(bv)
    if key[3]:
        common["bo"] = np.ascontiguousarray(bo)
    in_maps = [
        dict(common, xt=np.ascontiguousarray(x[b].T).astype(BF))
        for b in range(BATCH)
    ]
    return key, in_maps


def run(inputs, trace=False, **kw):
    key, in_maps = _prep(inputs)
    nc = _get_nc(key)
    res = run_bass_kernel_spmd(
        nc, in_maps, core_ids=list(range(BATCH)), trace=trace, **kw
    )
    outs = np.stack([res.results[b]["out"] for b in range(BATCH)])
    return outs.astype(np.float32), res


def kernel(**inputs):
    out, _ = run(inputs)
    return out


if __name__ == "__main__":
    rng = np.random.default_rng(0)
    ins = {
        "normalized_resid_pre": rng.standard_normal((8, SEQ, DM)).astype(np.float32),
        "W_Q": (0.02 * rng.standard_normal((NH, DM, DH))).astype(np.float32),
        "b_Q": np.zeros((NH, DH), np.float32),
        "W_K": (0.02 * rng.standard_normal((NH, DM, DH))).astype(np.float32),
        "b_K": np.zeros((NH, DH), np.float32),
        "W_V": (0.02 * rng.standard_normal((NH, DM, DH))).astype(np.float32),
        "b_V": np.zeros((NH, DH), np.float32),
        "W_O": (0.02 * rng.standard_normal((NH, DH, DM))).astype(np.float32),
        "b_O": np.zeros((DM,), np.float32),
    }
    out = kernel(**ins)
    print("kernel output", out.shape, out.dtype, float(np.abs(out).max()))


# revision 40
# speedup vs baseline: 1.1980x; 1.0016x over previous
"""Causal multi-head attention on 8 Trainium2 NeuronCores.

Problem: nn_Attention_46643344835180
  x: [8, 1024, 768], 12 heads x 64 dh, causal softmax attention + output proj.

Sharding: data-parallel over batch (8 batch elements -> 8 cores, no collectives).

v3: full bf16 compute (PSUM stays f32), host-side transpose of x (xT fed
directly), weights resident in SBUF, dual DMA rings (sync + scalar HWDGE),
QK/V psum->sbuf copies on the Scalar engine (idle during the projection
phase), and a globally software-pipelined attention sweep: scores of group
g+1 are issued before exp/PV of group g, with projection/output chunks
spread between groups as PE filler.

Per-core dataflow (batch element b):
  xT = x_b.T (host)                                                  [768, 1024]
  QT = Wq.T @ xT  (+bq)            heads stacked on partitions       [768, 1024]
  KT = Wk.T @ xT  (+bk)                                              [768, 1024]
  V  = x_b @ Wv   (+bv)            + interleaved ones column         [1024, 12*66]
  per head h, query-chunk qc (512):
    S^T[k,q] = KT_h.T @ QT_h          keys on partitions
    P^T = exp(S^T / 8)                ScalarE, batched over 2 key-blocks
    causal: one 128-wide-mask multiply per diagonal block
    z^T[65,512] += [V_h | 1].T @ P^T  row 64 accumulates the denominator
    ZT_h = z^T[0:64] * approx(1/z^T[64])   (denom -> reciprocal ->
           gpsimd partition_broadcast -> multiply)
  out = ZT.T @ Wo (+bo)                                              [1024, 768]
"""

import sys

sys.path.insert(0, "/opt/trn_rl_repo")

from collections import deque

import ml_dtypes
import numpy as np

import concourse.bass as bass
import concourse.mybir as mybir
import concourse.tile as tile
from concourse import bacc
from concourse.bass_utils import run_bass_kernel_spmd

F32 = mybir.dt.float32
BF16 = mybir.dt.bfloat16
AF = mybir.ActivationFunctionType

SEQ = 1024
DM = 768
NH = 12
DH = 64
VH = DH + 2  # V head stride: 64 dims + ones col + pad (keeps 4B alignment)
BATCH = 8
NQT = SEQ // 128  # 8 seq tiles of 128
NDT = DM // 128  # 6 d_model tiles
QC = 512  # query chunk (moving dim)
NQC = SEQ // QC  # 2
WARMUP = 64  # HAM warmup matmuls (bf16 N=128, ~110ns each cold)


def build(with_bq, with_bk, with_bv, with_bo, debug_taps=False):
    nc = bacc.Bacc("TRN2", target_bir_lowering=False, debug=False)

    xt = nc.dram_tensor("xt", [DM, SEQ], BF16, kind="ExternalInput")
    wq = nc.dram_tensor("wq", [DM, DM], BF16, kind="ExternalInput")
    wk = nc.dram_tensor("wk", [DM, DM], BF16, kind="ExternalInput")
    wv = nc.dram_tensor("wv", [DM, DM], BF16, kind="ExternalInput")
    wo = nc.dram_tensor("wo", [DM, DM], BF16, kind="ExternalInput")
    wmask = nc.dram_tensor("wmask", [128, 256], BF16, kind="ExternalInput")
    identin = nc.dram_tensor("identin", [128, 128], BF16, kind="ExternalInput")
    bq = bk = bv = bo = None
    if with_bq:
        bq = nc.dram_tensor("bq", [128, NDT], F32, kind="ExternalInput")
    if with_bk:
        bk = nc.dram_tensor("bk", [128, NDT], F32, kind="ExternalInput")
    if with_bv:
        bv = nc.dram_tensor("bv", [1, DM], F32, kind="ExternalInput")
    if with_bo:
        bo = nc.dram_tensor("bo", [1, DM], F32, kind="ExternalInput")
    out = nc.dram_tensor("out", [SEQ, DM], F32, kind="ExternalOutput")
    taps = {}
    if debug_taps:
        for nm in ("QT_d", "KT_d", "ZT_d"):
            taps[nm] = nc.dram_tensor(nm, [DM, SEQ], BF16, kind="ExternalOutput")
        taps["V_d"] = nc.dram_tensor(
            "V_d", [SEQ, NH * VH], BF16, kind="ExternalOutput"
        )

    with tile.TileContext(nc) as tc:
        with (
            tc.tile_pool(name="persist", bufs=1) as persist,
            tc.tile_pool(name="pt", bufs=4) as pt_pool,
            tc.tile_pool(name="small", bufs=2) as small,
            tc.tile_pool(name="outst", bufs=2) as out_pool,
            tc.tile_pool(name="ps_st", bufs=2, space="PSUM") as ps_st,
            tc.tile_pool(name="ps_z", bufs=3, space="PSUM") as ps_z,
            tc.tile_pool(name="ps_mm", bufs=1, space="PSUM") as ps_mm,
        ):
            # ---- sync ring: ident, xT, WV, mask/ones.  scalar ring: wq/wk
            # (column-split so head pairs 0-1 unblock early), wo later ----
            ident = persist.tile([128, 128], BF16, tag="ident", name="ident")
            nc.sync.dma_start(out=ident, in_=identin[:, :])
            warm_ps = ps_mm.tile(
                [128, 128], F32, tag="proj", name="warm", padded_shape=[128, QC]
            )
            for _ in range(WARMUP):
                nc.tensor.matmul(warm_ps, lhsT=ident, rhs=ident, start=True, stop=True)

            # single sync ring for all inputs in priority order (the two
            # HWDGE rings share ~210GB/s of HBM read bandwidth, and DMAs on
            # the scalar ring block the ACT compute stream behind them).
            # Startup-critical set first: xt query-half 0, wq/wk cols for
            # head pairs 0-1, mask, WV.  xt half 1 and the remaining weight
            # columns are issued after the first projection chunks (below).
            xT = [
                persist.tile([128, SEQ], BF16, tag=f"xT{d}", name=f"xT{d}")
                for d in range(NDT)
            ]
            for d in range(NDT):
                nc.sync.dma_start(
                    out=xT[d][:, 0:QC], in_=xt[d * 128 : (d + 1) * 128, 0:QC]
                )

            WQ = [
                persist.tile([128, DM], BF16, tag=f"WQ{d}", name=f"WQ{d}")
                for d in range(NDT)
            ]
            WK = [
                persist.tile([128, DM], BF16, tag=f"WK{d}", name=f"WK{d}")
                for d in range(NDT)
            ]
            WV = [
                persist.tile([128, DM], BF16, tag=f"WV{d}", name=f"WV{d}")
                for d in range(NDT)
            ]
            # full-row weight DMAs: 1536B partition lines run ~3x the
            # bandwidth of 512B column-piece lines, and fewer DMAs clear
            # the ring sooner for WV / xt half 1
            for src, dst in ((wq, WQ), (wk, WK)):
                for d in range(NDT):
                    nc.sync.dma_start(out=dst[d], in_=src[d * 128 : (d + 1) * 128, :])

            wm_t = persist.tile([128, 256], BF16, tag="wmask", name="wmask")
            nc.sync.dma_start(out=wm_t, in_=wmask[:, :])

            for d in range(NDT):
                nc.sync.dma_start(out=WV[d], in_=wv[d * 128 : (d + 1) * 128, :])

            bias_tiles = {}
            if with_bq:
                t = persist.tile([128, NDT], F32, tag="bq", name="bq")
                nc.scalar.dma_start(out=t, in_=bq[:, :])
                bias_tiles["bq"] = t
            if with_bk:
                t = persist.tile([128, NDT], F32, tag="bk", name="bk")
                nc.scalar.dma_start(out=t, in_=bk[:, :])
                bias_tiles["bk"] = t
            if with_bv:
                t = persist.tile([128, DM], F32, tag="bv", name="bv")
                nc.scalar.dma_start(out=t, in_=bv[0:1, :].to_broadcast((128, DM)))
                bias_tiles["bv"] = t
            if with_bo:
                t = persist.tile([128, DM], F32, tag="bo", name="bo")
                nc.scalar.dma_start(out=t, in_=bo[0:1, :].to_broadcast((128, DM)))
                bias_tiles["bo"] = t

            QT = [
                persist.tile([128, SEQ], BF16, tag=f"QT{d}", name=f"QT{d}")
                for d in range(NDT)
            ]
            KT = [
                persist.tile([128, SEQ], BF16, tag=f"KT{d}", name=f"KT{d}")
                for d in range(NDT)
            ]
            V = [
                persist.tile([128, NH * VH], BF16, tag=f"V{s}", name=f"V{s}")
                for s in range(NQT)
            ]
            for s in range(NQT):
                vv = V[s].rearrange("p (h e) -> p h e", e=VH)
                nc.vector.memset(vv[:, :, DH : DH + 1], 1.0)
            ZT = [
                persist.tile([128, SEQ], BF16, tag=f"ZT{d}", name=f"ZT{d}")
                for d in range(NDT)
            ]

            # ---- projection chunks (each = one psum round trip) ----
            def qk_chunk(hp, which, c, pool, tag):
                W, dst, bkey = (
                    (WQ, QT, "bq") if which == "q" else (WK, KT, "bk")
                )
                acc = pool.tile(
                    [128, QC], F32, tag=tag, name="proj",
                    padded_shape=[128, 2 * QC] if tag == "st" else [128, QC],
                )
                for d in range(NDT):
                    nc.tensor.matmul(
                        acc,
                        lhsT=W[d][:, hp * 128 : (hp + 1) * 128],
                        rhs=xT[d][:, c * QC : (c + 1) * QC],
                        start=(d == 0),
                        stop=(d == NDT - 1),
                    )
                o = dst[hp][:, c * QC : (c + 1) * QC]
                if bkey in bias_tiles:
                    nc.vector.tensor_scalar_add(
                        o, acc, bias_tiles[bkey][:, hp : hp + 1]
                    )
                else:
                    nc.vector.tensor_copy(o, acc)

            def qk_chunks(hp):
                return [
                    (lambda which=which, c=c: qk_chunk(hp, which, c, ps_mm, "proj"))
                    for which in ("q", "k")
                    for c in range(NQC)
                ]

            NVC = 2
            VC = DM // NVC  # 384

            def v_chunk(s, c, on_act=True):
                acc = ps_st.tile(
                    [128, VC], F32, tag="st", name="vacc",
                    padded_shape=[128, 2 * QC],
                )
                for d in range(NDT):
                    nc.tensor.matmul(
                        acc,
                        lhsT=xT[d][:, s * 128 : (s + 1) * 128],
                        rhs=WV[d][:, c * VC : (c + 1) * VC],
                        start=(d == 0),
                        stop=(d == NDT - 1),
                    )
                nh2 = VC // DH  # heads per chunk (6)
                o = V[s].rearrange("p (h e) -> p h e", e=VH)[
                    :, c * nh2 : (c + 1) * nh2, 0:DH
                ]
                if "bv" in bias_tiles:
                    nc.vector.tensor_add(
                        o,
                        acc.rearrange("p (h e) -> p h e", e=DH),
                        bias_tiles["bv"][:, c * VC : (c + 1) * VC].rearrange(
                            "p (h e) -> p h e", e=DH
                        ),
                    )
                elif on_act:
                    nc.scalar.activation(
                        o, acc.rearrange("p (h e) -> p h e", e=DH), AF.Copy
                    )
                else:
                    nc.vector.tensor_copy(
                        o, acc.rearrange("p (h e) -> p h e", e=DH)
                    )

            def v_chunks(s):
                return [lambda c=c: v_chunk(s, c) for c in range(NVC)]

            WO = []

            def wo_load():
                # sync ring: idle mid-attention (scalar ring would block ACT)
                for d in range(NDT):
                    t = persist.tile([128, DM], BF16, tag=f"WO{d}", name=f"WO{d}")
                    nc.sync.dma_start(out=t, in_=wo[d * 128 : (d + 1) * 128, :])
                    WO.append(t)

            def o_chunks(s):
                ot = [None]

                def chunk(c):
                    if c == 0:
                        ot[0] = out_pool.tile([128, DM], F32, tag="ostage", name="ostage")
                    pool, tag = ((ps_mm, "proj"), (ps_z, "z"))[c % 2]
                    acc = pool.tile(
                        [128, VC], F32, tag=tag, name="oacc",
                        padded_shape=[128, QC],
                    )
                    for d in range(NDT):
                        nc.tensor.matmul(
                            acc,
                            lhsT=ZT[d][:, s * 128 : (s + 1) * 128],
                            rhs=WO[d][:, c * VC : (c + 1) * VC],
                            start=(d == 0),
                            stop=(d == NDT - 1),
                        )
                    o = ot[0][:, c * VC : (c + 1) * VC]
                    if "bo" in bias_tiles:
                        nc.vector.tensor_add(
                            o, acc, bias_tiles["bo"][:, c * VC : (c + 1) * VC]
                        )
                    else:
                        # DVE: the ACT stream is exp-saturated mid-phase and
                        # an in-order ACT copy would delay psum recycling
                        nc.vector.tensor_copy(o, acc)
                    if c == NVC - 1:
                        # rows 512+ drain at the very end: use the scalar
                        # ring (idle by then) so the tail DMAs overlap
                        eng = nc.sync if s < 4 else nc.scalar
                        eng.dma_start(
                            out=out[s * 128 : (s + 1) * 128, :], in_=ot[0]
                        )

                return [lambda c=c: chunk(c) for c in range(NVC)]

            # ---- pipelined attention sweep ----
            zps_of = {}

            def issue_scores(hp, c, g, gsz):
                doffs = [max(0, (g + j) * 128 - c * QC) for j in range(gsz)]
                sts = {}
                for px in (0, 64):
                    sts[px] = ps_st.tile([128, gsz * QC], F32, tag="st", name="st")
                for j in range(gsz):
                    kb = g + j
                    off = doffs[j]
                    for px in (0, 64):
                        nc.tensor.matmul(
                            sts[px][:, j * QC + off : (j + 1) * QC],
                            lhsT=KT[hp][px : px + 64, kb * 128 : (kb + 1) * 128],
                            rhs=QT[hp][px : px + 64, c * QC + off : (c + 1) * QC],
                            start=True,
                            stop=True,
                        )
                return sts, doffs

            def issue_expv(hp, c, g, gsz, sts, doffs, last):
                nkb = 4 * (c + 1)
                if g == 0:
                    zps_of[(hp, c)] = {
                        px: ps_z.tile([128, QC], F32, tag="z", name="z")
                        for px in (0, 64)
                    }
                zps = zps_of[(hp, c)]
                pts = {}
                for px in (0, 64):
                    pt = pt_pool.tile([128, 2 * QC], BF16, tag="pt", name="pt")
                    nc.scalar.activation(
                        pt[:, doffs[0] : gsz * QC],
                        sts[px][:, doffs[0] : gsz * QC],
                        AF.Exp,
                        scale=0.125,
                    )
                    pts[px] = pt
                # diagonal groups (both blocks straddle the diagonal): one
                # paired mask multiply per px covers both 128-wide triangles
                # via a stride-640 view (blocks sit at doff0 + 640*j)
                if g * 128 - c * QC >= 0:
                    base = doffs[0]
                    msk = wm_t[:, :].rearrange("p (a b) -> p a b", b=128)
                    for px in (0, 64):
                        blk = pts[px][:, base : base + 768].rearrange(
                            "p (a b) -> p a b", b=128
                        )[:, 0:6:5, :]
                        nc.vector.tensor_mul(blk, blk, msk)
                for j in range(gsz):
                    kb = g + j
                    off = doffs[j]
                    for px in (0, 64):
                        pt = pts[px]
                        h = 2 * hp + (1 if px else 0)
                        nc.tensor.matmul(
                            zps[px][0 : DH + 1, off:QC],
                            lhsT=V[kb][:, h * VH : h * VH + DH + 1],
                            rhs=pt[:, j * QC + off : (j + 1) * QC],
                            start=(kb == 0),
                            stop=(kb == nkb - 1),
                        )
                if last:
                    for px in (0, 64):
                        dstage = small.tile([128, QC], F32, tag="dstage", name="dstage")
                        nc.vector.tensor_copy(dstage[0:1, :], zps[px][DH : DH + 1, :])
                        recip = small.tile([128, QC], F32, tag="recip", name="recip")
                        nc.vector.reciprocal_approx_fast(recip[0:1, :], dstage[0:1, :])
                        bcast = small.tile([64, QC], F32, tag="bcast", name="bcast")
                        nc.gpsimd.partition_broadcast(bcast, recip[0:1, :])
                        nc.vector.tensor_mul(
                            ZT[hp][px : px + 64, c * QC : (c + 1) * QC],
                            zps[px][0:64, :],
                            bcast,
                        )
                    del zps_of[(hp, c)]

            # ---- pre-phase: project heads 0-1 for query half 0 only (the
            # qc=1 halves are computed as fillers much later), then issue
            # the non-critical DMAs, then V tiles 0-1 ----
            qk_chunk(0, "q", 0, ps_mm, "proj")
            qk_chunk(0, "k", 0, ps_st, "st")
            for d in range(NDT):
                nc.sync.dma_start(
                    out=xT[d][:, QC:SEQ], in_=xt[d * 128 : (d + 1) * 128, QC:SEQ]
                )


            # qc=0 and qc=1 units interleaved: spreads the exp-heavy qc=1
            # units (ACT-bound) across the whole span instead of
            # back-loading them.  Every unit gets filler chunks so exp
            # latency is always hidden behind interposed PE work.
            # the first four units need only the startup-critical DMA set
            # (xt + head-pair-0/1 weight columns); later head pairs' weight
            # columns stream in per-pair just ahead of first use
            units = [
                (0, 0), (1, 0), (0, 1), (1, 1), (2, 0), (3, 0),
                (2, 1), (4, 0), (5, 0), (3, 1), (4, 1), (5, 1),
            ]

            def vc(s, c):
                return lambda: v_chunk(s, c)

            def qkc(hp, which, c):
                return lambda: qk_chunk(hp, which, c, ps_mm, "proj")

            fillers = {
                # v0/v1 here (not pre-phase): their WV-gated matmuls must
                # not sit in the PE stream ahead of the first scores.
                # DVE copies so the first exps aren't queued behind them.
                0: [
                    lambda: v_chunk(0, 0, on_act=False),
                    lambda: v_chunk(0, 1, on_act=False),
                    lambda: v_chunk(1, 0, on_act=False),
                    lambda: v_chunk(1, 1, on_act=False),
                    qkc(1, "q", 0), qkc(1, "k", 0), vc(2, 0), vc(3, 0),
                ],
                1: [qkc(0, "q", 1), qkc(0, "k", 1), vc(4, 0), vc(5, 0)],
                2: [qkc(1, "q", 1), qkc(1, "k", 1), vc(6, 0), vc(7, 0)],
                3: [qkc(2, "q", 0), qkc(2, "k", 0), vc(2, 1), vc(3, 1)],
                4: [qkc(3, "q", 0), qkc(3, "k", 0), vc(4, 1)],
                5: [qkc(2, "q", 1), qkc(2, "k", 1), vc(5, 1)],
                6: [qkc(4, "q", 0), qkc(4, "k", 0), vc(6, 1)],
                7: [qkc(5, "q", 0), qkc(5, "k", 0), vc(7, 1)],
                8: [qkc(3, "q", 1), qkc(3, "k", 1), wo_load],
                9: o_chunks(0) + [qkc(4, "q", 1), qkc(4, "k", 1)],
                10: o_chunks(1) + o_chunks(2) + [qkc(5, "q", 1), qkc(5, "k", 1)],
                11: o_chunks(3),
            }
            # units whose fillers read ZT written by the pending finalizer:
            # flush before popping fillers there (issue-order correctness)
            flush_first = {9, 10, 11}

            pending = [None]

            def flush():
                if pending[0] is not None:
                    fn = pending[0]
                    pending[0] = None
                    fn()

            for ui, (hp, c) in enumerate(units):
                nkb = 4 * (c + 1)
                glist = [(g, min(2, nkb - g)) for g in range(0, nkb, 2)]
                chunks = deque(fillers.get(ui, []))
                n = len(glist)
                for gi, (g, gsz) in enumerate(glist):
                    sts, doffs = issue_scores(hp, c, g, gsz)
                    if ui in flush_first:
                        flush()
                    k = -(-len(chunks) // (n - gi)) if chunks else 0
                    for i in range(k):
                        chunks.popleft()()
                        if i == 0:
                            flush()
                    if k == 0:
                        flush()
                    pending[0] = (
                        lambda hp=hp, c=c, g=g, gsz=gsz, sts=sts, doffs=doffs,
                        last=(gi == n - 1): issue_expv(hp, c, g, gsz, sts, doffs, last)
                    )
            flush()

            # ---- tail: output rows 512-1024.  Full-width accumulation on
            # the freed scores psum, one ACT copy (ACT is idle by now), out
            # DMAs alternating between the two rings.  s=4/5 accumulate
            # d=0..4 first so the PE is busy while the last unit's
            # normalization chain (recip/broadcast/ZT-mul for ZT[5]) runs.
            def o_full_mm(s, acc, drange, start):
                for lo, hi in ((0, QC), (QC, DM)):  # <=512 f32 cols per MM
                    for d in drange:
                        nc.tensor.matmul(
                            acc[:, lo:hi],
                            lhsT=ZT[d][:, s * 128 : (s + 1) * 128],
                            rhs=WO[d][:, lo:hi],
                            start=(start and d == drange[0]),
                            stop=(d == NDT - 1),
                        )

            def o_full_out(s, acc):
                ot = out_pool.tile([128, DM], F32, tag="ostage", name="ostage")
                if "bo" in bias_tiles:
                    nc.vector.tensor_add(ot, acc, bias_tiles["bo"])
                elif s % 2:  # alternate engines: the tail copies would
                    nc.scalar.activation(ot, acc, AF.Copy)  # serialize on ACT
                else:
                    nc.vector.tensor_copy(ot, acc)
                eng = nc.scalar if s % 2 else nc.sync
                eng.dma_start(out=out[s * 128 : (s + 1) * 128, :], in_=ot)

            accs = {}
            for s in (4, 5):
                accs[s] = ps_st.tile(
                    [128, DM], F32, tag="st", name="oacc",
                    padded_shape=[128, 2 * QC],
                )
                o_full_mm(s, accs[s], list(range(NDT - 1)), start=True)
            # s=6 partials on the proj/z psum rings: more PE work ahead of
            # the ZT[5] dependency
            acc6 = {}
            for cc, (pool, tag) in enumerate(((ps_mm, "proj"), (ps_z, "z"))):
                a = pool.tile(
                    [128, VC], F32, tag=tag, name="oacc", padded_shape=[128, QC]
                )
                for d in range(NDT - 1):
                    nc.tensor.matmul(
                        a,
                        lhsT=ZT[d][:, 6 * 128 : 7 * 128],
                        rhs=WO[d][:, cc * VC : (cc + 1) * VC],
                        start=(d == 0),
                        stop=False,
                    )
                acc6[cc] = a
            for s in (4, 5):
                o_full_mm(s, accs[s], [NDT - 1], start=False)
                o_full_out(s, accs[s])
            ot6 = out_pool.tile([128, DM], F32, tag="ostage", name="ostage")
            for cc in (0, 1):
                nc.tensor.matmul(
                    acc6[cc],
                    lhsT=ZT[NDT - 1][:, 6 * 128 : 7 * 128],
                    rhs=WO[NDT - 1][:, cc * VC : (cc + 1) * VC],
                    start=False,
                    stop=True,
                )
                o = ot6[:, cc * VC : (cc + 1) * VC]
                if "bo" in bias_tiles:
                    nc.vector.tensor_add(
                        o, acc6[cc], bias_tiles["bo"][:, cc * VC : (cc + 1) * VC]
                    )
                elif cc:
                    nc.scalar.activation(o, acc6[cc], AF.Copy)
                else:
                    nc.vector.tensor_copy(o, acc6[cc])
            nc.sync.dma_start(out=out[6 * 128 : 7 * 128, :], in_=ot6)
            acc = ps_st.tile(
                [128, DM], F32, tag="st", name="oacc",
                padded_shape=[128, 2 * QC],
            )
            o_full_mm(7, acc, list(range(NDT)), start=True)
            o_full_out(7, acc)

            if debug_taps:
                for nm, tiles in (("QT_d", QT), ("KT_d", KT), ("ZT_d", ZT)):
                    for d in range(NDT):
                        nc.sync.dma_start(
                            out=taps[nm][d * 128 : (d + 1) * 128, :],
                            in_=tiles[d][:, :],
                        )
                for s in range(NQT):
                    nc.sync.dma_start(
                        out=taps["V_d"][s * 128 : (s + 1) * 128, :], in_=V[s][:, :]
                    )

    nc.compile()
    return nc


_CACHE = {}


def _get_nc(key):
    if key not in _CACHE:
        _CACHE[key] = build(*key)
    return _CACHE[key]


def _prep(inputs):
    BF = ml_dtypes.bfloat16
    x = np.asarray(inputs["normalized_resid_pre"], np.float32)
    wq = np.ascontiguousarray(
        np.asarray(inputs["W_Q"], np.float32).transpose(1, 0, 2).reshape(DM, DM)
    ).astype(BF)
    wk = np.ascontiguousarray(
        np.asarray(inputs["W_K"], np.float32).transpose(1, 0, 2).reshape(DM, DM)
    ).astype(BF)
    wv = np.ascontiguousarray(
        np.asarray(inputs["W_V"], np.float32).transpose(1, 0, 2).reshape(DM, DM)
    ).astype(BF)
    wo = np.ascontiguousarray(
        np.asarray(inputs["W_O"], np.float32).reshape(DM, DM)
    ).astype(BF)
    bq = np.asarray(inputs["b_Q"], np.float32).reshape(NDT, 128).T
    bk = np.asarray(inputs["b_K"], np.float32).reshape(NDT, 128).T
    bv = np.asarray(inputs["b_V"], np.float32).reshape(1, DM)
    bo = np.asarray(inputs["b_O"], np.float32).reshape(1, DM)
    jj, uu = np.meshgrid(np.arange(128), np.arange(128), indexing="ij")
    wmask = np.tile((uu >= jj).astype(BF), (1, 2))
    key = (
        bool(np.any(bq)),
        bool(np.any(bk)),
        bool(np.any(bv)),
        bool(np.any(bo)),
    )
    common = {
        "wq": wq, "wk": wk, "wv": wv, "wo": wo, "wmask": wmask,
        "identin": np.eye(128, dtype=np.float32).astype(BF),
    }
    if key[0]:
        common["bq"] = np.ascontiguousarray(bq)
    if key[1]:
        common["bk"] = np.ascontiguousarray(bk)
    if key[2]:
        common["bv"] = np.ascontiguousarray(bv)
    if key[3]:
        common["bo"] = np.ascontiguousarray(bo)
    in_maps = [
        dict(common, xt=np.ascontiguousarray(x[b].T).astype(BF))
        for b in range(BATCH)
    ]
    return key, in_maps


def run(inputs, trace=False, **kw):
    key, in_maps = _prep(inputs)
    nc = _get_nc(key)
    res = run_bass_kernel_spmd(
        nc, in_maps, core_ids=list(range(BATCH)), trace=trace, **kw
    )
    outs = np.stack([res.results[b]["out"] for b in range(BATCH)])
    return outs.astype(np.float32), res


def kernel(**inputs):
    out, _ = run(inputs)
    return out


if __name__ == "__main__":
    rng = np.random.default_rng(0)
    ins = {
        "normalized_resid_pre": rng.standard_normal((8, SEQ, DM)).astype(np.float32),
        "W_Q": (0.02 * rng.standard_normal((NH, DM, DH))).astype(np.float32),
        "b_Q": np.zeros((NH, DH), np.float32),
        "W_K": (0.02 * rng.standard_normal((NH, DM, DH))).astype(np.float32),
        "b_K": np.zeros((NH, DH), np.float32),
        "W_V": (0.02 * rng.standard_normal((NH, DM, DH))).astype(np.float32),
        "b_V": np.zeros((NH, DH), np.float32),
        "W_O": (0.02 * rng.standard_normal((NH, DH, DM))).astype(np.float32),
        "b_O": np.zeros((DM,), np.float32),
    }
    out = kernel(**ins)
    print("kernel output", out.shape, out.dtype, float(np.abs(out).max()))


# revision 58
# speedup vs baseline: 1.2284x; 1.0253x over previous
"""Causal multi-head attention on 8 Trainium2 NeuronCores.

Problem: nn_Attention_46643344835180
  x: [8, 1024, 768], 12 heads x 64 dh, causal softmax attention + output proj.

Sharding: data-parallel over batch (8 batch elements -> 8 cores, no collectives).

v3: full bf16 compute (PSUM stays f32), host-side transpose of x (xT fed
directly), weights resident in SBUF, dual DMA rings (sync + scalar HWDGE),
QK/V psum->sbuf copies on the Scalar engine (idle during the projection
phase), and a globally software-pipelined attention sweep: scores of group
g+1 are issued before exp/PV of group g, with projection/output chunks
spread between groups as PE filler.

Per-core dataflow (batch element b):
  xT = x_b.T (host)                                                  [768, 1024]
  QT = Wq.T @ xT  (+bq)            heads stacked on partitions       [768, 1024]
  KT = Wk.T @ xT  (+bk)                                              [768, 1024]
  V  = x_b @ Wv   (+bv)            + interleaved ones column         [1024, 12*66]
  per head h, query-chunk qc (512):
    S^T[k,q] = KT_h.T @ QT_h          keys on partitions
    P^T = exp(S^T / 8)                ScalarE, batched over 2 key-blocks
    causal: one 128-wide-mask multiply per diagonal block
    z^T[65,512] += [V_h | 1].T @ P^T  row 64 accumulates the denominator
    ZT_h = z^T[0:64] * approx(1/z^T[64])   (denom -> reciprocal ->
           gpsimd partition_broadcast -> multiply)
  out = ZT.T @ Wo (+bo)                                              [1024, 768]
"""

import sys

sys.path.insert(0, "/opt/trn_rl_repo")

from collections import deque

import ml_dtypes
import numpy as np

import concourse.bass as bass
import concourse.mybir as mybir
import concourse.tile as tile
from concourse import bacc
from concourse.bass_utils import run_bass_kernel_spmd

F32 = mybir.dt.float32
BF16 = mybir.dt.bfloat16
AF = mybir.ActivationFunctionType

SEQ = 1024
DM = 768
NH = 12
DH = 64
VH = DH + 2  # V head stride: 64 dims + ones col + pad (keeps 4B alignment)
BATCH = 8
NQT = SEQ // 128  # 8 seq tiles of 128
NDT = DM // 128  # 6 d_model tiles
QC = 512  # query chunk (moving dim)
NQC = SEQ // QC  # 2
PTW = 640  # per-key-block stride inside a pt tile (= QC + diag width)
WARMUP = 88  # HAM warmup matmuls (bf16 N=128, ~110ns each cold): sized to
# bridge from ident-arrival (~9us) to the DMA-gated first projections
# (~19us) so the PE clock doesn't re-throttle before real work starts


def build(with_bq, with_bk, with_bv, with_bo, debug_taps=False):
    nc = bacc.Bacc("TRN2", target_bir_lowering=False, debug=False)

    xt = nc.dram_tensor("xt", [DM, SEQ], BF16, kind="ExternalInput")
    wq = nc.dram_tensor("wq", [DM, DM], BF16, kind="ExternalInput")
    wk = nc.dram_tensor("wk", [DM, DM], BF16, kind="ExternalInput")
    wv = nc.dram_tensor("wv", [DM, DM], BF16, kind="ExternalInput")
    wo = nc.dram_tensor("wo", [DM, DM], BF16, kind="ExternalInput")
    wmask = nc.dram_tensor("wmask", [128, 256], BF16, kind="ExternalInput")
    identin = nc.dram_tensor("identin", [128, 128], BF16, kind="ExternalInput")
    bq = bk = bv = bo = None
    if with_bq:
        bq = nc.dram_tensor("bq", [128, NDT], F32, kind="ExternalInput")
    if with_bk:
        bk = nc.dram_tensor("bk", [128, NDT], F32, kind="ExternalInput")
    if with_bv:
        bv = nc.dram_tensor("bv", [1, DM], F32, kind="ExternalInput")
    if with_bo:
        bo = nc.dram_tensor("bo", [1, DM], F32, kind="ExternalInput")
    out = nc.dram_tensor("out", [SEQ, DM], F32, kind="ExternalOutput")
    taps = {}
    if debug_taps:
        for nm in ("QT_d", "KT_d", "ZT_d"):
            taps[nm] = nc.dram_tensor(nm, [DM, SEQ], BF16, kind="ExternalOutput")
        taps["V_d"] = nc.dram_tensor(
            "V_d", [SEQ, NH * VH], BF16, kind="ExternalOutput"
        )

    with tile.TileContext(nc) as tc:
        with (
            tc.tile_pool(name="persist", bufs=1) as persist,
            tc.tile_pool(name="pt", bufs=6) as pt_pool,
            tc.tile_pool(name="small", bufs=2) as small,
            tc.tile_pool(name="outst", bufs=2) as out_pool,
            tc.tile_pool(name="ps_st", bufs=2, space="PSUM") as ps_st,
            tc.tile_pool(name="ps_z", bufs=3, space="PSUM") as ps_z,
            tc.tile_pool(name="ps_mm", bufs=1, space="PSUM") as ps_mm,
        ):
            # ---- sync ring: ident, xT, WV, mask/ones.  scalar ring: wq/wk
            # (column-split so head pairs 0-1 unblock early), wo later ----
            ident = persist.tile([128, 128], BF16, tag="ident", name="ident")
            nc.sync.dma_start(out=ident, in_=identin[:, :])
            warm_ps = ps_mm.tile(
                [128, 128], F32, tag="proj", name="warm", padded_shape=[128, QC]
            )
            for _ in range(WARMUP):
                nc.tensor.matmul(warm_ps, lhsT=ident, rhs=ident, start=True, stop=True)

            # single sync ring for all inputs in priority order (the two
            # HWDGE rings share ~210GB/s of HBM read bandwidth, and DMAs on
            # the scalar ring block the ACT compute stream behind them).
            # Startup-critical set first: xt query-half 0, wq/wk cols for
            # head pairs 0-1, mask, WV.  xt half 1 and the remaining weight
            # columns are issued after the first projection chunks (below).
            xT = [
                persist.tile([128, SEQ], BF16, tag=f"xT{d}", name=f"xT{d}")
                for d in range(NDT)
            ]
            for d in range(NDT):
                nc.sync.dma_start(
                    out=xT[d][:, 0:QC], in_=xt[d * 128 : (d + 1) * 128, 0:QC]
                )

            WQ = [
                persist.tile([128, DM], BF16, tag=f"WQ{d}", name=f"WQ{d}")
                for d in range(NDT)
            ]
            WK = [
                persist.tile([128, DM], BF16, tag=f"WK{d}", name=f"WK{d}")
                for d in range(NDT)
            ]
            WV = [
                persist.tile([128, DM], BF16, tag=f"WV{d}", name=f"WV{d}")
                for d in range(NDT)
            ]
            # full-row weight DMAs: 1536B partition lines run ~3x the
            # bandwidth of 512B column-piece lines, and fewer DMAs clear
            # the ring sooner for WV / xt half 1
            for src, dst in ((wq, WQ), (wk, WK)):
                for d in range(NDT):
                    nc.sync.dma_start(out=dst[d], in_=src[d * 128 : (d + 1) * 128, :])

            wm_t = persist.tile([128, 256], BF16, tag="wmask", name="wmask")
            nc.sync.dma_start(out=wm_t, in_=wmask[:, :])

            for d in range(NDT):
                nc.sync.dma_start(out=WV[d], in_=wv[d * 128 : (d + 1) * 128, :])

            bias_tiles = {}
            if with_bq:
                t = persist.tile([128, NDT], F32, tag="bq", name="bq")
                nc.scalar.dma_start(out=t, in_=bq[:, :])
                bias_tiles["bq"] = t
            if with_bk:
                t = persist.tile([128, NDT], F32, tag="bk", name="bk")
                nc.scalar.dma_start(out=t, in_=bk[:, :])
                bias_tiles["bk"] = t
            if with_bv:
                t = persist.tile([128, DM], F32, tag="bv", name="bv")
                nc.scalar.dma_start(out=t, in_=bv[0:1, :].to_broadcast((128, DM)))
                bias_tiles["bv"] = t
            if with_bo:
                t = persist.tile([128, DM], F32, tag="bo", name="bo")
                nc.scalar.dma_start(out=t, in_=bo[0:1, :].to_broadcast((128, DM)))
                bias_tiles["bo"] = t

            QT = [
                persist.tile([128, SEQ], BF16, tag=f"QT{d}", name=f"QT{d}")
                for d in range(NDT)
            ]
            KT = [
                persist.tile([128, SEQ], BF16, tag=f"KT{d}", name=f"KT{d}")
                for d in range(NDT)
            ]
            V = [
                persist.tile([128, NH * VH], BF16, tag=f"V{s}", name=f"V{s}")
                for s in range(NQT)
            ]
            for s in range(NQT):
                vv = V[s].rearrange("p (h e) -> p h e", e=VH)
                nc.vector.memset(vv[:, :, DH : DH + 1], 1.0)
            ZT = [
                persist.tile([128, SEQ], BF16, tag=f"ZT{d}", name=f"ZT{d}")
                for d in range(NDT)
            ]

            # ---- projection chunks (each = one psum round trip) ----
            def qk_chunk(hp, which, c, pool, tag):
                W, dst, bkey = (
                    (WQ, QT, "bq") if which == "q" else (WK, KT, "bk")
                )
                acc = pool.tile(
                    [128, QC], F32, tag=tag, name="proj",
                    padded_shape=[128, 2 * QC] if tag == "st" else [128, QC],
                )
                for d in range(NDT):
                    nc.tensor.matmul(
                        acc,
                        lhsT=W[d][:, hp * 128 : (hp + 1) * 128],
                        rhs=xT[d][:, c * QC : (c + 1) * QC],
                        start=(d == 0),
                        stop=(d == NDT - 1),
                    )
                o = dst[hp][:, c * QC : (c + 1) * QC]
                if bkey in bias_tiles:
                    nc.vector.tensor_scalar_add(
                        o, acc, bias_tiles[bkey][:, hp : hp + 1]
                    )
                else:
                    nc.vector.tensor_copy(o, acc)

            def qk_chunks(hp):
                return [
                    (lambda which=which, c=c: qk_chunk(hp, which, c, ps_mm, "proj"))
                    for which in ("q", "k")
                    for c in range(NQC)
                ]

            NVC = 2
            VC = DM // NVC  # 384

            def v_chunk(s, c, on_act=True):
                acc = ps_st.tile(
                    [128, VC], F32, tag="st", name="vacc",
                    padded_shape=[128, 2 * QC],
                )
                for d in range(NDT):
                    nc.tensor.matmul(
                        acc,
                        lhsT=xT[d][:, s * 128 : (s + 1) * 128],
                        rhs=WV[d][:, c * VC : (c + 1) * VC],
                        start=(d == 0),
                        stop=(d == NDT - 1),
                    )
                nh2 = VC // DH  # heads per chunk (6)
                o = V[s].rearrange("p (h e) -> p h e", e=VH)[
                    :, c * nh2 : (c + 1) * nh2, 0:DH
                ]
                if "bv" in bias_tiles:
                    nc.vector.tensor_add(
                        o,
                        acc.rearrange("p (h e) -> p h e", e=DH),
                        bias_tiles["bv"][:, c * VC : (c + 1) * VC].rearrange(
                            "p (h e) -> p h e", e=DH
                        ),
                    )
                elif on_act:
                    nc.scalar.activation(
                        o, acc.rearrange("p (h e) -> p h e", e=DH), AF.Copy
                    )
                else:
                    nc.vector.tensor_copy(
                        o, acc.rearrange("p (h e) -> p h e", e=DH)
                    )

            def v_chunks(s):
                return [lambda c=c: v_chunk(s, c) for c in range(NVC)]

            WO = []

            def wo_load():
                # sync ring: idle mid-attention (scalar ring would block ACT)
                for d in range(NDT):
                    t = persist.tile([128, DM], BF16, tag=f"WO{d}", name=f"WO{d}")
                    nc.sync.dma_start(out=t, in_=wo[d * 128 : (d + 1) * 128, :])
                    WO.append(t)

            def o_chunks(s):
                ot = [None]

                def chunk(c):
                    if c == 0:
                        ot[0] = out_pool.tile([128, DM], F32, tag="ostage", name="ostage")
                    pool, tag = ((ps_mm, "proj"), (ps_z, "z"))[c % 2]
                    acc = pool.tile(
                        [128, VC], F32, tag=tag, name="oacc",
                        padded_shape=[128, QC],
                    )
                    for d in range(NDT):
                        nc.tensor.matmul(
                            acc,
                            lhsT=ZT[d][:, s * 128 : (s + 1) * 128],
                            rhs=WO[d][:, c * VC : (c + 1) * VC],
                            start=(d == 0),
                            stop=(d == NDT - 1),
                        )
                    o = ot[0][:, c * VC : (c + 1) * VC]
                    if "bo" in bias_tiles:
                        nc.vector.tensor_add(
                            o, acc, bias_tiles["bo"][:, c * VC : (c + 1) * VC]
                        )
                    else:
                        # DVE: the ACT stream is exp-saturated mid-phase and
                        # an in-order ACT copy would delay psum recycling
                        nc.vector.tensor_copy(o, acc)
                    if c == NVC - 1:
                        # rows 512+ drain at the very end: use the scalar
                        # ring (idle by then) so the tail DMAs overlap
                        eng = nc.sync if s < 4 else nc.scalar
                        eng.dma_start(
                            out=out[s * 128 : (s + 1) * 128, :], in_=ot[0]
                        )

                return [lambda c=c: chunk(c) for c in range(NVC)]

            # ---- pipelined attention sweep ----
            zps_of = {}

            def issue_scores(hp, c, g, gsz):
                doffs = [max(0, (g + j) * 128 - c * QC) for j in range(gsz)]
                sts = {}
                for px in (0, 64):
                    sts[px] = ps_st.tile([128, gsz * QC], F32, tag="st", name="st")
                for j in range(gsz):
                    kb = g + j
                    off = doffs[j]
                    for px in (0, 64):
                        nc.tensor.matmul(
                            sts[px][:, j * QC + off : (j + 1) * QC],
                            lhsT=KT[hp][px : px + 64, kb * 128 : (kb + 1) * 128],
                            rhs=QT[hp][px : px + 64, c * QC + off : (c + 1) * QC],
                            start=True,
                            stop=True,
                        )
                return sts, doffs

            def issue_expv(hp, c, g, gsz, sts, doffs, last):
                nkb = 4 * (c + 1)
                if g == 0:
                    zps_of[(hp, c)] = {
                        px: ps_z.tile([128, QC], F32, tag="z", name="z")
                        for px in (0, 64)
                    }
                zps = zps_of[(hp, c)]
                # pt layout: block j at column j*PTW (PTW=640) so one 3D
                # ragged exp [p, 2, QC-doff0] skips the fully-masked leading
                # columns of BOTH blocks (strides: sts 512, pt 640)
                pts = {}
                for px in (0, 64):
                    pt = pt_pool.tile([128, 2 * PTW], BF16, tag="pt", name="pt")
                    nc.scalar.activation(
                        pt.rearrange("p (j u) -> p j u", u=PTW)[
                            :, 0:gsz, doffs[0] : QC
                        ],
                        sts[px].rearrange("p (j q) -> p j q", q=QC)[
                            :, :, doffs[0] : QC
                        ],
                        AF.Exp,
                        scale=0.125,
                    )
                    pts[px] = pt
                # diagonal groups (both blocks straddle the diagonal): one
                # paired mask multiply per px covers both 128-wide triangles
                # via a stride-768 view (triangles sit at doff0 + 768*j)
                if g * 128 - c * QC >= 0:
                    base = doffs[0]
                    msk = wm_t[:, :].rearrange("p (a b) -> p a b", b=128)
                    for px in (0, 64):
                        blk = pts[px][:, base : base + 896].rearrange(
                            "p (a b) -> p a b", b=128
                        )[:, 0:7:6, :]
                        nc.vector.tensor_mul(blk, blk, msk)
                for j in range(gsz):
                    kb = g + j
                    off = doffs[j]
                    for px in (0, 64):
                        pt = pts[px]
                        h = 2 * hp + (1 if px else 0)
                        nc.tensor.matmul(
                            zps[px][0 : DH + 1, off:QC],
                            lhsT=V[kb][:, h * VH : h * VH + DH + 1],
                            rhs=pt[:, j * PTW + off : j * PTW + QC],
                            start=(kb == 0),
                            stop=(kb == nkb - 1),
                        )
                if last:
                    for px in (0, 64):
                        dstage = small.tile([128, QC], F32, tag="dstage", name="dstage")
                        nc.vector.tensor_copy(dstage[0:1, :], zps[px][DH : DH + 1, :])
                        recip = small.tile([128, QC], F32, tag="recip", name="recip")
                        nc.vector.reciprocal_approx_fast(recip[0:1, :], dstage[0:1, :])
                        bcast = small.tile([64, QC], F32, tag="bcast", name="bcast")
                        nc.gpsimd.partition_broadcast(bcast, recip[0:1, :])
                        nc.vector.tensor_mul(
                            ZT[hp][px : px + 64, c * QC : (c + 1) * QC],
                            zps[px][0:64, :],
                            bcast,
                        )
                    del zps_of[(hp, c)]

            # ---- pre-phase: project heads 0-1 for query half 0 only (the
            # qc=1 halves are computed as fillers much later), then issue
            # the non-critical DMAs, then V tiles 0-1 ----
            qk_chunk(0, "q", 0, ps_mm, "proj")
            qk_chunk(0, "k", 0, ps_st, "st")
            for d in range(NDT):
                nc.sync.dma_start(
                    out=xT[d][:, QC:SEQ], in_=xt[d * 128 : (d + 1) * 128, QC:SEQ]
                )


            # qc=0 and qc=1 units interleaved: spreads the exp-heavy qc=1
            # units (ACT-bound) across the whole span instead of
            # back-loading them.  Every unit gets filler chunks so exp
            # latency is always hidden behind interposed PE work.
            # the first four units need only the startup-critical DMA set
            # (xt + head-pair-0/1 weight columns); later head pairs' weight
            # columns stream in per-pair just ahead of first use
            units = [
                (0, 0), (1, 0), (0, 1), (1, 1), (2, 0), (3, 0),
                (2, 1), (4, 0), (5, 0), (3, 1), (4, 1), (5, 1),
            ]

            def vc(s, c, on_act=True):
                return lambda: v_chunk(s, c, on_act=on_act)

            def qkc(hp, which, c):
                return lambda: qk_chunk(hp, which, c, ps_mm, "proj")

            fillers = {
                # v0/v1 here (not pre-phase): their WV-gated matmuls must
                # not sit in the PE stream ahead of the first scores.
                # DVE copies so the first exps aren't queued behind them.
                0: [
                    lambda: v_chunk(0, 0, on_act=False),
                    lambda: v_chunk(0, 1, on_act=False),
                    lambda: v_chunk(1, 0, on_act=False),
                    lambda: v_chunk(1, 1, on_act=False),
                    qkc(1, "q", 0), qkc(1, "k", 0), vc(2, 0), vc(3, 0),
                ],
                1: [qkc(0, "q", 1), qkc(0, "k", 1), vc(4, 0), vc(5, 0)],
                2: [qkc(1, "q", 1), qkc(1, "k", 1), vc(6, 0), vc(7, 0)],
                # qc=1 v-copies on DVE: on ACT they delay the exp queued
                # behind them, stalling the next scores group's psum WAR
                # (the qc=0 ones stay on ACT — the early DVE stream gates
                # the QK copies there)
                3: [qkc(2, "q", 0), qkc(2, "k", 0),
                    vc(2, 1, on_act=False), vc(3, 1, on_act=False)],
                4: [qkc(3, "q", 0), qkc(3, "k", 0), vc(4, 1, on_act=False)],
                5: [qkc(2, "q", 1), qkc(2, "k", 1), vc(5, 1, on_act=False)],
                6: [qkc(4, "q", 0), qkc(4, "k", 0), vc(6, 1, on_act=False)],
                7: [qkc(5, "q", 0), qkc(5, "k", 0), vc(7, 1, on_act=False)],
                8: [qkc(3, "q", 1), qkc(3, "k", 1), wo_load],
                9: o_chunks(0) + [qkc(4, "q", 1), qkc(4, "k", 1)],
                10: o_chunks(1) + [qkc(5, "q", 1), qkc(5, "k", 1)],
                11: o_chunks(2) + o_chunks(3),
            }
            # units whose fillers read ZT written by the pending finalizer:
            # flush before popping fillers there (issue-order correctness)
            flush_first = {9, 10, 11}

            pending = [None]

            def flush():
                if pending[0] is not None:
                    fn = pending[0]
                    pending[0] = None
                    fn()

            for ui, (hp, c) in enumerate(units):
                nkb = 4 * (c + 1)
                glist = [(g, min(2, nkb - g)) for g in range(0, nkb, 2)]
                chunks = deque(fillers.get(ui, []))
                n = len(glist)
                for gi, (g, gsz) in enumerate(glist):
                    sts, doffs = issue_scores(hp, c, g, gsz)
                    if ui in flush_first:
                        flush()
                    k = -(-len(chunks) // (n - gi)) if chunks else 0
                    for i in range(k):
                        chunks.popleft()()
                        if i == 0:
                            flush()
                    if k == 0:
                        flush()
                    pending[0] = (
                        lambda hp=hp, c=c, g=g, gsz=gsz, sts=sts, doffs=doffs,
                        last=(gi == n - 1): issue_expv(hp, c, g, gsz, sts, doffs, last)
                    )
            flush()

            # ---- tail: output rows 512-1024.  Full-width accumulation on
            # the freed scores psum, one ACT copy (ACT is idle by now), out
            # DMAs alternating between the two rings.  s=4/5 accumulate
            # d=0..4 first so the PE is busy while the last unit's
            # normalization chain (recip/broadcast/ZT-mul for ZT[5]) runs.
            def o_full_mm(s, acc, drange, start):
                for lo, hi in ((0, QC), (QC, DM)):  # <=512 f32 cols per MM
                    for d in drange:
                        nc.tensor.matmul(
                            acc[:, lo:hi],
                            lhsT=ZT[d][:, s * 128 : (s + 1) * 128],
                            rhs=WO[d][:, lo:hi],
                            start=(start and d == drange[0]),
                            stop=(d == NDT - 1),
                        )

            def o_full_out(s, acc):
                ot = out_pool.tile([128, DM], F32, tag="ostage", name="ostage")
                if "bo" in bias_tiles:
                    nc.vector.tensor_add(ot, acc, bias_tiles["bo"])
                elif s % 2:  # alternate engines: the tail copies would
                    nc.scalar.activation(ot, acc, AF.Copy)  # serialize on ACT
                else:
                    nc.vector.tensor_copy(ot, acc)
                eng = nc.scalar if s % 2 else nc.sync
                eng.dma_start(out=out[s * 128 : (s + 1) * 128, :], in_=ot)

            accs = {}
            for s in (4, 5):
                accs[s] = ps_st.tile(
                    [128, DM], F32, tag="st", name="oacc",
                    padded_shape=[128, 2 * QC],
                )
                o_full_mm(s, accs[s], list(range(NDT - 1)), start=True)
            # s=6 partials on the proj/z psum rings: more PE work ahead of
            # the ZT[5] dependency
            acc6 = {}
            for cc, (pool, tag) in enumerate(((ps_mm, "proj"), (ps_z, "z"))):
                a = pool.tile(
                    [128, VC], F32, tag=tag, name="oacc", padded_shape=[128, QC]
                )
                for d in range(NDT - 1):
                    nc.tensor.matmul(
                        a,
                        lhsT=ZT[d][:, 6 * 128 : 7 * 128],
                        rhs=WO[d][:, cc * VC : (cc + 1) * VC],
                        start=(d == 0),
                        stop=False,
                    )
                acc6[cc] = a
            for s in (4, 5):
                o_full_mm(s, accs[s], [NDT - 1], start=False)
                o_full_out(s, accs[s])
            ot6 = out_pool.tile([128, DM], F32, tag="ostage", name="ostage")
            for cc in (0, 1):
                nc.tensor.matmul(
                    acc6[cc],
                    lhsT=ZT[NDT - 1][:, 6 * 128 : 7 * 128],
                    rhs=WO[NDT - 1][:, cc * VC : (cc + 1) * VC],
                    start=False,
                    stop=True,
                )
                o = ot6[:, cc * VC : (cc + 1) * VC]
                if "bo" in bias_tiles:
                    nc.vector.tensor_add(
                        o, acc6[cc], bias_tiles["bo"][:, cc * VC : (cc + 1) * VC]
                    )
                elif cc:
                    nc.scalar.activation(o, acc6[cc], AF.Copy)
                else:
                    nc.vector.tensor_copy(o, acc6[cc])
            nc.sync.dma_start(out=out[6 * 128 : 7 * 128, :], in_=ot6)
            acc = ps_st.tile(
                [128, DM], F32, tag="st", name="oacc",
                padded_shape=[128, 2 * QC],
            )
            o_full_mm(7, acc, list(range(NDT)), start=True)
            o_full_out(7, acc)

            if debug_taps:
                for nm, tiles in (("QT_d", QT), ("KT_d", KT), ("ZT_d", ZT)):
                    for d in range(NDT):
                        nc.sync.dma_start(
                            out=taps[nm][d * 128 : (d + 1) * 128, :],
                            in_=tiles[d][:, :],
                        )
                for s in range(NQT):
                    nc.sync.dma_start(
                        out=taps["V_d"][s * 128 : (s + 1) * 128, :], in_=V[s][:, :]
                    )

    nc.compile()
    return nc


_CACHE = {}


def _get_nc(key):
    if key not in _CACHE:
        _CACHE[key] = build(*key)
    return _CACHE[key]


def _prep(inputs):
    BF = ml_dtypes.bfloat16
    x = np.asarray(inputs["normalized_resid_pre"], np.float32)
    wq = np.ascontiguousarray(
        np.asarray(inputs["W_Q"], np.float32).transpose(1, 0, 2).reshape(DM, DM)
    ).astype(BF)
    wk = np.ascontiguousarray(
        np.asarray(inputs["W_K"], np.float32).transpose(1, 0, 2).reshape(DM, DM)
    ).astype(BF)
    wv = np.ascontiguousarray(
        np.asarray(inputs["W_V"], np.float32).transpose(1, 0, 2).reshape(DM, DM)
    ).astype(BF)
    wo = np.ascontiguousarray(
        np.asarray(inputs["W_O"], np.float32).reshape(DM, DM)
    ).astype(BF)
    bq = np.asarray(inputs["b_Q"], np.float32).reshape(NDT, 128).T
    bk = np.asarray(inputs["b_K"], np.float32).reshape(NDT, 128).T
    bv = np.asarray(inputs["b_V"], np.float32).reshape(1, DM)
    bo = np.asarray(inputs["b_O"], np.float32).reshape(1, DM)
    jj, uu = np.meshgrid(np.arange(128), np.arange(128), indexing="ij")
    wmask = np.tile((uu >= jj).astype(BF), (1, 2))
    key = (
        bool(np.any(bq)),
        bool(np.any(bk)),
        bool(np.any(bv)),
        bool(np.any(bo)),
    )
    common = {
        "wq": wq, "wk": wk, "wv": wv, "wo": wo, "wmask": wmask,
        "identin": np.eye(128, dtype=np.float32).astype(BF),
    }
    if key[0]:
        common["bq"] = np.ascontiguousarray(bq)
    if key[1]:
        common["bk"] = np.ascontiguousarray(bk)
    if key[2]:
        common["bv"] = np.ascontiguousarray(bv)
    if key[3]:
        common["bo"] = np.ascontiguousarray(bo)
    in_maps = [
        dict(common, xt=np.ascontiguousarray(x[b].T).astype(BF))
        for b in range(BATCH)
    ]
    return key, in_maps


def run(inputs, trace=False, **kw):
    key, in_maps = _prep(inputs)
    nc = _get_nc(key)
    res = run_bass_kernel_spmd(
        nc, in_maps, core_ids=list(range(BATCH)), trace=trace, **kw
    )
    outs = np.stack([res.results[b]["out"] for b in range(BATCH)])
    return outs.astype(np.float32), res


def kernel(**inputs):
    out, _ = run(inputs)
    return out


if __name__ == "__main__":
    rng = np.random.default_rng(0)
    ins = {
        "normalized_resid_pre": rng.standard_normal((8, SEQ, DM)).astype(np.float32),
        "W_Q": (0.02 * rng.standard_normal((NH, DM, DH))).astype(np.float32),
        "b_Q": np.zeros((NH, DH), np.float32),
        "W_K": (0.02 * rng.standard_normal((NH, DM, DH))).astype(np.float32),
        "b_K": np.zeros((NH, DH), np.float32),
        "W_V": (0.02 * rng.standard_normal((NH, DM, DH))).astype(np.float32),
        "b_V": np.zeros((NH, DH), np.float32),
        "W_O": (0.02 * rng.standard_normal((NH, DH, DM))).astype(np.float32),
        "b_O": np.zeros((DM,), np.float32),
    }
    out = kernel(**ins)
    print("kernel output", out.shape, out.dtype, float(np.abs(out).max()))


# revision 59
# speedup vs baseline: 1.2701x; 1.0339x over previous
"""Causal multi-head attention on 8 Trainium2 NeuronCores.

Problem: nn_Attention_46643344835180
  x: [8, 1024, 768], 12 heads x 64 dh, causal softmax attention + output proj.

Sharding: data-parallel over batch (8 batch elements -> 8 cores, no collectives).

v3: full bf16 compute (PSUM stays f32), host-side transpose of x (xT fed
directly), weights resident in SBUF, dual DMA rings (sync + scalar HWDGE),
QK/V psum->sbuf copies on the Scalar engine (idle during the projection
phase), and a globally software-pipelined attention sweep: scores of group
g+1 are issued before exp/PV of group g, with projection/output chunks
spread between groups as PE filler.

Per-core dataflow (batch element b):
  xT = x_b.T (host)                                                  [768, 1024]
  QT = Wq.T @ xT  (+bq)            heads stacked on partitions       [768, 1024]
  KT = Wk.T @ xT  (+bk)                                              [768, 1024]
  V  = x_b @ Wv   (+bv)            + interleaved ones column         [1024, 12*66]
  per head h, query-chunk qc (512):
    S^T[k,q] = KT_h.T @ QT_h          keys on partitions
    P^T = exp(S^T / 8)                ScalarE, batched over 2 key-blocks
    causal: one 128-wide-mask multiply per diagonal block
    z^T[65,512] += [V_h | 1].T @ P^T  row 64 accumulates the denominator
    ZT_h = z^T[0:64] * approx(1/z^T[64])   (denom -> reciprocal ->
           gpsimd partition_broadcast -> multiply)
  out = ZT.T @ Wo (+bo)                                              [1024, 768]
"""

import sys

sys.path.insert(0, "/opt/trn_rl_repo")

from collections import deque

import ml_dtypes
import numpy as np

import concourse.bass as bass
import concourse.mybir as mybir
import concourse.tile as tile
from concourse import bacc
from concourse.bass_utils import run_bass_kernel_spmd

F32 = mybir.dt.float32
BF16 = mybir.dt.bfloat16
AF = mybir.ActivationFunctionType

SEQ = 1024
DM = 768
NH = 12
DH = 64
VH = DH + 2  # V head stride: 64 dims + ones col + pad (keeps 4B alignment)
BATCH = 8
NQT = SEQ // 128  # 8 seq tiles of 128
NDT = DM // 128  # 6 d_model tiles
QC = 512  # query chunk (moving dim)
NQC = SEQ // QC  # 2
PTW = 640  # per-key-block stride inside a pt tile (= QC + diag width)
WARMUP = 88  # HAM warmup matmuls (bf16 N=128, ~110ns each cold): sized to
# bridge from ident-arrival (~9us) to the DMA-gated first projections
# (~19us) so the PE clock doesn't re-throttle before real work starts


def build(with_bq, with_bk, with_bv, with_bo, debug_taps=False):
    nc = bacc.Bacc("TRN2", target_bir_lowering=False, debug=False)

    xt = nc.dram_tensor("xt", [DM, SEQ], BF16, kind="ExternalInput")
    wq = nc.dram_tensor("wq", [DM, DM], BF16, kind="ExternalInput")
    wk = nc.dram_tensor("wk", [DM, DM], BF16, kind="ExternalInput")
    wv = nc.dram_tensor("wv", [DM, DM], BF16, kind="ExternalInput")
    wo = nc.dram_tensor("wo", [DM, DM], BF16, kind="ExternalInput")
    wmask = nc.dram_tensor("wmask", [128, 256], BF16, kind="ExternalInput")
    identin = nc.dram_tensor("identin", [128, 128], BF16, kind="ExternalInput")
    bq = bk = bv = bo = None
    if with_bq:
        bq = nc.dram_tensor("bq", [128, NDT], F32, kind="ExternalInput")
    if with_bk:
        bk = nc.dram_tensor("bk", [128, NDT], F32, kind="ExternalInput")
    if with_bv:
        bv = nc.dram_tensor("bv", [1, DM], F32, kind="ExternalInput")
    if with_bo:
        bo = nc.dram_tensor("bo", [1, DM], F32, kind="ExternalInput")
    out = nc.dram_tensor("out", [SEQ, DM], F32, kind="ExternalOutput")
    taps = {}
    if debug_taps:
        for nm in ("QT_d", "KT_d", "ZT_d"):
            taps[nm] = nc.dram_tensor(nm, [DM, SEQ], BF16, kind="ExternalOutput")
        taps["V_d"] = nc.dram_tensor(
            "V_d", [SEQ, NH * VH], BF16, kind="ExternalOutput"
        )

    with tile.TileContext(nc) as tc:
        with (
            tc.tile_pool(name="persist", bufs=1) as persist,
            tc.tile_pool(name="pt", bufs=6) as pt_pool,
            tc.tile_pool(name="small", bufs=2) as small,
            tc.tile_pool(name="outst", bufs=2) as out_pool,
            tc.tile_pool(name="ps_st", bufs=2, space="PSUM") as ps_st,
            tc.tile_pool(name="ps_z", bufs=3, space="PSUM") as ps_z,
            tc.tile_pool(name="ps_mm", bufs=1, space="PSUM") as ps_mm,
        ):
            # ---- sync ring: ident, xT, WV, mask/ones.  scalar ring: wq/wk
            # (column-split so head pairs 0-1 unblock early), wo later ----
            ident = persist.tile([128, 128], BF16, tag="ident", name="ident")
            nc.sync.dma_start(out=ident, in_=identin[:, :])
            warm_ps = ps_mm.tile(
                [128, 128], F32, tag="proj", name="warm", padded_shape=[128, QC]
            )
            for _ in range(WARMUP):
                nc.tensor.matmul(warm_ps, lhsT=ident, rhs=ident, start=True, stop=True)

            # single sync ring for all inputs in priority order (the two
            # HWDGE rings share ~210GB/s of HBM read bandwidth, and DMAs on
            # the scalar ring block the ACT compute stream behind them).
            # Startup-critical set first: xt query-half 0, wq/wk cols for
            # head pairs 0-1, mask, WV.  xt half 1 and the remaining weight
            # columns are issued after the first projection chunks (below).
            xT = [
                persist.tile([128, SEQ], BF16, tag=f"xT{d}", name=f"xT{d}")
                for d in range(NDT)
            ]
            for d in range(NDT):
                nc.sync.dma_start(
                    out=xT[d][:, 0:QC], in_=xt[d * 128 : (d + 1) * 128, 0:QC]
                )

            WQ = [
                persist.tile([128, DM], BF16, tag=f"WQ{d}", name=f"WQ{d}")
                for d in range(NDT)
            ]
            WK = [
                persist.tile([128, DM], BF16, tag=f"WK{d}", name=f"WK{d}")
                for d in range(NDT)
            ]
            WV = [
                persist.tile([128, DM], BF16, tag=f"WV{d}", name=f"WV{d}")
                for d in range(NDT)
            ]
            # full-row weight DMAs: 1536B partition lines run ~3x the
            # bandwidth of 512B column-piece lines, and fewer DMAs clear
            # the ring sooner for WV / xt half 1
            for src, dst in ((wq, WQ), (wk, WK)):
                for d in range(NDT):
                    nc.sync.dma_start(out=dst[d], in_=src[d * 128 : (d + 1) * 128, :])

            wm_t = persist.tile([128, 256], BF16, tag="wmask", name="wmask")
            nc.sync.dma_start(out=wm_t, in_=wmask[:, :])

            for d in range(NDT):
                nc.sync.dma_start(out=WV[d], in_=wv[d * 128 : (d + 1) * 128, :])

            bias_tiles = {}
            if with_bq:
                t = persist.tile([128, NDT], F32, tag="bq", name="bq")
                nc.scalar.dma_start(out=t, in_=bq[:, :])
                bias_tiles["bq"] = t
            if with_bk:
                t = persist.tile([128, NDT], F32, tag="bk", name="bk")
                nc.scalar.dma_start(out=t, in_=bk[:, :])
                bias_tiles["bk"] = t
            if with_bv:
                t = persist.tile([128, DM], F32, tag="bv", name="bv")
                nc.scalar.dma_start(out=t, in_=bv[0:1, :].to_broadcast((128, DM)))
                bias_tiles["bv"] = t
            if with_bo:
                t = persist.tile([128, DM], F32, tag="bo", name="bo")
                nc.scalar.dma_start(out=t, in_=bo[0:1, :].to_broadcast((128, DM)))
                bias_tiles["bo"] = t

            QT = [
                persist.tile([128, SEQ], BF16, tag=f"QT{d}", name=f"QT{d}")
                for d in range(NDT)
            ]
            KT = [
                persist.tile([128, SEQ], BF16, tag=f"KT{d}", name=f"KT{d}")
                for d in range(NDT)
            ]
            V = [
                persist.tile([128, NH * VH], BF16, tag=f"V{s}", name=f"V{s}")
                for s in range(NQT)
            ]
            for s in range(NQT):
                vv = V[s].rearrange("p (h e) -> p h e", e=VH)
                nc.vector.memset(vv[:, :, DH : DH + 1], 1.0)
            ZT = [
                persist.tile([128, SEQ], BF16, tag=f"ZT{d}", name=f"ZT{d}")
                for d in range(NDT)
            ]

            # ---- projection chunks (each = one psum round trip) ----
            def qk_chunk(hp, which, c, pool, tag):
                W, dst, bkey = (
                    (WQ, QT, "bq") if which == "q" else (WK, KT, "bk")
                )
                acc = pool.tile(
                    [128, QC], F32, tag=tag, name="proj",
                    padded_shape=[128, 2 * QC] if tag == "st" else [128, QC],
                )
                for d in range(NDT):
                    nc.tensor.matmul(
                        acc,
                        lhsT=W[d][:, hp * 128 : (hp + 1) * 128],
                        rhs=xT[d][:, c * QC : (c + 1) * QC],
                        start=(d == 0),
                        stop=(d == NDT - 1),
                    )
                o = dst[hp][:, c * QC : (c + 1) * QC]
                if bkey in bias_tiles:
                    nc.vector.tensor_scalar_add(
                        o, acc, bias_tiles[bkey][:, hp : hp + 1]
                    )
                else:
                    nc.vector.tensor_copy(o, acc)

            def qk_chunks(hp):
                return [
                    (lambda which=which, c=c: qk_chunk(hp, which, c, ps_mm, "proj"))
                    for which in ("q", "k")
                    for c in range(NQC)
                ]

            NVC = 2
            VC = DM // NVC  # 384

            def v_chunk(s, c, on_act=True):
                acc = ps_st.tile(
                    [128, VC], F32, tag="st", name="vacc",
                    padded_shape=[128, 2 * QC],
                )
                for d in range(NDT):
                    nc.tensor.matmul(
                        acc,
                        lhsT=xT[d][:, s * 128 : (s + 1) * 128],
                        rhs=WV[d][:, c * VC : (c + 1) * VC],
                        start=(d == 0),
                        stop=(d == NDT - 1),
                    )
                nh2 = VC // DH  # heads per chunk (6)
                o = V[s].rearrange("p (h e) -> p h e", e=VH)[
                    :, c * nh2 : (c + 1) * nh2, 0:DH
                ]
                if "bv" in bias_tiles:
                    nc.vector.tensor_add(
                        o,
                        acc.rearrange("p (h e) -> p h e", e=DH),
                        bias_tiles["bv"][:, c * VC : (c + 1) * VC].rearrange(
                            "p (h e) -> p h e", e=DH
                        ),
                    )
                elif on_act:
                    nc.scalar.activation(
                        o, acc.rearrange("p (h e) -> p h e", e=DH), AF.Copy
                    )
                else:
                    nc.vector.tensor_copy(
                        o, acc.rearrange("p (h e) -> p h e", e=DH)
                    )

            def v_chunks(s):
                return [lambda c=c: v_chunk(s, c) for c in range(NVC)]

            WO = []

            def wo_load():
                # sync ring: idle mid-attention (scalar ring would block ACT)
                for d in range(NDT):
                    t = persist.tile([128, DM], BF16, tag=f"WO{d}", name=f"WO{d}")
                    nc.sync.dma_start(out=t, in_=wo[d * 128 : (d + 1) * 128, :])
                    WO.append(t)

            def o_chunks(s):
                ot = [None]

                def chunk(c):
                    if c == 0:
                        ot[0] = out_pool.tile([128, DM], F32, tag="ostage", name="ostage")
                    pool, tag = ((ps_mm, "proj"), (ps_z, "z"))[c % 2]
                    acc = pool.tile(
                        [128, VC], F32, tag=tag, name="oacc",
                        padded_shape=[128, QC],
                    )
                    for d in range(NDT):
                        nc.tensor.matmul(
                            acc,
                            lhsT=ZT[d][:, s * 128 : (s + 1) * 128],
                            rhs=WO[d][:, c * VC : (c + 1) * VC],
                            start=(d == 0),
                            stop=(d == NDT - 1),
                        )
                    o = ot[0][:, c * VC : (c + 1) * VC]
                    if "bo" in bias_tiles:
                        nc.vector.tensor_add(
                            o, acc, bias_tiles["bo"][:, c * VC : (c + 1) * VC]
                        )
                    else:
                        # DVE: the ACT stream is exp-saturated mid-phase and
                        # an in-order ACT copy would delay psum recycling
                        nc.vector.tensor_copy(o, acc)
                    if c == NVC - 1:
                        # rows 512+ drain at the very end: use the scalar
                        # ring (idle by then) so the tail DMAs overlap
                        eng = nc.sync if s < 4 else nc.scalar
                        eng.dma_start(
                            out=out[s * 128 : (s + 1) * 128, :], in_=ot[0]
                        )

                return [lambda c=c: chunk(c) for c in range(NVC)]

            # ---- pipelined attention sweep ----
            zps_of = {}

            def issue_scores(hp, c, g, gsz):
                doffs = [max(0, (g + j) * 128 - c * QC) for j in range(gsz)]
                sts = {}
                for px in (0, 64):
                    sts[px] = ps_st.tile([128, gsz * QC], F32, tag="st", name="st")
                for j in range(gsz):
                    kb = g + j
                    off = doffs[j]
                    for px in (0, 64):
                        nc.tensor.matmul(
                            sts[px][:, j * QC + off : (j + 1) * QC],
                            lhsT=KT[hp][px : px + 64, kb * 128 : (kb + 1) * 128],
                            rhs=QT[hp][px : px + 64, c * QC + off : (c + 1) * QC],
                            start=True,
                            stop=True,
                        )
                return sts, doffs

            def issue_expv(hp, c, g, gsz, sts, doffs, last):
                nkb = 4 * (c + 1)
                if g == 0:
                    zps_of[(hp, c)] = {
                        px: ps_z.tile([128, QC], F32, tag="z", name="z")
                        for px in (0, 64)
                    }
                zps = zps_of[(hp, c)]
                # pt layout: block j at column j*PTW (PTW=640) so one 3D
                # ragged exp [p, 2, QC-doff0] skips the fully-masked leading
                # columns of BOTH blocks (strides: sts 512, pt 640)
                pts = {}
                for px in (0, 64):
                    pt = pt_pool.tile([128, 2 * PTW], BF16, tag="pt", name="pt")
                    nc.scalar.activation(
                        pt.rearrange("p (j u) -> p j u", u=PTW)[
                            :, 0:gsz, doffs[0] : QC
                        ],
                        sts[px].rearrange("p (j q) -> p j q", q=QC)[
                            :, :, doffs[0] : QC
                        ],
                        AF.Exp,
                        scale=0.125,
                    )
                    pts[px] = pt
                # diagonal groups (both blocks straddle the diagonal): one
                # paired mask multiply per px covers both 128-wide triangles
                # via a stride-768 view (triangles sit at doff0 + 768*j)
                if g * 128 - c * QC >= 0:
                    base = doffs[0]
                    msk = wm_t[:, :].rearrange("p (a b) -> p a b", b=128)
                    for px in (0, 64):
                        blk = pts[px][:, base : base + 896].rearrange(
                            "p (a b) -> p a b", b=128
                        )[:, 0:7:6, :]
                        nc.vector.tensor_mul(blk, blk, msk)
                for j in range(gsz):
                    kb = g + j
                    off = doffs[j]
                    for px in (0, 64):
                        pt = pts[px]
                        h = 2 * hp + (1 if px else 0)
                        nc.tensor.matmul(
                            zps[px][0 : DH + 1, off:QC],
                            lhsT=V[kb][:, h * VH : h * VH + DH + 1],
                            rhs=pt[:, j * PTW + off : j * PTW + QC],
                            start=(kb == 0),
                            stop=(kb == nkb - 1),
                        )
                if last:
                    for px in (0, 64):
                        dstage = small.tile([128, QC], F32, tag="dstage", name="dstage")
                        nc.vector.tensor_copy(dstage[0:1, :], zps[px][DH : DH + 1, :])
                        recip = small.tile([128, QC], F32, tag="recip", name="recip")
                        nc.vector.reciprocal_approx_fast(recip[0:1, :], dstage[0:1, :])
                        bcast = small.tile([64, QC], F32, tag="bcast", name="bcast")
                        nc.gpsimd.partition_broadcast(bcast, recip[0:1, :])
                        nc.vector.tensor_mul(
                            ZT[hp][px : px + 64, c * QC : (c + 1) * QC],
                            zps[px][0:64, :],
                            bcast,
                        )
                    del zps_of[(hp, c)]

            # ---- pre-phase: project heads 0-1 for query half 0 only (the
            # qc=1 halves are computed as fillers much later), then issue
            # the non-critical DMAs, then V tiles 0-1 ----
            qk_chunk(0, "q", 0, ps_mm, "proj")
            qk_chunk(0, "k", 0, ps_st, "st")
            for d in range(NDT):
                nc.sync.dma_start(
                    out=xT[d][:, QC:SEQ], in_=xt[d * 128 : (d + 1) * 128, QC:SEQ]
                )


            # qc=0 and qc=1 units interleaved: spreads the exp-heavy qc=1
            # units (ACT-bound) across the whole span instead of
            # back-loading them.  Every unit gets filler chunks so exp
            # latency is always hidden behind interposed PE work.
            # the first four units need only the startup-critical DMA set
            # (xt + head-pair-0/1 weight columns); later head pairs' weight
            # columns stream in per-pair just ahead of first use
            units = [
                (0, 0), (1, 0), (0, 1), (1, 1), (2, 0), (3, 0),
                (2, 1), (4, 0), (5, 0), (3, 1), (4, 1), (5, 1),
            ]

            def vc(s, c):
                return lambda: v_chunk(s, c)

            def qkc(hp, which, c):
                return lambda: qk_chunk(hp, which, c, ps_mm, "proj")

            fillers = {
                # v0/v1 here (not pre-phase): their WV-gated matmuls must
                # not sit in the PE stream ahead of the first scores.
                # DVE copies so the first exps aren't queued behind them.
                0: [
                    lambda: v_chunk(0, 0, on_act=False),
                    lambda: v_chunk(0, 1, on_act=False),
                    lambda: v_chunk(1, 0, on_act=False),
                    lambda: v_chunk(1, 1, on_act=False),
                    qkc(1, "q", 0), qkc(1, "k", 0), vc(2, 0), vc(3, 0),
                ],
                1: [qkc(0, "q", 1), qkc(0, "k", 1), vc(4, 0), vc(5, 0)],
                2: [qkc(1, "q", 1), qkc(1, "k", 1), vc(6, 0), vc(7, 0)],
                3: [qkc(2, "q", 0), qkc(2, "k", 0), vc(2, 1), vc(3, 1)],
                4: [qkc(3, "q", 0), qkc(3, "k", 0), vc(4, 1)],
                5: [qkc(2, "q", 1), qkc(2, "k", 1), vc(5, 1)],
                6: [qkc(4, "q", 0), qkc(4, "k", 0), vc(6, 1)],
                7: [qkc(5, "q", 0), qkc(5, "k", 0), vc(7, 1)],
                8: [qkc(3, "q", 1), qkc(3, "k", 1), wo_load],
                9: o_chunks(0) + [qkc(4, "q", 1), qkc(4, "k", 1)],
                10: o_chunks(1) + [qkc(5, "q", 1), qkc(5, "k", 1)],
                11: o_chunks(2) + o_chunks(3),
            }
            # units whose fillers read ZT written by the pending finalizer:
            # flush before popping fillers there (issue-order correctness)
            flush_first = {9, 10, 11}

            pending = [None]

            def flush():
                if pending[0] is not None:
                    fn = pending[0]
                    pending[0] = None
                    fn()

            for ui, (hp, c) in enumerate(units):
                nkb = 4 * (c + 1)
                glist = [(g, min(2, nkb - g)) for g in range(0, nkb, 2)]
                chunks = deque(fillers.get(ui, []))
                n = len(glist)
                for gi, (g, gsz) in enumerate(glist):
                    sts, doffs = issue_scores(hp, c, g, gsz)
                    if ui in flush_first:
                        flush()
                    k = -(-len(chunks) // (n - gi)) if chunks else 0
                    for i in range(k):
                        chunks.popleft()()
                        if i == 0:
                            flush()
                    if k == 0:
                        flush()
                    pending[0] = (
                        lambda hp=hp, c=c, g=g, gsz=gsz, sts=sts, doffs=doffs,
                        last=(gi == n - 1): issue_expv(hp, c, g, gsz, sts, doffs, last)
                    )
            flush()

            # ---- tail: output rows 512-1024.  Full-width accumulation on
            # the freed scores psum, one ACT copy (ACT is idle by now), out
            # DMAs alternating between the two rings.  s=4/5 accumulate
            # d=0..4 first so the PE is busy while the last unit's
            # normalization chain (recip/broadcast/ZT-mul for ZT[5]) runs.
            def o_full_mm(s, acc, drange, start):
                for lo, hi in ((0, QC), (QC, DM)):  # <=512 f32 cols per MM
                    for d in drange:
                        nc.tensor.matmul(
                            acc[:, lo:hi],
                            lhsT=ZT[d][:, s * 128 : (s + 1) * 128],
                            rhs=WO[d][:, lo:hi],
                            start=(start and d == drange[0]),
                            stop=(d == NDT - 1),
                        )

            def o_full_out(s, acc):
                ot = out_pool.tile([128, DM], F32, tag="ostage", name="ostage")
                if "bo" in bias_tiles:
                    nc.vector.tensor_add(ot, acc, bias_tiles["bo"])
                elif s % 2:  # alternate engines: the tail copies would
                    nc.scalar.activation(ot, acc, AF.Copy)  # serialize on ACT
                else:
                    nc.vector.tensor_copy(ot, acc)
                eng = nc.scalar if s % 2 else nc.sync
                eng.dma_start(out=out[s * 128 : (s + 1) * 128, :], in_=ot)

            accs = {}
            for s in (4, 5):
                accs[s] = ps_st.tile(
                    [128, DM], F32, tag="st", name="oacc",
                    padded_shape=[128, 2 * QC],
                )
                o_full_mm(s, accs[s], list(range(NDT - 1)), start=True)
            # s=6 partials on the proj/z psum rings: more PE work ahead of
            # the ZT[5] dependency
            acc6 = {}
            for cc, (pool, tag) in enumerate(((ps_mm, "proj"), (ps_z, "z"))):
                a = pool.tile(
                    [128, VC], F32, tag=tag, name="oacc", padded_shape=[128, QC]
                )
                for d in range(NDT - 1):
                    nc.tensor.matmul(
                        a,
                        lhsT=ZT[d][:, 6 * 128 : 7 * 128],
                        rhs=WO[d][:, cc * VC : (cc + 1) * VC],
                        start=(d == 0),
                        stop=False,
                    )
                acc6[cc] = a
            for s in (4, 5):
                o_full_mm(s, accs[s], [NDT - 1], start=False)
                o_full_out(s, accs[s])
            ot6 = out_pool.tile([128, DM], F32, tag="ostage", name="ostage")
            for cc in (0, 1):
                nc.tensor.matmul(
                    acc6[cc],
                    lhsT=ZT[NDT - 1][:, 6 * 128 : 7 * 128],
                    rhs=WO[NDT - 1][:, cc * VC : (cc + 1) * VC],
                    start=False,
                    stop=True,
                )
                o = ot6[:, cc * VC : (cc + 1) * VC]
                if "bo" in bias_tiles:
                    nc.vector.tensor_add(
                        o, acc6[cc], bias_tiles["bo"][:, cc * VC : (cc + 1) * VC]
                    )
                elif cc:
                    nc.scalar.activation(o, acc6[cc], AF.Copy)
                else:
                    nc.vector.tensor_copy(o, acc6[cc])
            nc.sync.dma_start(out=out[6 * 128 : 7 * 128, :], in_=ot6)
            acc = ps_st.tile(
                [128, DM], F32, tag="st", name="oacc",
                padded_shape=[128, 2 * QC],
            )
            o_full_mm(7, acc, list(range(NDT)), start=True)
            o_full_out(7, acc)

            if debug_taps:
                for nm, tiles in (("QT_d", QT), ("KT_d", KT), ("ZT_d", ZT)):
                    for d in range(NDT):
                        nc.sync.dma_start(
                            out=taps[nm][d * 128 : (d + 1) * 128, :],
                            in_=tiles[d][:, :],
                        )
                for s in range(NQT):
                    nc.sync.dma_start(
                        out=taps["V_d"][s * 128 : (s + 1) * 128, :], in_=V[s][:, :]
                    )

    nc.compile()
    return nc


_CACHE = {}


def _get_nc(key):
    if key not in _CACHE:
        _CACHE[key] = build(*key)
    return _CACHE[key]


def _prep(inputs):
    BF = ml_dtypes.bfloat16
    x = np.asarray(inputs["normalized_resid_pre"], np.float32)
    wq = np.ascontiguousarray(
        np.asarray(inputs["W_Q"], np.float32).transpose(1, 0, 2).reshape(DM, DM)
    ).astype(BF)
    wk = np.ascontiguousarray(
        np.asarray(inputs["W_K"], np.float32).transpose(1, 0, 2).reshape(DM, DM)
    ).astype(BF)
    wv = np.ascontiguousarray(
        np.asarray(inputs["W_V"], np.float32).transpose(1, 0, 2).reshape(DM, DM)
    ).astype(BF)
    wo = np.ascontiguousarray(
        np.asarray(inputs["W_O"], np.float32).reshape(DM, DM)
    ).astype(BF)
    bq = np.asarray(inputs["b_Q"], np.float32).reshape(NDT, 128).T
    bk = np.asarray(inputs["b_K"], np.float32).reshape(NDT, 128).T
    bv = np.asarray(inputs["b_V"], np.float32).reshape(1, DM)
    bo = np.asarray(inputs["b_O"], np.float32).reshape(1, DM)
    jj, uu = np.meshgrid(np.arange(128), np.arange(128), indexing="ij")
    wmask = np.tile((uu >= jj).astype(BF), (1, 2))
    key = (
        bool(np.any(bq)),
        bool(np.any(bk)),
        bool(np.any(bv)),
        bool(np.any(bo)),
    )
    common = {
        "wq": wq, "wk": wk, "wv": wv, "wo": wo, "wmask": wmask,
        "identin": np.eye(128, dtype=np.float32).astype(BF),
    }
    if key[0]:
        common["bq"] = np.ascontiguousarray(bq)
    if key[1]:
        common["bk"] = np.ascontiguousarray(bk)
    if key[2]:
        common["bv"] = np.ascontiguousarray(bv)
    if key[3]:
        common["bo"] = np.ascontiguousarray(bo)
    in_maps = [
        dict(common, xt=np.ascontiguousarray(x[b].T).astype(BF))
        for b in range(BATCH)
    ]
    return key, in_maps


def run(inputs, trace=False, **kw):
    key, in_maps = _prep(inputs)
    nc = _get_nc(key)
    res = run_bass_kernel_spmd(
        nc, in_maps, core_ids=list(range(BATCH)), trace=trace, **kw
    )
    outs = np.stack([res.results[b]["out"] for b in range(BATCH)])
    return outs.astype(np.float32), res


def kernel(**inputs):
    out, _ = run(inputs)
    return out


if __name__ == "__main__":
    rng = np.random.default_rng(0)
    ins = {
        "normalized_resid_pre": rng.standard_normal((8, SEQ, DM)).astype(np.float32),
        "W_Q": (0.02 * rng.standard_normal((NH, DM, DH))).astype(np.float32),
        "b_Q": np.zeros((NH, DH), np.float32),
        "W_K": (0.02 * rng.standard_normal((NH, DM, DH))).astype(np.float32),
        "b_K": np.zeros((NH, DH), np.float32),
        "W_V": (0.02 * rng.standard_normal((NH, DM, DH))).astype(np.float32),
        "b_V": np.zeros((NH, DH), np.float32),
        "W_O": (0.02 * rng.standard_normal((NH, DH, DM))).astype(np.float32),
        "b_O": np.zeros((DM,), np.float32),
    }
    out = kernel(**ins)
    print("kernel output", out.shape, out.dtype, float(np.abs(out).max()))
